# revision 7
# baseline (speedup 1.0000x reference)
"""Trainium2 Bass kernel for nn_BlockV2 (conv -> LN -> minGRU -> MLP x4).

Strategy: data-parallel over batch (B=8 -> 8 cores). Per core, activations
are kept in [D_partitions, T_free] layout and streamed through each layer in
chunks of 512 tokens; inter-layer activations ping-pong through DRAM.
The minGRU recurrence h_t = c_t*h_{t-1} + v_t runs on the VectorE
tensor_tensor_scan instruction (fp32 state), chained across chunks.
Matmul inputs are bf16 (fp32 PSUM accumulate); everything on the
LN/scan/residual path stays fp32 (the late-layer signal is a ~5e-3
variation on an O(1) baseline, which bf16 storage would destroy).
LayerNorm is two-pass (center, then variance of centered values) to avoid
E[x^2]-mu^2 cancellation. Emission is software-pipelined: chunk c+1's
LN/MLP matmuls interleave with chunk c's conv/GRU tail so TensorE never
idles long enough to re-throttle (HAM).
"""
import sys

sys.path.insert(0, "/opt/trn_rl_repo")

from contextlib import ExitStack

import numpy as np
import ml_dtypes

import concourse.bass as bass
import concourse.tile as tile
from concourse import bacc, mybir

f32 = mybir.dt.float32
bf16 = mybir.dt.bfloat16
Alu = mybir.AluOpType
Act = mybir.ActivationFunctionType
BF = ml_dtypes.bfloat16

B, D, L, K, H = 8, 512, 4, 4, 2048
N_CORES = 8
LN_EPS = 1e-5
P = 128


def build_nc(T=4096, CH=512, has_lnb=False, stats_fp32=True, q_fp32=False,
             rstd_recip=False):
    NCH = T // CH
    DT = D // P      # 4 d-tiles
    HT = H // P      # 16 h-tiles
    E2 = 2 * D
    MT2 = E2 // P    # 8 m-tiles of the kh matmul

    nc = bacc.Bacc("TRN2", target_bir_lowering=False, debug=False)

    xT = nc.dram_tensor("xT", [D, T + 3], f32, kind="ExternalInput")
    fwT = nc.dram_tensor("fwT", [L, D, E2], bf16, kind="ExternalInput")
    pwT = nc.dram_tensor("pwT", [L, D, D], bf16, kind="ExternalInput")
    w1T = nc.dram_tensor("w1T", [L, D, H], bf16, kind="ExternalInput")
    w2T = nc.dram_tensor("w2T", [L, H, D], bf16, kind="ExternalInput")
    dwK = nc.dram_tensor("dwK", [L, D, K], f32, kind="ExternalInput")
    dwb = nc.dram_tensor("dwb", [L, D], f32, kind="ExternalInput")
    pwb = nc.dram_tensor("pwb", [L, D], f32, kind="ExternalInput")
    b1v = nc.dram_tensor("b1v", [L, H], f32, kind="ExternalInput")
    b2v = nc.dram_tensor("b2v", [L, D], f32, kind="ExternalInput")
    lng = nc.dram_tensor("lng", [L + 1, D], f32, kind="ExternalInput")
    lnb = nc.dram_tensor("lnb", [L + 1, D], f32, kind="ExternalInput")
    out_t = nc.dram_tensor("out", [D, T], f32, kind="ExternalOutput")
    xs = [nc.dram_tensor(f"xs{i}", [D, T], f32) for i in range(2)]

    def dram3(tensor, c, width):
        return tensor.ap().rearrange("(dt p) t -> p dt t", p=P)[:, :, c * CH: c * CH + width]

    with tile.TileContext(nc) as tc, ExitStack() as ctx:
        sing = ctx.enter_context(tc.tile_pool(name="sing", bufs=1))
        wpool = ctx.enter_context(tc.tile_pool(name="w", bufs=1))
        big = ctx.enter_context(tc.tile_pool(name="big", bufs=10))
        small = ctx.enter_context(tc.tile_pool(name="small", bufs=8))
        hidp = ctx.enter_context(tc.tile_pool(name="hid", bufs=2))
        statp = ctx.enter_context(tc.tile_pool(name="stat", bufs=3))
        psmm = ctx.enter_context(tc.tile_pool(name="psmm", bufs=4, space="PSUM"))
        psst = ctx.enter_context(tc.tile_pool(name="psst", bufs=2, space="PSUM"))
        psbc = ctx.enter_context(tc.tile_pool(name="psbc", bufs=2, space="PSUM"))

        ones_col = sing.tile([P, 1], bf16)
        nc.vector.memset(ones_col, 1.0)
        ones_colf = sing.tile([P, 1], f32)
        nc.vector.memset(ones_colf, 1.0)
        ones_row = sing.tile([1, P], f32)
        nc.vector.memset(ones_row, 1.0)
        eps1 = sing.tile([1, 1], f32)
        nc.vector.memset(eps1, LN_EPS)
        dw_sb = sing.tile([P, L * DT, K], f32)
        nc.sync.dma_start(out=dw_sb, in_=dwK.ap().rearrange("l (dt p) k -> p (l dt) k", p=P))
        dwb_sb = sing.tile([P, L * DT], f32)
        nc.sync.dma_start(out=dwb_sb, in_=dwb.ap().rearrange("l (dt p) -> p (l dt)", p=P))
        pwb_sb = sing.tile([P, L * DT], f32)
        nc.sync.dma_start(out=pwb_sb, in_=pwb.ap().rearrange("l (dt p) -> p (l dt)", p=P))
        b1_sb = sing.tile([P, L * HT], f32)
        nc.sync.dma_start(out=b1_sb, in_=b1v.ap().rearrange("l (ht p) -> p (l ht)", p=P))
        b2_sb = sing.tile([P, L * DT], f32)
        nc.sync.dma_start(out=b2_sb, in_=b2v.ap().rearrange("l (dt p) -> p (l dt)", p=P))
        lng_sb = sing.tile([P, (L + 1) * DT], f32)
        nc.sync.dma_start(out=lng_sb, in_=lng.ap().rearrange("l (dt p) -> p (l dt)", p=P))
        lnb_sb = sing.tile([P, (L + 1) * DT], f32)
        nc.sync.dma_start(out=lnb_sb, in_=lnb.ap().rearrange("l (dt p) -> p (l dt)", p=P))

        def load_w(kind, dram, l, shape):
            t = wpool.tile(shape, bf16, tag=kind, name=f"{kind}{l}")
            nc.sync.dma_start(out=t, in_=dram.ap()[l].rearrange("(kt p) e -> p kt e", p=P))
            return t

        def layernorm_chunk(x_tile, slot, out_bf16):
            """x_tile: [P, DT, CH] f32; centers x_tile IN PLACE; returns LN out."""
            S_ps = psst.tile([1, CH], f32, tag="ps_stat", name="S_ps")
            if stats_fp32:
                for kt in range(DT):
                    nc.tensor.matmul(S_ps[:, :], ones_colf[:, :], x_tile[:, kt, :],
                                     start=(kt == 0), stop=(kt == DT - 1))
            else:
                x_bf = small.tile([P, DT, CH], bf16, tag="small", name="x_bf")
                for d in range(DT):
                    nc.scalar.activation(out=x_bf[:, d, :], in_=x_tile[:, d, :], func=Act.Copy)
                for kt in range(DT):
                    nc.tensor.matmul(S_ps[:, :], ones_col[:, :], x_bf[:, kt, :],
                                     start=(kt == 0), stop=(kt == DT - 1))
            S_sb = statp.tile([1, CH], f32, tag="stat", name="S_sb")
            nc.vector.tensor_copy(out=S_sb[:, :], in_=S_ps[:, :])
            muB = psbc.tile([P, CH], f32, tag="ps_bc", name="muB")
            nc.tensor.matmul(muB[:, :], ones_row[:, :], S_sb[:, :], start=True, stop=True)
            for d in range(DT):
                nc.vector.scalar_tensor_tensor(
                    x_tile[:, d, :], muB[:, :], -1.0 / D, x_tile[:, d, :], Alu.mult, Alu.add)
            Q_ps = psst.tile([1, CH], f32, tag="ps_stat", name="Q_ps")
            if q_fp32:
                xsq = big.tile([P, DT, CH], f32, tag="big", name="xsq")
                for d in range(DT):
                    nc.vector.tensor_mul(xsq[:, d, :], x_tile[:, d, :], x_tile[:, d, :])
                for kt in range(DT):
                    nc.tensor.matmul(Q_ps[:, :], ones_colf[:, :], xsq[:, kt, :],
                                     start=(kt == 0), stop=(kt == DT - 1))
            else:
                xsq = small.tile([P, DT, CH], bf16, tag="small", name="xsq")
                for d in range(DT):
                    nc.vector.tensor_mul(xsq[:, d, :], x_tile[:, d, :], x_tile[:, d, :])
                for kt in range(DT):
                    nc.tensor.matmul(Q_ps[:, :], ones_col[:, :], xsq[:, kt, :],
                                     start=(kt == 0), stop=(kt == DT - 1))
            rstd = statp.tile([1, CH], f32, tag="stat", name="rstd")
            if rstd_recip:
                sd = statp.tile([1, CH], f32, tag="stat", name="sd")
                nc.scalar.activation(out=sd[:, :], in_=Q_ps[:, :], func=Act.Sqrt,
                                     bias=eps1[:, :], scale=1.0 / D)
                nc.vector.reciprocal(out=rstd[:, :], in_=sd[:, :])
            else:
                lnv = statp.tile([1, CH], f32, tag="stat", name="lnv")
                nc.scalar.activation(out=lnv[:, :], in_=Q_ps[:, :], func=Act.Ln,
                                     bias=eps1[:, :], scale=1.0 / D)
                nc.scalar.activation(out=rstd[:, :], in_=lnv[:, :], func=Act.Exp, scale=-0.5)
            rstdB = psbc.tile([P, CH], f32, tag="ps_bc", name="rstdB")
            nc.tensor.matmul(rstdB[:, :], ones_row[:, :], rstd[:, :], start=True, stop=True)
            if out_bf16:
                a_t = small.tile([P, DT, CH], bf16, tag="small", name="a_t")
            else:
                a_t = big.tile([P, DT, CH], f32, tag="big", name="a_t")
            for d in range(DT):
                nc.vector.scalar_tensor_tensor(
                    a_t[:, d, :], x_tile[:, d, :], lng_sb[:, slot * DT + d: slot * DT + d + 1],
                    rstdB[:, :], Alu.mult, Alu.mult)
            if has_lnb:
                for d in range(DT):
                    nc.vector.tensor_scalar(
                        out=a_t[:, d, :], in0=a_t[:, d, :],
                        scalar1=lnb_sb[:, slot * DT + d: slot * DT + d + 1], scalar2=None,
                        op0=Alu.add)
            return a_t

        def mlp_chunk(a_t, l, w1_sb, w2_sb, out_tile, out_off):
            hid = hidp.tile([P, HT, CH], bf16, tag="hid", name="hid")
            for mt in range(HT):
                ps = psmm.tile([P, CH], f32, tag="mm", name="ps1")
                for kt in range(DT):
                    nc.tensor.matmul(ps[:, :], w1_sb[:, kt, bass.ts(mt, P)], a_t[:, kt, :],
                                     start=(kt == 0), stop=(kt == DT - 1))
                nc.scalar.activation(out=hid[:, mt, :], in_=ps[:, :], func=Act.Relu,
                                     bias=b1_sb[:, l * HT + mt: l * HT + mt + 1], scale=1.0)
            for mt in range(DT):
                ps = psmm.tile([P, CH], f32, tag="mm", name="ps2")
                for kt in range(HT):
                    nc.tensor.matmul(ps[:, :], w2_sb[:, kt, bass.ts(mt, P)], hid[:, kt, :],
                                     start=(kt == 0), stop=(kt == HT - 1))
                nc.scalar.activation(out=out_tile[:, mt, out_off: out_off + CH], in_=ps[:, :],
                                     func=Act.Identity,
                                     bias=b2_sb[:, l * DT + mt: l * DT + mt + 1], scale=1.0)

        def conv_chunk(m_t, l, pw_sb, want_bf):
            """m_t: [P, DT, CH+3] f32 with data at cols 3..; returns (cv f32, cv_bf)."""
            acc = big.tile([P, DT, CH], f32, tag="big", name="acc")
            y = small.tile([P, DT, CH], bf16, tag="small", name="y")
            for d in range(DT):
                nc.vector.tensor_scalar(
                    out=acc[:, d, :], in0=m_t[:, d, 0: CH],
                    scalar1=dw_sb[:, l * DT + d, 0:1], scalar2=dwb_sb[:, l * DT + d: l * DT + d + 1],
                    op0=Alu.mult, op1=Alu.add)
                for j in range(1, K - 1):
                    nc.vector.scalar_tensor_tensor(
                        acc[:, d, :], m_t[:, d, j: j + CH], dw_sb[:, l * DT + d, j: j + 1],
                        acc[:, d, :], Alu.mult, Alu.add)
                nc.vector.scalar_tensor_tensor(
                    y[:, d, :], m_t[:, d, K - 1: K - 1 + CH], dw_sb[:, l * DT + d, K - 1: K],
                    acc[:, d, :], Alu.mult, Alu.add)
            cv = big.tile([P, DT, CH], f32, tag="big", name="cv")
            cv_bf = small.tile([P, DT, CH], bf16, tag="small", name="cv_bf") if want_bf else None
            for mt in range(DT):
                ps = psmm.tile([P, CH], f32, tag="mm", name="ps3")
                for kt in range(DT):
                    nc.tensor.matmul(ps[:, :], pw_sb[:, kt, bass.ts(mt, P)], y[:, kt, :],
                                     start=(kt == 0), stop=(kt == DT - 1))
                nc.scalar.activation(out=cv[:, mt, :], in_=ps[:, :], func=Act.Identity,
                                     bias=pwb_sb[:, l * DT + mt: l * DT + mt + 1], scale=1.0)
                if want_bf:
                    nc.scalar.activation(out=cv_bf[:, mt, :], in_=ps[:, :], func=Act.Identity,
                                         bias=pwb_sb[:, l * DT + mt: l * DT + mt + 1], scale=1.0)
            return cv, cv_bf

        def gru_chunk(rhs_bf, res_t, fw_sb, h_prev):
            """kh matmul + gates + scan + residual (in place into res_t). Returns h tile."""
            z = big.tile([P, DT, CH], f32, tag="big", name="z")
            cf = big.tile([P, DT, CH], f32, tag="big", name="cf")
            s = big.tile([P, DT, CH], f32, tag="big", name="s")
            v = big.tile([P, DT, CH], f32, tag="big", name="v")
            h = big.tile([P, DT, CH], f32, tag="big", name="h")
            for mt in range(MT2):
                ps = psmm.tile([P, CH], f32, tag="mm", name="ps4")
                for kt in range(DT):
                    nc.tensor.matmul(ps[:, :], fw_sb[:, kt, bass.ts(mt, P)], rhs_bf[:, kt, :],
                                     start=(kt == 0), stop=(kt == DT - 1))
                if mt < DT:
                    nc.scalar.activation(out=z[:, mt, :], in_=ps[:, :], func=Act.Sigmoid)
                    nc.scalar.activation(out=cf[:, mt, :], in_=ps[:, :], func=Act.Sigmoid,
                                         scale=-1.0)
                else:
                    d = mt - DT
                    nc.scalar.activation(out=s[:, d, :], in_=ps[:, :], func=Act.Sigmoid)
                    nc.vector.scalar_tensor_tensor(
                        s[:, d, :], ps[:, :], 0.5, s[:, d, :], Alu.add, Alu.max)
            for d in range(DT):
                nc.vector.tensor_mul(v[:, d, :], z[:, d, :], s[:, d, :])
            for d in range(DT):
                init = 0.5 if h_prev is None else h_prev[:, d, CH - 1: CH]
                nc.vector.tensor_tensor_scan(h[:, d, :], cf[:, d, :], v[:, d, :], init,
                                             Alu.mult, Alu.add)
            for d in range(DT):
                nc.vector.tensor_add(res_t[:, d, :], h[:, d, :], res_t[:, d, :])
            return h

        # ---------- layer 0: conv0 -> ln1 -> gru0 (+ residual on ln1 out) ----------
        # software-pipelined: stage A(c) = conv+LN (PE-heavy), stage B(c) = GRU tail
        fw_sb = load_w("fw", fwT, 0, [P, DT, E2])
        pw_sb = load_w("pw", pwT, 0, [P, DT, D])

        def l0_stageA(c):
            x_in = big.tile([P, DT, CH + 3], f32, tag="big", name="x_in")
            nc.sync.dma_start(out=x_in, in_=xT.ap().rearrange("(dt p) t -> p dt t", p=P)[:, :, c * CH: c * CH + CH + 3])
            cv, _ = conv_chunk(x_in, 0, pw_sb, want_bf=False)
            n = layernorm_chunk(cv, 0, out_bf16=False)
            n_bf = small.tile([P, DT, CH], bf16, tag="small", name="n_bf")
            for d in range(DT):
                nc.scalar.activation(out=n_bf[:, d, :], in_=n[:, d, :], func=Act.Copy)
            return n, n_bf

        state = {"h": None}

        def l0_stageB(c, n, n_bf):
            state["h"] = gru_chunk(n_bf, n, fw_sb, state["h"])
            nc.sync.dma_start(out=dram3(xs[0], c, CH), in_=n)

        pend = []
        for c in range(NCH):
            pend.append((c, l0_stageA(c)))
            if len(pend) > 2:
                c0, art = pend.pop(0)
                l0_stageB(c0, *art)
        for c0, art in pend:
            l0_stageB(c0, *art)

        # ---------- mid iterations i=0..2: ln2_i, mlp_i, conv_{i+1}, gru_{i+1} ----------
        for i in range(L - 1):
            src, dst = xs[i % 2], xs[(i + 1) % 2]
            w1_sb = load_w("w1", w1T, i, [P, DT, H])
            w2_sb = load_w("w2", w2T, i, [P, HT, D])
            fw_sb = load_w("fw", fwT, i + 1, [P, DT, E2])
            pw_sb = load_w("pw", pwT, i + 1, [P, DT, D])
            state["h"] = None
            m_prev = None

            def mid_stageA(c, m_prev):
                x_in = big.tile([P, DT, CH], f32, tag="big", name="x_in")
                nc.sync.dma_start(out=x_in, in_=dram3(src, c, CH))
                a = layernorm_chunk(x_in, 1 + i, out_bf16=True)
                m = small.tile([P, DT, CH + 3], bf16, tag="small", name="m")
                mlp_chunk(a, i, w1_sb, w2_sb, m, 3)
                if c == 0:
                    nc.vector.memset(m[:, :, 0:3], 0.0)
                else:
                    nc.vector.tensor_copy(out=m[:, :, 0:3], in_=m_prev[:, :, CH: CH + 3])
                return m

            def mid_stageB(c, m):
                cv, cv_bf = conv_chunk(m, i + 1, pw_sb, want_bf=True)
                state["h"] = gru_chunk(cv_bf, cv, fw_sb, state["h"])
                nc.sync.dma_start(out=dram3(dst, c, CH), in_=cv)

            pend = []
            for c in range(NCH):
                m = mid_stageA(c, m_prev)
                m_prev = m
                pend.append((c, m))
                if len(pend) > 2:
                    c0, art = pend.pop(0)
                    mid_stageB(c0, art)
            for c0, art in pend:
                mid_stageB(c0, art)

        # ---------- tail: ln2_3 + mlp_3 ----------
        src = xs[(L - 1) % 2]
        w1_sb = load_w("w1", w1T, L - 1, [P, DT, H])
        w2_sb = load_w("w2", w2T, L - 1, [P, HT, D])
        for c in range(NCH):
            x_in = big.tile([P, DT, CH], f32, tag="big", name="x_in")
            nc.sync.dma_start(out=x_in, in_=dram3(src, c, CH))
            a = layernorm_chunk(x_in, L, out_bf16=True)
            o = big.tile([P, DT, CH], f32, tag="big", name="o")
            mlp_chunk(a, L - 1, w1_sb, w2_sb, o, 0)
            nc.sync.dma_start(out=dram3(out_t, c, CH), in_=o)

    return nc


_CACHE = {}


def get_compiled_nc(T=4096, CH=512, has_lnb=False, **kw):
    key = (T, CH, has_lnb, tuple(sorted(kw.items())))
    if key not in _CACHE:
        nc = build_nc(T, CH, has_lnb, **kw)
        nc.compile()
        _CACHE[key] = nc
    return _CACHE[key]


def make_host_inputs(inputs, T=4096):
    f = np.float32
    w = {
        "fwT": np.ascontiguousarray(np.transpose(np.asarray(inputs["f_w"], f), (0, 2, 1))).astype(BF),
        "pwT": np.ascontiguousarray(np.transpose(np.asarray(inputs["conv_pw_w"], f), (0, 2, 1))).astype(BF),
        "w1T": np.ascontiguousarray(np.transpose(np.asarray(inputs["mlp_w1"], f), (0, 2, 1))).astype(BF),
        "w2T": np.ascontiguousarray(np.transpose(np.asarray(inputs["mlp_w2"], f), (0, 2, 1))).astype(BF),
        "dwK": np.ascontiguousarray(np.transpose(np.asarray(inputs["conv_dw_w"], f), (0, 2, 1))).astype(f),
        "dwb": np.asarray(inputs["conv_dw_b"], f),
        "pwb": np.asarray(inputs["conv_pw_b"], f),
        "b1v": np.asarray(inputs["mlp_b1"], f),
        "b2v": np.asarray(inputs["mlp_b2"], f),
        "lng": np.concatenate([np.asarray(inputs["ln1_g"], f)[None], np.asarray(inputs["ln2_g"], f)], 0),
        "lnb": np.concatenate([np.asarray(inputs["ln1_b"], f)[None], np.asarray(inputs["ln2_b"], f)], 0),
    }
    x = np.asarray(inputs["x"], f)
    nb = x.shape[0]
    in_maps = []
    for b in range(nb):
        xTp = np.zeros((D, T + 3), f)
        xTp[:, 3:] = x[b, :T].T
        in_maps.append({"xT": xTp, **w})
    has_lnb = bool(np.any(w["lnb"] != 0.0))
    return in_maps, has_lnb


def kernel(**inputs):
    from concourse.bass_utils import run_bass_kernel_spmd

    T = int(np.asarray(inputs["x"]).shape[1])
    in_maps, has_lnb = make_host_inputs(inputs, T)
    nc = get_compiled_nc(T=T, has_lnb=has_lnb)
    res = run_bass_kernel_spmd(nc, in_maps, core_ids=list(range(len(in_maps))))
    out = np.stack([r["out"].T for r in res.results])
    return np.ascontiguousarray(out.astype(np.float32))


# revision 8
# speedup vs baseline: 1.0262x; 1.0262x over previous
"""Trainium2 Bass kernel for nn_BlockV2 (conv -> LN -> minGRU -> MLP x4).

Strategy: data-parallel over batch (B=8 -> 8 cores). Per core, activations
are kept in [D_partitions, T_free] layout and streamed through each layer in
chunks of 512 tokens; inter-layer activations ping-pong through DRAM.
The minGRU recurrence h_t = c_t*h_{t-1} + v_t runs on the VectorE
tensor_tensor_scan instruction (fp32 state), chained across chunks.
Matmul inputs are bf16 (fp32 PSUM accumulate); everything on the
LN/scan/residual path stays fp32 (the late-layer signal is a ~5e-3
variation on an O(1) baseline, which bf16 storage would destroy).
LayerNorm is two-pass (center, then variance of centered values) to avoid
E[x^2]-mu^2 cancellation. Emission is software-pipelined: chunk c+1's
LN/MLP matmuls interleave with chunk c's conv/GRU tail so TensorE never
idles long enough to re-throttle (HAM).
"""
import sys

sys.path.insert(0, "/opt/trn_rl_repo")

from contextlib import ExitStack

import numpy as np
import ml_dtypes

import concourse.bass as bass
import concourse.tile as tile
from concourse import bacc, mybir

f32 = mybir.dt.float32
bf16 = mybir.dt.bfloat16
Alu = mybir.AluOpType
Act = mybir.ActivationFunctionType
BF = ml_dtypes.bfloat16

B, D, L, K, H = 8, 512, 4, 4, 2048
N_CORES = 8
LN_EPS = 1e-5
P = 128


def build_nc(T=4096, CH=512, has_lnb=False, stats_fp32=True, q_fp32=False,
             rstd_recip=False):
    NCH = T // CH
    DT = D // P      # 4 d-tiles
    HT = H // P      # 16 h-tiles
    E2 = 2 * D
    MT2 = E2 // P    # 8 m-tiles of the kh matmul

    nc = bacc.Bacc("TRN2", target_bir_lowering=False, debug=False)

    xT = nc.dram_tensor("xT", [D, T + 3], f32, kind="ExternalInput")
    fwT = nc.dram_tensor("fwT", [L, D, E2], bf16, kind="ExternalInput")
    pwT = nc.dram_tensor("pwT", [L, D, D], bf16, kind="ExternalInput")
    w1T = nc.dram_tensor("w1T", [L, D, H], bf16, kind="ExternalInput")
    w2T = nc.dram_tensor("w2T", [L, H, D], bf16, kind="ExternalInput")
    dwK = nc.dram_tensor("dwK", [L, D, K], f32, kind="ExternalInput")
    dwb = nc.dram_tensor("dwb", [L, D], f32, kind="ExternalInput")
    pwb = nc.dram_tensor("pwb", [L, D], f32, kind="ExternalInput")
    b1v = nc.dram_tensor("b1v", [L, H], f32, kind="ExternalInput")
    b2v = nc.dram_tensor("b2v", [L, D], f32, kind="ExternalInput")
    lng = nc.dram_tensor("lng", [L + 1, D], f32, kind="ExternalInput")
    lnb = nc.dram_tensor("lnb", [L + 1, D], f32, kind="ExternalInput")
    out_t = nc.dram_tensor("out", [D, T], f32, kind="ExternalOutput")
    xs = [nc.dram_tensor(f"xs{i}", [D, T], f32) for i in range(2)]

    def dram3(tensor, c, width):
        return tensor.ap().rearrange("(dt p) t -> p dt t", p=P)[:, :, c * CH: c * CH + width]

    with tile.TileContext(nc) as tc, ExitStack() as ctx:
        sing = ctx.enter_context(tc.tile_pool(name="sing", bufs=1))
        wpool = ctx.enter_context(tc.tile_pool(name="w", bufs=1))
        big = ctx.enter_context(tc.tile_pool(name="big", bufs=11))
        small = ctx.enter_context(tc.tile_pool(name="small", bufs=8))
        hidp = ctx.enter_context(tc.tile_pool(name="hid", bufs=2))
        statp = ctx.enter_context(tc.tile_pool(name="stat", bufs=3))
        snipp = ctx.enter_context(tc.tile_pool(name="snip", bufs=2))
        psmm = ctx.enter_context(tc.tile_pool(name="psmm", bufs=4, space="PSUM"))
        psst = ctx.enter_context(tc.tile_pool(name="psst", bufs=2, space="PSUM"))
        psbc = ctx.enter_context(tc.tile_pool(name="psbc", bufs=2, space="PSUM"))

        ones_col = sing.tile([P, 1], bf16)
        nc.vector.memset(ones_col, 1.0)
        ones_colf = sing.tile([P, 1], f32)
        nc.vector.memset(ones_colf, 1.0)
        ones_row = sing.tile([1, P], f32)
        nc.vector.memset(ones_row, 1.0)
        eps1 = sing.tile([1, 1], f32)
        nc.vector.memset(eps1, LN_EPS)
        dw_sb = sing.tile([P, L * DT, K], f32)
        nc.sync.dma_start(out=dw_sb, in_=dwK.ap().rearrange("l (dt p) k -> p (l dt) k", p=P))
        dwb_sb = sing.tile([P, L * DT], f32)
        nc.sync.dma_start(out=dwb_sb, in_=dwb.ap().rearrange("l (dt p) -> p (l dt)", p=P))
        pwb_sb = sing.tile([P, L * DT], f32)
        nc.sync.dma_start(out=pwb_sb, in_=pwb.ap().rearrange("l (dt p) -> p (l dt)", p=P))
        b1_sb = sing.tile([P, L * HT], f32)
        nc.sync.dma_start(out=b1_sb, in_=b1v.ap().rearrange("l (ht p) -> p (l ht)", p=P))
        b2_sb = sing.tile([P, L * DT], f32)
        nc.sync.dma_start(out=b2_sb, in_=b2v.ap().rearrange("l (dt p) -> p (l dt)", p=P))
        lng_sb = sing.tile([P, (L + 1) * DT], f32)
        nc.sync.dma_start(out=lng_sb, in_=lng.ap().rearrange("l (dt p) -> p (l dt)", p=P))
        lnb_sb = sing.tile([P, (L + 1) * DT], f32)
        nc.sync.dma_start(out=lnb_sb, in_=lnb.ap().rearrange("l (dt p) -> p (l dt)", p=P))

        def load_w(kind, dram, l, shape):
            t = wpool.tile(shape, bf16, tag=kind, name=f"{kind}{l}")
            nc.sync.dma_start(out=t, in_=dram.ap()[l].rearrange("(kt p) e -> p kt e", p=P))
            return t

        def layernorm_chunk(x_tile, slot, out_bf16):
            """x_tile: [P, DT, CH] f32; centers x_tile IN PLACE; returns LN out."""
            S_ps = psst.tile([1, CH], f32, tag="ps_stat", name="S_ps")
            if stats_fp32:
                for kt in range(DT):
                    nc.tensor.matmul(S_ps[:, :], ones_colf[:, :], x_tile[:, kt, :],
                                     start=(kt == 0), stop=(kt == DT - 1))
            else:
                x_bf = small.tile([P, DT, CH], bf16, tag="small", name="x_bf")
                for d in range(DT):
                    nc.scalar.activation(out=x_bf[:, d, :], in_=x_tile[:, d, :], func=Act.Copy)
                for kt in range(DT):
                    nc.tensor.matmul(S_ps[:, :], ones_col[:, :], x_bf[:, kt, :],
                                     start=(kt == 0), stop=(kt == DT - 1))
            S_sb = statp.tile([1, CH], f32, tag="stat", name="S_sb")
            nc.vector.tensor_copy(out=S_sb[:, :], in_=S_ps[:, :])
            muB = psbc.tile([P, CH], f32, tag="ps_bc", name="muB")
            nc.tensor.matmul(muB[:, :], ones_row[:, :], S_sb[:, :], start=True, stop=True)
            for d in range(DT):
                nc.vector.scalar_tensor_tensor(
                    x_tile[:, d, :], muB[:, :], -1.0 / D, x_tile[:, d, :], Alu.mult, Alu.add)
            Q_ps = psst.tile([1, CH], f32, tag="ps_stat", name="Q_ps")
            if q_fp32:
                xsq = big.tile([P, DT, CH], f32, tag="big", name="xsq")
                for d in range(DT):
                    nc.vector.tensor_mul(xsq[:, d, :], x_tile[:, d, :], x_tile[:, d, :])
                for kt in range(DT):
                    nc.tensor.matmul(Q_ps[:, :], ones_colf[:, :], xsq[:, kt, :],
                                     start=(kt == 0), stop=(kt == DT - 1))
            else:
                xsq = small.tile([P, DT, CH], bf16, tag="small", name="xsq")
                for d in range(DT):
                    nc.vector.tensor_mul(xsq[:, d, :], x_tile[:, d, :], x_tile[:, d, :])
                for kt in range(DT):
                    nc.tensor.matmul(Q_ps[:, :], ones_col[:, :], xsq[:, kt, :],
                                     start=(kt == 0), stop=(kt == DT - 1))
            rstd = statp.tile([1, CH], f32, tag="stat", name="rstd")
            if rstd_recip:
                sd = statp.tile([1, CH], f32, tag="stat", name="sd")
                nc.scalar.activation(out=sd[:, :], in_=Q_ps[:, :], func=Act.Sqrt,
                                     bias=eps1[:, :], scale=1.0 / D)
                nc.vector.reciprocal(out=rstd[:, :], in_=sd[:, :])
            else:
                lnv = statp.tile([1, CH], f32, tag="stat", name="lnv")
                nc.scalar.activation(out=lnv[:, :], in_=Q_ps[:, :], func=Act.Ln,
                                     bias=eps1[:, :], scale=1.0 / D)
                nc.scalar.activation(out=rstd[:, :], in_=lnv[:, :], func=Act.Exp, scale=-0.5)
            rstdB = psbc.tile([P, CH], f32, tag="ps_bc", name="rstdB")
            nc.tensor.matmul(rstdB[:, :], ones_row[:, :], rstd[:, :], start=True, stop=True)
            if out_bf16:
                a_t = small.tile([P, DT, CH], bf16, tag="small", name="a_t")
            else:
                a_t = big.tile([P, DT, CH], f32, tag="big", name="a_t")
            for d in range(DT):
                nc.vector.scalar_tensor_tensor(
                    a_t[:, d, :], x_tile[:, d, :], lng_sb[:, slot * DT + d: slot * DT + d + 1],
                    rstdB[:, :], Alu.mult, Alu.mult)
            if has_lnb:
                for d in range(DT):
                    nc.vector.tensor_scalar(
                        out=a_t[:, d, :], in0=a_t[:, d, :],
                        scalar1=lnb_sb[:, slot * DT + d: slot * DT + d + 1], scalar2=None,
                        op0=Alu.add)
            return a_t

        def mlp_chunk(a_t, l, w1_sb, w2_sb, out_tile, out_off):
            hid = hidp.tile([P, HT, CH], bf16, tag="hid", name="hid")
            for mt in range(HT):
                ps = psmm.tile([P, CH], f32, tag="mm", name="ps1")
                for kt in range(DT):
                    nc.tensor.matmul(ps[:, :], w1_sb[:, kt, bass.ts(mt, P)], a_t[:, kt, :],
                                     start=(kt == 0), stop=(kt == DT - 1))
                nc.scalar.activation(out=hid[:, mt, :], in_=ps[:, :], func=Act.Relu,
                                     bias=b1_sb[:, l * HT + mt: l * HT + mt + 1], scale=1.0)
            for mt in range(DT):
                ps = psmm.tile([P, CH], f32, tag="mm", name="ps2")
                for kt in range(HT):
                    nc.tensor.matmul(ps[:, :], w2_sb[:, kt, bass.ts(mt, P)], hid[:, kt, :],
                                     start=(kt == 0), stop=(kt == HT - 1))
                nc.scalar.activation(out=out_tile[:, mt, out_off: out_off + CH], in_=ps[:, :],
                                     func=Act.Identity,
                                     bias=b2_sb[:, l * DT + mt: l * DT + mt + 1], scale=1.0)

        def conv_chunk(m_t, l, pw_sb, want_bf):
            """m_t: [P, DT, CH+3] f32 with data at cols 3..; returns (cv f32, cv_bf)."""
            acc = big.tile([P, DT, CH], f32, tag="big", name="acc")
            y = small.tile([P, DT, CH], bf16, tag="small", name="y")
            for d in range(DT):
                nc.vector.tensor_scalar(
                    out=acc[:, d, :], in0=m_t[:, d, 0: CH],
                    scalar1=dw_sb[:, l * DT + d, 0:1], scalar2=dwb_sb[:, l * DT + d: l * DT + d + 1],
                    op0=Alu.mult, op1=Alu.add)
                for j in range(1, K - 1):
                    nc.vector.scalar_tensor_tensor(
                        acc[:, d, :], m_t[:, d, j: j + CH], dw_sb[:, l * DT + d, j: j + 1],
                        acc[:, d, :], Alu.mult, Alu.add)
                nc.vector.scalar_tensor_tensor(
                    y[:, d, :], m_t[:, d, K - 1: K - 1 + CH], dw_sb[:, l * DT + d, K - 1: K],
                    acc[:, d, :], Alu.mult, Alu.add)
            cv = big.tile([P, DT, CH], f32, tag="big", name="cv")
            cv_bf = small.tile([P, DT, CH], bf16, tag="small", name="cv_bf") if want_bf else None
            for mt in range(DT):
                ps = psmm.tile([P, CH], f32, tag="mm", name="ps3")
                for kt in range(DT):
                    nc.tensor.matmul(ps[:, :], pw_sb[:, kt, bass.ts(mt, P)], y[:, kt, :],
                                     start=(kt == 0), stop=(kt == DT - 1))
                nc.scalar.activation(out=cv[:, mt, :], in_=ps[:, :], func=Act.Identity,
                                     bias=pwb_sb[:, l * DT + mt: l * DT + mt + 1], scale=1.0)
                if want_bf:
                    nc.scalar.activation(out=cv_bf[:, mt, :], in_=ps[:, :], func=Act.Identity,
                                         bias=pwb_sb[:, l * DT + mt: l * DT + mt + 1], scale=1.0)
            return cv, cv_bf

        def gru_chunk(rhs_bf, res_t, fw_sb, h_prev):
            """kh matmul + gates + scan (in place over v) + residual. Returns
            a [P, DT, 1] snippet holding the last scan column (next chunk's init)."""
            z = big.tile([P, DT, CH], f32, tag="big", name="z")
            cf = big.tile([P, DT, CH], f32, tag="big", name="cf")
            s = big.tile([P, DT, CH], f32, tag="big", name="s")
            v = big.tile([P, DT, CH], f32, tag="big", name="v")
            for mt in range(MT2):
                ps = psmm.tile([P, CH], f32, tag="mm", name="ps4")
                for kt in range(DT):
                    nc.tensor.matmul(ps[:, :], fw_sb[:, kt, bass.ts(mt, P)], rhs_bf[:, kt, :],
                                     start=(kt == 0), stop=(kt == DT - 1))
                if mt < DT:
                    nc.scalar.activation(out=z[:, mt, :], in_=ps[:, :], func=Act.Sigmoid)
                    nc.scalar.activation(out=cf[:, mt, :], in_=ps[:, :], func=Act.Sigmoid,
                                         scale=-1.0)
                else:
                    d = mt - DT
                    nc.scalar.activation(out=s[:, d, :], in_=ps[:, :], func=Act.Sigmoid)
                    nc.vector.scalar_tensor_tensor(
                        s[:, d, :], ps[:, :], 0.5, s[:, d, :], Alu.add, Alu.max)
            for d in range(DT):
                nc.vector.tensor_mul(v[:, d, :], z[:, d, :], s[:, d, :])
            for d in range(DT):
                init = 0.5 if h_prev is None else h_prev[:, d, 0:1]
                nc.vector.tensor_tensor_scan(v[:, d, :], cf[:, d, :], v[:, d, :], init,
                                             Alu.mult, Alu.add)
            snip = snipp.tile([P, DT, 1], f32, tag="snip", name="snip")
            nc.vector.tensor_copy(out=snip[:, :, :], in_=v[:, :, CH - 1: CH])
            for d in range(DT):
                nc.vector.tensor_add(res_t[:, d, :], v[:, d, :], res_t[:, d, :])
            return snip

        # ---------- layer 0: conv0 -> ln1 -> gru0 (+ residual on ln1 out) ----------
        # software-pipelined: stage A(c) = conv+LN (PE-heavy), stage B(c) = GRU tail
        fw_sb = load_w("fw", fwT, 0, [P, DT, E2])
        pw_sb = load_w("pw", pwT, 0, [P, DT, D])

        def l0_stageA(c):
            x_in = big.tile([P, DT, CH + 3], f32, tag="big", name="x_in")
            nc.sync.dma_start(out=x_in, in_=xT.ap().rearrange("(dt p) t -> p dt t", p=P)[:, :, c * CH: c * CH + CH + 3])
            cv, _ = conv_chunk(x_in, 0, pw_sb, want_bf=False)
            n = layernorm_chunk(cv, 0, out_bf16=False)
            n_bf = small.tile([P, DT, CH], bf16, tag="small", name="n_bf")
            for d in range(DT):
                nc.scalar.activation(out=n_bf[:, d, :], in_=n[:, d, :], func=Act.Copy)
            return n, n_bf

        state = {"h": None}

        def l0_stageB(c, n, n_bf):
            state["h"] = gru_chunk(n_bf, n, fw_sb, state["h"])
            nc.sync.dma_start(out=dram3(xs[0], c, CH), in_=n)

        pend = []
        for c in range(NCH):
            pend.append((c, l0_stageA(c)))
            if len(pend) > 2:
                c0, art = pend.pop(0)
                l0_stageB(c0, *art)
        for c0, art in pend:
            l0_stageB(c0, *art)

        # ---------- mid iterations i=0..2: ln2_i, mlp_i, conv_{i+1}, gru_{i+1} ----------
        for i in range(L - 1):
            src, dst = xs[i % 2], xs[(i + 1) % 2]
            w1_sb = load_w("w1", w1T, i, [P, DT, H])
            w2_sb = load_w("w2", w2T, i, [P, HT, D])
            fw_sb = load_w("fw", fwT, i + 1, [P, DT, E2])
            pw_sb = load_w("pw", pwT, i + 1, [P, DT, D])
            state["h"] = None
            m_prev = None

            def mid_stageA(c, m_prev):
                x_in = big.tile([P, DT, CH], f32, tag="big", name="x_in")
                nc.sync.dma_start(out=x_in, in_=dram3(src, c, CH))
                a = layernorm_chunk(x_in, 1 + i, out_bf16=True)
                m = small.tile([P, DT, CH + 3], bf16, tag="small", name="m")
                mlp_chunk(a, i, w1_sb, w2_sb, m, 3)
                if c == 0:
                    nc.vector.memset(m[:, :, 0:3], 0.0)
                else:
                    nc.vector.tensor_copy(out=m[:, :, 0:3], in_=m_prev[:, :, CH: CH + 3])
                return m

            def mid_stageB(c, m):
                cv, cv_bf = conv_chunk(m, i + 1, pw_sb, want_bf=True)
                state["h"] = gru_chunk(cv_bf, cv, fw_sb, state["h"])
                nc.sync.dma_start(out=dram3(dst, c, CH), in_=cv)

            pend = []
            for c in range(NCH):
                m = mid_stageA(c, m_prev)
                m_prev = m
                pend.append((c, m))
                if len(pend) > 2:
                    c0, art = pend.pop(0)
                    mid_stageB(c0, art)
            for c0, art in pend:
                mid_stageB(c0, art)

        # ---------- tail: ln2_3 + mlp_3 ----------
        src = xs[(L - 1) % 2]
        w1_sb = load_w("w1", w1T, L - 1, [P, DT, H])
        w2_sb = load_w("w2", w2T, L - 1, [P, HT, D])
        for c in range(NCH):
            x_in = big.tile([P, DT, CH], f32, tag="big", name="x_in")
            nc.sync.dma_start(out=x_in, in_=dram3(src, c, CH))
            a = layernorm_chunk(x_in, L, out_bf16=True)
            o = big.tile([P, DT, CH], f32, tag="big", name="o")
            mlp_chunk(a, L - 1, w1_sb, w2_sb, o, 0)
            nc.sync.dma_start(out=dram3(out_t, c, CH), in_=o)

    return nc


_CACHE = {}


def get_compiled_nc(T=4096, CH=512, has_lnb=False, **kw):
    key = (T, CH, has_lnb, tuple(sorted(kw.items())))
    if key not in _CACHE:
        nc = build_nc(T, CH, has_lnb, **kw)
        nc.compile()
        _CACHE[key] = nc
    return _CACHE[key]


def make_host_inputs(inputs, T=4096):
    f = np.float32
    w = {
        "fwT": np.ascontiguousarray(np.transpose(np.asarray(inputs["f_w"], f), (0, 2, 1))).astype(BF),
        "pwT": np.ascontiguousarray(np.transpose(np.asarray(inputs["conv_pw_w"], f), (0, 2, 1))).astype(BF),
        "w1T": np.ascontiguousarray(np.transpose(np.asarray(inputs["mlp_w1"], f), (0, 2, 1))).astype(BF),
        "w2T": np.ascontiguousarray(np.transpose(np.asarray(inputs["mlp_w2"], f), (0, 2, 1))).astype(BF),
        "dwK": np.ascontiguousarray(np.transpose(np.asarray(inputs["conv_dw_w"], f), (0, 2, 1))).astype(f),
        "dwb": np.asarray(inputs["conv_dw_b"], f),
        "pwb": np.asarray(inputs["conv_pw_b"], f),
        "b1v": np.asarray(inputs["mlp_b1"], f),
        "b2v": np.asarray(inputs["mlp_b2"], f),
        "lng": np.concatenate([np.asarray(inputs["ln1_g"], f)[None], np.asarray(inputs["ln2_g"], f)], 0),
        "lnb": np.concatenate([np.asarray(inputs["ln1_b"], f)[None], np.asarray(inputs["ln2_b"], f)], 0),
    }
    x = np.asarray(inputs["x"], f)
    nb = x.shape[0]
    in_maps = []
    for b in range(nb):
        xTp = np.zeros((D, T + 3), f)
        xTp[:, 3:] = x[b, :T].T
        in_maps.append({"xT": xTp, **w})
    has_lnb = bool(np.any(w["lnb"] != 0.0))
    return in_maps, has_lnb


def kernel(**inputs):
    from concourse.bass_utils import run_bass_kernel_spmd

    T = int(np.asarray(inputs["x"]).shape[1])
    in_maps, has_lnb = make_host_inputs(inputs, T)
    nc = get_compiled_nc(T=T, has_lnb=has_lnb)
    res = run_bass_kernel_spmd(nc, in_maps, core_ids=list(range(len(in_maps))))
    out = np.stack([r["out"].T for r in res.results])
    return np.ascontiguousarray(out.astype(np.float32))


# revision 11
# speedup vs baseline: 1.0333x; 1.0069x over previous
"""Trainium2 Bass kernel for nn_BlockV2 (conv -> LN -> minGRU -> MLP x4).

Strategy: data-parallel over batch (B=8 -> 8 cores). Per core, activations
are kept in [D_partitions, T_free] layout and streamed through each layer in
chunks of 512 tokens; inter-layer activations ping-pong through DRAM.
The minGRU recurrence h_t = c_t*h_{t-1} + v_t runs on the VectorE
tensor_tensor_scan instruction (fp32 state), chained across chunks.
Matmul inputs are bf16 (fp32 PSUM accumulate); everything on the
LN/scan/residual path stays fp32 (the late-layer signal is a ~5e-3
variation on an O(1) baseline, which bf16 storage would destroy).
LayerNorm is two-pass (center, then variance of centered values) to avoid
E[x^2]-mu^2 cancellation. Emission is software-pipelined: chunk c+1's
LN/MLP matmuls interleave with chunk c's conv/GRU tail so TensorE never
idles long enough to re-throttle (HAM).
"""
import sys

sys.path.insert(0, "/opt/trn_rl_repo")

from contextlib import ExitStack

import numpy as np
import ml_dtypes

import concourse.bass as bass
import concourse.tile as tile
from concourse import bacc, mybir

f32 = mybir.dt.float32
bf16 = mybir.dt.bfloat16
Alu = mybir.AluOpType
Act = mybir.ActivationFunctionType
BF = ml_dtypes.bfloat16

B, D, L, K, H = 8, 512, 4, 4, 2048
N_CORES = 8
LN_EPS = 1e-5
P = 128


def build_nc(T=4096, CH=512, has_lnb=False, stats_fp32=True, q_fp32=False,
             rstd_recip=False):
    NCH = T // CH
    DT = D // P      # 4 d-tiles
    HT = H // P      # 16 h-tiles
    E2 = 2 * D
    MT2 = E2 // P    # 8 m-tiles of the kh matmul

    nc = bacc.Bacc("TRN2", target_bir_lowering=False, debug=False)

    xT = nc.dram_tensor("xT", [D, T + 3], f32, kind="ExternalInput")
    fwT = nc.dram_tensor("fwT", [L, D, E2], bf16, kind="ExternalInput")
    pwT = nc.dram_tensor("pwT", [L, D, D], bf16, kind="ExternalInput")
    w1T = nc.dram_tensor("w1T", [L, D, H], bf16, kind="ExternalInput")
    w2T = nc.dram_tensor("w2T", [L, H, D], bf16, kind="ExternalInput")
    dwK = nc.dram_tensor("dwK", [L, D, K], f32, kind="ExternalInput")
    dwb = nc.dram_tensor("dwb", [L, D], f32, kind="ExternalInput")
    pwb = nc.dram_tensor("pwb", [L, D], f32, kind="ExternalInput")
    b1v = nc.dram_tensor("b1v", [L, H], f32, kind="ExternalInput")
    b2v = nc.dram_tensor("b2v", [L, D], f32, kind="ExternalInput")
    lng = nc.dram_tensor("lng", [L + 1, D], f32, kind="ExternalInput")
    lnb = nc.dram_tensor("lnb", [L + 1, D], f32, kind="ExternalInput")
    out_t = nc.dram_tensor("out", [D, T], f32, kind="ExternalOutput")
    xs = [nc.dram_tensor(f"xs{i}", [D, T], f32) for i in range(2)]

    def dram3(tensor, c, width):
        return tensor.ap().rearrange("(dt p) t -> p dt t", p=P)[:, :, c * CH: c * CH + width]

    with tile.TileContext(nc) as tc, ExitStack() as ctx:
        sing = ctx.enter_context(tc.tile_pool(name="sing", bufs=1))
        wpool = ctx.enter_context(tc.tile_pool(name="w", bufs=1))
        big = ctx.enter_context(tc.tile_pool(name="big", bufs=11))
        small = ctx.enter_context(tc.tile_pool(name="small", bufs=8))
        hidp = ctx.enter_context(tc.tile_pool(name="hid", bufs=2))
        statp = ctx.enter_context(tc.tile_pool(name="stat", bufs=3))
        snipp = ctx.enter_context(tc.tile_pool(name="snip", bufs=2))
        psmm = ctx.enter_context(tc.tile_pool(name="psmm", bufs=4, space="PSUM"))
        psst = ctx.enter_context(tc.tile_pool(name="psst", bufs=2, space="PSUM"))
        psbc = ctx.enter_context(tc.tile_pool(name="psbc", bufs=2, space="PSUM"))

        ones_col = sing.tile([P, 1], bf16)
        nc.vector.memset(ones_col, 1.0)
        ones_colf = sing.tile([P, 1], f32)
        nc.vector.memset(ones_colf, 1.0)
        ones_row = sing.tile([1, P], f32)
        nc.vector.memset(ones_row, 1.0)
        eps1 = sing.tile([1, 1], f32)
        nc.vector.memset(eps1, LN_EPS)
        dw_sb = sing.tile([P, L * DT, K], f32)
        nc.sync.dma_start(out=dw_sb, in_=dwK.ap().rearrange("l (dt p) k -> p (l dt) k", p=P))
        dwb_sb = sing.tile([P, L * DT], f32)
        nc.sync.dma_start(out=dwb_sb, in_=dwb.ap().rearrange("l (dt p) -> p (l dt)", p=P))
        pwb_sb = sing.tile([P, L * DT], f32)
        nc.sync.dma_start(out=pwb_sb, in_=pwb.ap().rearrange("l (dt p) -> p (l dt)", p=P))
        b1_sb = sing.tile([P, L * HT], f32)
        nc.sync.dma_start(out=b1_sb, in_=b1v.ap().rearrange("l (ht p) -> p (l ht)", p=P))
        b2_sb = sing.tile([P, L * DT], f32)
        nc.sync.dma_start(out=b2_sb, in_=b2v.ap().rearrange("l (dt p) -> p (l dt)", p=P))
        lng_sb = sing.tile([P, (L + 1) * DT], f32)
        nc.sync.dma_start(out=lng_sb, in_=lng.ap().rearrange("l (dt p) -> p (l dt)", p=P))
        lnb_sb = sing.tile([P, (L + 1) * DT], f32)
        nc.sync.dma_start(out=lnb_sb, in_=lnb.ap().rearrange("l (dt p) -> p (l dt)", p=P))

        def load_w(kind, dram, l, shape):
            t = wpool.tile(shape, bf16, tag=kind, name=f"{kind}{l}")
            nc.sync.dma_start(out=t, in_=dram.ap()[l].rearrange("(kt p) e -> p kt e", p=P))
            return t

        def layernorm_chunk(x_tile, slot, out_bf16):
            """x_tile: [P, DT, CH] f32; centers x_tile IN PLACE; returns LN out."""
            S_ps = psst.tile([1, CH], f32, tag="ps_stat", name="S_ps")
            if stats_fp32:
                for kt in range(DT):
                    nc.tensor.matmul(S_ps[:, :], ones_colf[:, :], x_tile[:, kt, :],
                                     start=(kt == 0), stop=(kt == DT - 1))
            else:
                x_bf = small.tile([P, DT, CH], bf16, tag="small", name="x_bf")
                for d in range(DT):
                    nc.scalar.activation(out=x_bf[:, d, :], in_=x_tile[:, d, :], func=Act.Copy)
                for kt in range(DT):
                    nc.tensor.matmul(S_ps[:, :], ones_col[:, :], x_bf[:, kt, :],
                                     start=(kt == 0), stop=(kt == DT - 1))
            S_sb = statp.tile([1, CH], f32, tag="stat", name="S_sb")
            nc.vector.tensor_copy(out=S_sb[:, :], in_=S_ps[:, :])
            muB = psbc.tile([P, CH], f32, tag="ps_bc", name="muB")
            nc.tensor.matmul(muB[:, :], ones_row[:, :], S_sb[:, :], start=True, stop=True)
            for d in range(DT):
                nc.vector.scalar_tensor_tensor(
                    x_tile[:, d, :], muB[:, :], -1.0 / D, x_tile[:, d, :], Alu.mult, Alu.add)
            Q_ps = psst.tile([1, CH], f32, tag="ps_stat", name="Q_ps")
            if q_fp32:
                xsq = big.tile([P, DT, CH], f32, tag="big", name="xsq")
                for d in range(DT):
                    nc.vector.tensor_mul(xsq[:, d, :], x_tile[:, d, :], x_tile[:, d, :])
                for kt in range(DT):
                    nc.tensor.matmul(Q_ps[:, :], ones_colf[:, :], xsq[:, kt, :],
                                     start=(kt == 0), stop=(kt == DT - 1))
            else:
                xsq = small.tile([P, DT, CH], bf16, tag="small", name="xsq")
                for d in range(DT):
                    nc.vector.tensor_mul(xsq[:, d, :], x_tile[:, d, :], x_tile[:, d, :])
                for kt in range(DT):
                    nc.tensor.matmul(Q_ps[:, :], ones_col[:, :], xsq[:, kt, :],
                                     start=(kt == 0), stop=(kt == DT - 1))
            rstd = statp.tile([1, CH], f32, tag="stat", name="rstd")
            if rstd_recip:
                sd = statp.tile([1, CH], f32, tag="stat", name="sd")
                nc.scalar.activation(out=sd[:, :], in_=Q_ps[:, :], func=Act.Sqrt,
                                     bias=eps1[:, :], scale=1.0 / D)
                nc.vector.reciprocal(out=rstd[:, :], in_=sd[:, :])
            else:
                lnv = statp.tile([1, CH], f32, tag="stat", name="lnv")
                nc.scalar.activation(out=lnv[:, :], in_=Q_ps[:, :], func=Act.Ln,
                                     bias=eps1[:, :], scale=1.0 / D)
                nc.scalar.activation(out=rstd[:, :], in_=lnv[:, :], func=Act.Exp, scale=-0.5)
            rstdB = psbc.tile([P, CH], f32, tag="ps_bc", name="rstdB")
            nc.tensor.matmul(rstdB[:, :], ones_row[:, :], rstd[:, :], start=True, stop=True)
            if out_bf16:
                a_t = small.tile([P, DT, CH], bf16, tag="small", name="a_t")
            else:
                a_t = big.tile([P, DT, CH], f32, tag="big", name="a_t")
            for d in range(DT):
                nc.vector.scalar_tensor_tensor(
                    a_t[:, d, :], x_tile[:, d, :], lng_sb[:, slot * DT + d: slot * DT + d + 1],
                    rstdB[:, :], Alu.mult, Alu.mult)
            if has_lnb:
                for d in range(DT):
                    nc.vector.tensor_scalar(
                        out=a_t[:, d, :], in0=a_t[:, d, :],
                        scalar1=lnb_sb[:, slot * DT + d: slot * DT + d + 1], scalar2=None,
                        op0=Alu.add)
            return a_t

        def mlp_chunk(a_t, l, w1_sb, w2_sb, out_tile, out_off):
            hid = hidp.tile([P, HT, CH], bf16, tag="hid", name="hid")
            for mt in range(HT):
                ps = psmm.tile([P, CH], f32, tag="mm", name="ps1")
                for kt in range(DT):
                    nc.tensor.matmul(ps[:, :], w1_sb[:, kt, bass.ts(mt, P)], a_t[:, kt, :],
                                     start=(kt == 0), stop=(kt == DT - 1))
                nc.scalar.activation(out=hid[:, mt, :], in_=ps[:, :], func=Act.Relu,
                                     bias=b1_sb[:, l * HT + mt: l * HT + mt + 1], scale=1.0)
            for mt in range(DT):
                ps = psmm.tile([P, CH], f32, tag="mm", name="ps2")
                for kt in range(HT):
                    nc.tensor.matmul(ps[:, :], w2_sb[:, kt, bass.ts(mt, P)], hid[:, kt, :],
                                     start=(kt == 0), stop=(kt == HT - 1))
                nc.scalar.activation(out=out_tile[:, mt, out_off: out_off + CH], in_=ps[:, :],
                                     func=Act.Identity,
                                     bias=b2_sb[:, l * DT + mt: l * DT + mt + 1], scale=1.0)

        def conv_chunk(m_t, l, pw_sb, want_bf):
            """m_t: [P, DT, CH+3] f32 with data at cols 3..; returns (cv f32, cv_bf)."""
            acc = big.tile([P, DT, CH], f32, tag="big", name="acc")
            y = small.tile([P, DT, CH], bf16, tag="small", name="y")
            for d in range(DT):
                nc.vector.tensor_scalar(
                    out=acc[:, d, :], in0=m_t[:, d, 0: CH],
                    scalar1=dw_sb[:, l * DT + d, 0:1], scalar2=dwb_sb[:, l * DT + d: l * DT + d + 1],
                    op0=Alu.mult, op1=Alu.add)
                for j in range(1, K - 1):
                    nc.vector.scalar_tensor_tensor(
                        acc[:, d, :], m_t[:, d, j: j + CH], dw_sb[:, l * DT + d, j: j + 1],
                        acc[:, d, :], Alu.mult, Alu.add)
                nc.vector.scalar_tensor_tensor(
                    y[:, d, :], m_t[:, d, K - 1: K - 1 + CH], dw_sb[:, l * DT + d, K - 1: K],
                    acc[:, d, :], Alu.mult, Alu.add)
            cv = big.tile([P, DT, CH], f32, tag="big", name="cv")
            cv_bf = small.tile([P, DT, CH], bf16, tag="small", name="cv_bf") if want_bf else None
            for mt in range(DT):
                ps = psmm.tile([P, CH], f32, tag="mm", name="ps3")
                for kt in range(DT):
                    nc.tensor.matmul(ps[:, :], pw_sb[:, kt, bass.ts(mt, P)], y[:, kt, :],
                                     start=(kt == 0), stop=(kt == DT - 1))
                nc.scalar.activation(out=cv[:, mt, :], in_=ps[:, :], func=Act.Identity,
                                     bias=pwb_sb[:, l * DT + mt: l * DT + mt + 1], scale=1.0)
                if want_bf:
                    nc.scalar.activation(out=cv_bf[:, mt, :], in_=ps[:, :], func=Act.Identity,
                                         bias=pwb_sb[:, l * DT + mt: l * DT + mt + 1], scale=1.0)
            return cv, cv_bf

        def gru_chunk(rhs_bf, res_t, fw_sb, h_prev):
            """kh matmul + gates + scan (in place over v) + residual. Returns
            a [P, DT, 1] snippet holding the last scan column (next chunk's init)."""
            z = big.tile([P, DT, CH], f32, tag="big", name="z")
            cf = big.tile([P, DT, CH], f32, tag="big", name="cf")
            s = big.tile([P, DT, CH], f32, tag="big", name="s")
            v = big.tile([P, DT, CH], f32, tag="big", name="v")
            for mt in range(MT2):
                ps = psmm.tile([P, CH], f32, tag="mm", name="ps4")
                for kt in range(DT):
                    nc.tensor.matmul(ps[:, :], fw_sb[:, kt, bass.ts(mt, P)], rhs_bf[:, kt, :],
                                     start=(kt == 0), stop=(kt == DT - 1))
                if mt < DT:
                    nc.scalar.activation(out=z[:, mt, :], in_=ps[:, :], func=Act.Sigmoid)
                    nc.scalar.activation(out=cf[:, mt, :], in_=ps[:, :], func=Act.Sigmoid,
                                         scale=-1.0)
                else:
                    d = mt - DT
                    nc.scalar.activation(out=s[:, d, :], in_=ps[:, :], func=Act.Sigmoid)
                    nc.vector.scalar_tensor_tensor(
                        s[:, d, :], ps[:, :], 0.5, s[:, d, :], Alu.add, Alu.max)
            for d in range(DT):
                nc.vector.tensor_mul(v[:, d, :], z[:, d, :], s[:, d, :])
            for d in range(DT):
                init = 0.5 if h_prev is None else h_prev[:, d, 0:1]
                nc.vector.tensor_tensor_scan(v[:, d, :], cf[:, d, :], v[:, d, :], init,
                                             Alu.mult, Alu.add)
            snip = snipp.tile([P, DT, 1], f32, tag="snip", name="snip")
            nc.vector.tensor_copy(out=snip[:, :, :], in_=v[:, :, CH - 1: CH])
            for d in range(DT):
                nc.vector.tensor_add(res_t[:, d, :], v[:, d, :], res_t[:, d, :])
            return snip

        # ---------- layer 0: conv0 -> ln1 -> gru0 (+ residual on ln1 out) ----------
        # software-pipelined: stage A(c) = conv+LN (PE-heavy), stage B(c) = GRU tail
        fw_sb = load_w("fw", fwT, 0, [P, DT, E2])
        pw_sb = load_w("pw", pwT, 0, [P, DT, D])

        def l0_stageA(c):
            x_in = big.tile([P, DT, CH + 3], f32, tag="big", name="x_in")
            nc.sync.dma_start(out=x_in, in_=xT.ap().rearrange("(dt p) t -> p dt t", p=P)[:, :, c * CH: c * CH + CH + 3])
            cv, _ = conv_chunk(x_in, 0, pw_sb, want_bf=False)
            n = layernorm_chunk(cv, 0, out_bf16=False)
            n_bf = small.tile([P, DT, CH], bf16, tag="small", name="n_bf")
            for d in range(DT):
                nc.scalar.activation(out=n_bf[:, d, :], in_=n[:, d, :], func=Act.Copy)
            return n, n_bf

        state = {"h": None}

        def l0_stageB(c, n, n_bf):
            state["h"] = gru_chunk(n_bf, n, fw_sb, state["h"])
            nc.sync.dma_start(out=dram3(xs[0], c, CH), in_=n)

        pend = []
        for c in range(NCH):
            pend.append((c, l0_stageA(c)))
            if len(pend) > 1:
                c0, art = pend.pop(0)
                l0_stageB(c0, *art)
        for c0, art in pend:
            l0_stageB(c0, *art)

        # ---------- mid iterations i=0..2: ln2_i, mlp_i, conv_{i+1}, gru_{i+1} ----------
        for i in range(L - 1):
            src, dst = xs[i % 2], xs[(i + 1) % 2]
            w1_sb = load_w("w1", w1T, i, [P, DT, H])
            w2_sb = load_w("w2", w2T, i, [P, HT, D])
            fw_sb = load_w("fw", fwT, i + 1, [P, DT, E2])
            pw_sb = load_w("pw", pwT, i + 1, [P, DT, D])
            state["h"] = None
            m_prev = None

            def mid_stageA(c, m_prev):
                x_in = big.tile([P, DT, CH], f32, tag="big", name="x_in")
                nc.sync.dma_start(out=x_in, in_=dram3(src, c, CH))
                a = layernorm_chunk(x_in, 1 + i, out_bf16=True)
                m = small.tile([P, DT, CH + 3], bf16, tag="small", name="m")
                mlp_chunk(a, i, w1_sb, w2_sb, m, 3)
                if c == 0:
                    nc.vector.memset(m[:, :, 0:3], 0.0)
                else:
                    nc.vector.tensor_copy(out=m[:, :, 0:3], in_=m_prev[:, :, CH: CH + 3])
                return m

            def mid_stageB(c, m):
                cv, cv_bf = conv_chunk(m, i + 1, pw_sb, want_bf=True)
                state["h"] = gru_chunk(cv_bf, cv, fw_sb, state["h"])
                nc.sync.dma_start(out=dram3(dst, c, CH), in_=cv)

            pend = []
            for c in range(NCH):
                m = mid_stageA(c, m_prev)
                m_prev = m
                pend.append((c, m))
                if len(pend) > 1:
                    c0, art = pend.pop(0)
                    mid_stageB(c0, art)
            for c0, art in pend:
                mid_stageB(c0, art)

        # ---------- tail: ln2_3 + mlp_3 ----------
        src = xs[(L - 1) % 2]
        w1_sb = load_w("w1", w1T, L - 1, [P, DT, H])
        w2_sb = load_w("w2", w2T, L - 1, [P, HT, D])
        for c in range(NCH):
            x_in = big.tile([P, DT, CH], f32, tag="big", name="x_in")
            nc.sync.dma_start(out=x_in, in_=dram3(src, c, CH))
            a = layernorm_chunk(x_in, L, out_bf16=True)
            o = big.tile([P, DT, CH], f32, tag="big", name="o")
            mlp_chunk(a, L - 1, w1_sb, w2_sb, o, 0)
            nc.sync.dma_start(out=dram3(out_t, c, CH), in_=o)

    return nc


_CACHE = {}


def get_compiled_nc(T=4096, CH=512, has_lnb=False, **kw):
    key = (T, CH, has_lnb, tuple(sorted(kw.items())))
    if key not in _CACHE:
        nc = build_nc(T, CH, has_lnb, **kw)
        nc.compile()
        _CACHE[key] = nc
    return _CACHE[key]


def make_host_inputs(inputs, T=4096):
    f = np.float32
    w = {
        "fwT": np.ascontiguousarray(np.transpose(np.asarray(inputs["f_w"], f), (0, 2, 1))).astype(BF),
        "pwT": np.ascontiguousarray(np.transpose(np.asarray(inputs["conv_pw_w"], f), (0, 2, 1))).astype(BF),
        "w1T": np.ascontiguousarray(np.transpose(np.asarray(inputs["mlp_w1"], f), (0, 2, 1))).astype(BF),
        "w2T": np.ascontiguousarray(np.transpose(np.asarray(inputs["mlp_w2"], f), (0, 2, 1))).astype(BF),
        "dwK": np.ascontiguousarray(np.transpose(np.asarray(inputs["conv_dw_w"], f), (0, 2, 1))).astype(f),
        "dwb": np.asarray(inputs["conv_dw_b"], f),
        "pwb": np.asarray(inputs["conv_pw_b"], f),
        "b1v": np.asarray(inputs["mlp_b1"], f),
        "b2v": np.asarray(inputs["mlp_b2"], f),
        "lng": np.concatenate([np.asarray(inputs["ln1_g"], f)[None], np.asarray(inputs["ln2_g"], f)], 0),
        "lnb": np.concatenate([np.asarray(inputs["ln1_b"], f)[None], np.asarray(inputs["ln2_b"], f)], 0),
    }
    x = np.asarray(inputs["x"], f)
    nb = x.shape[0]
    in_maps = []
    for b in range(nb):
        xTp = np.zeros((D, T + 3), f)
        xTp[:, 3:] = x[b, :T].T
        in_maps.append({"xT": xTp, **w})
    has_lnb = bool(np.any(w["lnb"] != 0.0))
    return in_maps, has_lnb


def kernel(**inputs):
    from concourse.bass_utils import run_bass_kernel_spmd

    T = int(np.asarray(inputs["x"]).shape[1])
    in_maps, has_lnb = make_host_inputs(inputs, T)
    nc = get_compiled_nc(T=T, has_lnb=has_lnb)
    res = run_bass_kernel_spmd(nc, in_maps, core_ids=list(range(len(in_maps))))
    out = np.stack([r["out"].T for r in res.results])
    return np.ascontiguousarray(out.astype(np.float32))


# revision 12
# speedup vs baseline: 1.0664x; 1.0320x over previous
"""Trainium2 Bass kernel for nn_BlockV2 (conv -> LN -> minGRU -> MLP x4).

Strategy: data-parallel over batch (B=8 -> 8 cores). Per core, activations
are kept in [D_partitions, T_free] layout and streamed through each layer in
chunks of 512 tokens; inter-layer activations ping-pong through DRAM.
The minGRU recurrence h_t = c_t*h_{t-1} + v_t runs on the VectorE
tensor_tensor_scan instruction (fp32 state), chained across chunks.
Matmul inputs are bf16 (fp32 PSUM accumulate); everything on the
LN/scan/residual path stays fp32 (the late-layer signal is a ~5e-3
variation on an O(1) baseline, which bf16 storage would destroy).
LayerNorm is two-pass (center, then variance of centered values) to avoid
E[x^2]-mu^2 cancellation. Emission is software-pipelined: chunk c+1's
LN/MLP matmuls interleave with chunk c's conv/GRU tail so TensorE never
idles long enough to re-throttle (HAM).
"""
import sys

sys.path.insert(0, "/opt/trn_rl_repo")

from contextlib import ExitStack

import numpy as np
import ml_dtypes

import concourse.bass as bass
import concourse.tile as tile
from concourse import bacc, mybir

f32 = mybir.dt.float32
bf16 = mybir.dt.bfloat16
Alu = mybir.AluOpType
Act = mybir.ActivationFunctionType
BF = ml_dtypes.bfloat16

B, D, L, K, H = 8, 512, 4, 4, 2048
N_CORES = 8
LN_EPS = 1e-5
P = 128


def build_nc(T=4096, CH=512, has_lnb=False, stats_fp32=True, q_fp32=False,
             rstd_recip=False):
    NCH = T // CH
    DT = D // P      # 4 d-tiles
    HT = H // P      # 16 h-tiles
    E2 = 2 * D
    MT2 = E2 // P    # 8 m-tiles of the kh matmul

    nc = bacc.Bacc("TRN2", target_bir_lowering=False, debug=False)

    xT = nc.dram_tensor("xT", [D, T + 3], f32, kind="ExternalInput")
    fwT = nc.dram_tensor("fwT", [L, D, E2], bf16, kind="ExternalInput")
    pwT = nc.dram_tensor("pwT", [L, D, D], bf16, kind="ExternalInput")
    w1T = nc.dram_tensor("w1T", [L, D, H], bf16, kind="ExternalInput")
    w2T = nc.dram_tensor("w2T", [L, H, D], bf16, kind="ExternalInput")
    dwK = nc.dram_tensor("dwK", [L, D, K], f32, kind="ExternalInput")
    dwb = nc.dram_tensor("dwb", [L, D], f32, kind="ExternalInput")
    pwb = nc.dram_tensor("pwb", [L, D], f32, kind="ExternalInput")
    b1v = nc.dram_tensor("b1v", [L, H], f32, kind="ExternalInput")
    b2v = nc.dram_tensor("b2v", [L, D], f32, kind="ExternalInput")
    lng = nc.dram_tensor("lng", [L + 1, D], f32, kind="ExternalInput")
    lnb = nc.dram_tensor("lnb", [L + 1, D], f32, kind="ExternalInput")
    out_t = nc.dram_tensor("out", [D, T], f32, kind="ExternalOutput")
    xs = [nc.dram_tensor(f"xs{i}", [D, T], f32) for i in range(2)]

    def dram3(tensor, c, width):
        return tensor.ap().rearrange("(dt p) t -> p dt t", p=P)[:, :, c * CH: c * CH + width]

    with tile.TileContext(nc) as tc, ExitStack() as ctx:
        sing = ctx.enter_context(tc.tile_pool(name="sing", bufs=1))
        wpool = ctx.enter_context(tc.tile_pool(name="w", bufs=1))
        big = ctx.enter_context(tc.tile_pool(name="big", bufs=11))
        small = ctx.enter_context(tc.tile_pool(name="small", bufs=7))
        hidp = ctx.enter_context(tc.tile_pool(name="hid", bufs=2))
        statp = ctx.enter_context(tc.tile_pool(name="stat", bufs=4))
        psmm = ctx.enter_context(tc.tile_pool(name="psmm", bufs=4, space="PSUM"))
        psst = ctx.enter_context(tc.tile_pool(name="psst", bufs=2, space="PSUM"))
        psbc = ctx.enter_context(tc.tile_pool(name="psbc", bufs=2, space="PSUM"))

        ones_col = sing.tile([P, 1], bf16)
        nc.vector.memset(ones_col, 1.0)
        ones_colf = sing.tile([P, 1], f32)
        nc.vector.memset(ones_colf, 1.0)
        ones_row = sing.tile([1, P], f32)
        nc.vector.memset(ones_row, 1.0)
        eps1 = sing.tile([1, 1], f32)
        nc.vector.memset(eps1, LN_EPS)
        dw_sb = sing.tile([P, L * DT, K], f32)
        nc.sync.dma_start(out=dw_sb, in_=dwK.ap().rearrange("l (dt p) k -> p (l dt) k", p=P))
        dwb_sb = sing.tile([P, L * DT], f32)
        nc.sync.dma_start(out=dwb_sb, in_=dwb.ap().rearrange("l (dt p) -> p (l dt)", p=P))
        pwb_sb = sing.tile([P, L * DT], f32)
        nc.sync.dma_start(out=pwb_sb, in_=pwb.ap().rearrange("l (dt p) -> p (l dt)", p=P))
        b1_sb = sing.tile([P, L * HT], f32)
        nc.sync.dma_start(out=b1_sb, in_=b1v.ap().rearrange("l (ht p) -> p (l ht)", p=P))
        b2_sb = sing.tile([P, L * DT], f32)
        nc.sync.dma_start(out=b2_sb, in_=b2v.ap().rearrange("l (dt p) -> p (l dt)", p=P))
        lng_sb = sing.tile([P, (L + 1) * DT], f32)
        nc.sync.dma_start(out=lng_sb, in_=lng.ap().rearrange("l (dt p) -> p (l dt)", p=P))
        lnb_sb = sing.tile([P, (L + 1) * DT], f32)
        nc.sync.dma_start(out=lnb_sb, in_=lnb.ap().rearrange("l (dt p) -> p (l dt)", p=P))

        def load_w(kind, dram, l, shape):
            t = wpool.tile(shape, bf16, tag=kind, name=f"{kind}{l}")
            nc.sync.dma_start(out=t, in_=dram.ap()[l].rearrange("(kt p) e -> p kt e", p=P))
            return t

        def layernorm_chunk(x_tile, slot, out_bf16):
            """x_tile: [P, DT, CH] f32; centers x_tile IN PLACE; returns LN out."""
            S_ps = psst.tile([1, CH], f32, tag="ps_stat", name="S_ps")
            if stats_fp32:
                for kt in range(DT):
                    nc.tensor.matmul(S_ps[:, :], ones_colf[:, :], x_tile[:, kt, :],
                                     start=(kt == 0), stop=(kt == DT - 1))
            else:
                x_bf = small.tile([P, DT, CH], bf16, tag="small", name="x_bf")
                for d in range(DT):
                    nc.scalar.activation(out=x_bf[:, d, :], in_=x_tile[:, d, :], func=Act.Copy)
                for kt in range(DT):
                    nc.tensor.matmul(S_ps[:, :], ones_col[:, :], x_bf[:, kt, :],
                                     start=(kt == 0), stop=(kt == DT - 1))
            S_sb = statp.tile([1, CH], f32, tag="stat", name="S_sb")
            nc.vector.tensor_copy(out=S_sb[:, :], in_=S_ps[:, :])
            muB = psbc.tile([P, CH], f32, tag="ps_bc", name="muB")
            nc.tensor.matmul(muB[:, :], ones_row[:, :], S_sb[:, :], start=True, stop=True)
            for d in range(DT):
                nc.vector.scalar_tensor_tensor(
                    x_tile[:, d, :], muB[:, :], -1.0 / D, x_tile[:, d, :], Alu.mult, Alu.add)
            Q_ps = psst.tile([1, CH], f32, tag="ps_stat", name="Q_ps")
            if q_fp32:
                xsq = big.tile([P, DT, CH], f32, tag="big", name="xsq")
                for d in range(DT):
                    nc.vector.tensor_mul(xsq[:, d, :], x_tile[:, d, :], x_tile[:, d, :])
                for kt in range(DT):
                    nc.tensor.matmul(Q_ps[:, :], ones_colf[:, :], xsq[:, kt, :],
                                     start=(kt == 0), stop=(kt == DT - 1))
            else:
                xsq = small.tile([P, DT, CH], bf16, tag="small", name="xsq")
                for d in range(DT):
                    nc.vector.tensor_mul(xsq[:, d, :], x_tile[:, d, :], x_tile[:, d, :])
                for kt in range(DT):
                    nc.tensor.matmul(Q_ps[:, :], ones_col[:, :], xsq[:, kt, :],
                                     start=(kt == 0), stop=(kt == DT - 1))
            rstd = statp.tile([1, CH], f32, tag="stat", name="rstd")
            if rstd_recip:
                sd = statp.tile([1, CH], f32, tag="stat", name="sd")
                nc.scalar.activation(out=sd[:, :], in_=Q_ps[:, :], func=Act.Sqrt,
                                     bias=eps1[:, :], scale=1.0 / D)
                nc.vector.reciprocal(out=rstd[:, :], in_=sd[:, :])
            else:
                lnv = statp.tile([1, CH], f32, tag="stat", name="lnv")
                nc.scalar.activation(out=lnv[:, :], in_=Q_ps[:, :], func=Act.Ln,
                                     bias=eps1[:, :], scale=1.0 / D)
                nc.scalar.activation(out=rstd[:, :], in_=lnv[:, :], func=Act.Exp, scale=-0.5)
            rstdB = psbc.tile([P, CH], f32, tag="ps_bc", name="rstdB")
            nc.tensor.matmul(rstdB[:, :], ones_row[:, :], rstd[:, :], start=True, stop=True)
            if out_bf16:
                a_t = small.tile([P, DT, CH], bf16, tag="small", name="a_t")
            else:
                a_t = big.tile([P, DT, CH], f32, tag="big", name="a_t")
            for d in range(DT):
                nc.vector.scalar_tensor_tensor(
                    a_t[:, d, :], x_tile[:, d, :], lng_sb[:, slot * DT + d: slot * DT + d + 1],
                    rstdB[:, :], Alu.mult, Alu.mult)
            if has_lnb:
                for d in range(DT):
                    nc.vector.tensor_scalar(
                        out=a_t[:, d, :], in0=a_t[:, d, :],
                        scalar1=lnb_sb[:, slot * DT + d: slot * DT + d + 1], scalar2=None,
                        op0=Alu.add)
            return a_t

        def mlp_chunk(a_t, l, w1_sb, w2_sb, out_tile, out_off):
            hid = hidp.tile([P, HT, CH], bf16, tag="hid", name="hid")
            for mt in range(HT):
                ps = psmm.tile([P, CH], f32, tag="mm", name="ps1")
                for kt in range(DT):
                    nc.tensor.matmul(ps[:, :], w1_sb[:, kt, bass.ts(mt, P)], a_t[:, kt, :],
                                     start=(kt == 0), stop=(kt == DT - 1))
                nc.scalar.activation(out=hid[:, mt, :], in_=ps[:, :], func=Act.Relu,
                                     bias=b1_sb[:, l * HT + mt: l * HT + mt + 1], scale=1.0)
            for mt in range(DT):
                ps = psmm.tile([P, CH], f32, tag="mm", name="ps2")
                for kt in range(HT):
                    nc.tensor.matmul(ps[:, :], w2_sb[:, kt, bass.ts(mt, P)], hid[:, kt, :],
                                     start=(kt == 0), stop=(kt == HT - 1))
                nc.scalar.activation(out=out_tile[:, mt, out_off: out_off + CH], in_=ps[:, :],
                                     func=Act.Identity,
                                     bias=b2_sb[:, l * DT + mt: l * DT + mt + 1], scale=1.0)

        def conv_chunk(m_t, l, pw_sb, want_bf):
            """m_t: [P, DT, CH+3] f32 with data at cols 3..; returns (cv f32, cv_bf)."""
            acc = big.tile([P, DT, CH], f32, tag="big", name="acc")
            y = small.tile([P, DT, CH], bf16, tag="small", name="y")
            for d in range(DT):
                nc.vector.tensor_scalar(
                    out=acc[:, d, :], in0=m_t[:, d, 0: CH],
                    scalar1=dw_sb[:, l * DT + d, 0:1], scalar2=dwb_sb[:, l * DT + d: l * DT + d + 1],
                    op0=Alu.mult, op1=Alu.add)
                for j in range(1, K - 1):
                    nc.vector.scalar_tensor_tensor(
                        acc[:, d, :], m_t[:, d, j: j + CH], dw_sb[:, l * DT + d, j: j + 1],
                        acc[:, d, :], Alu.mult, Alu.add)
                nc.vector.scalar_tensor_tensor(
                    y[:, d, :], m_t[:, d, K - 1: K - 1 + CH], dw_sb[:, l * DT + d, K - 1: K],
                    acc[:, d, :], Alu.mult, Alu.add)
            cv = big.tile([P, DT, CH], f32, tag="big", name="cv")
            cv_bf = small.tile([P, DT, CH], bf16, tag="small", name="cv_bf") if want_bf else None
            for mt in range(DT):
                ps = psmm.tile([P, CH], f32, tag="mm", name="ps3")
                for kt in range(DT):
                    nc.tensor.matmul(ps[:, :], pw_sb[:, kt, bass.ts(mt, P)], y[:, kt, :],
                                     start=(kt == 0), stop=(kt == DT - 1))
                nc.scalar.activation(out=cv[:, mt, :], in_=ps[:, :], func=Act.Identity,
                                     bias=pwb_sb[:, l * DT + mt: l * DT + mt + 1], scale=1.0)
                if want_bf:
                    nc.scalar.activation(out=cv_bf[:, mt, :], in_=ps[:, :], func=Act.Identity,
                                         bias=pwb_sb[:, l * DT + mt: l * DT + mt + 1], scale=1.0)
            return cv, cv_bf

        def gru_chunk(rhs_bf, res_t, fw_sb, h_prev):
            """kh matmul + gates + scan + residual (in place into res_t). Returns h tile."""
            z = big.tile([P, DT, CH], f32, tag="big", name="z")
            cf = big.tile([P, DT, CH], f32, tag="big", name="cf")
            s = big.tile([P, DT, CH], f32, tag="big", name="s")
            v = big.tile([P, DT, CH], f32, tag="big", name="v")
            h = big.tile([P, DT, CH], f32, tag="big", name="h")
            for mt in range(MT2):
                ps = psmm.tile([P, CH], f32, tag="mm", name="ps4")
                for kt in range(DT):
                    nc.tensor.matmul(ps[:, :], fw_sb[:, kt, bass.ts(mt, P)], rhs_bf[:, kt, :],
                                     start=(kt == 0), stop=(kt == DT - 1))
                if mt < DT:
                    nc.scalar.activation(out=z[:, mt, :], in_=ps[:, :], func=Act.Sigmoid)
                    nc.scalar.activation(out=cf[:, mt, :], in_=ps[:, :], func=Act.Sigmoid,
                                         scale=-1.0)
                else:
                    d = mt - DT
                    nc.scalar.activation(out=s[:, d, :], in_=ps[:, :], func=Act.Sigmoid)
                    nc.vector.scalar_tensor_tensor(
                        s[:, d, :], ps[:, :], 0.5, s[:, d, :], Alu.add, Alu.max)
            for d in range(DT):
                nc.vector.tensor_mul(v[:, d, :], z[:, d, :], s[:, d, :])
            for d in range(DT):
                init = 0.5 if h_prev is None else h_prev[:, d, CH - 1: CH]
                nc.vector.tensor_tensor_scan(h[:, d, :], cf[:, d, :], v[:, d, :], init,
                                             Alu.mult, Alu.add)
            for d in range(DT):
                nc.vector.tensor_add(res_t[:, d, :], h[:, d, :], res_t[:, d, :])
            return h

        # ---------- layer 0: conv0 -> ln1 -> gru0 (+ residual on ln1 out) ----------
        # software-pipelined: stage A(c) = conv+LN (PE-heavy), stage B(c) = GRU tail
        fw_sb = load_w("fw", fwT, 0, [P, DT, E2])
        pw_sb = load_w("pw", pwT, 0, [P, DT, D])

        def l0_stageA(c):
            x_in = big.tile([P, DT, CH + 3], f32, tag="big", name="x_in")
            nc.sync.dma_start(out=x_in, in_=xT.ap().rearrange("(dt p) t -> p dt t", p=P)[:, :, c * CH: c * CH + CH + 3])
            cv, _ = conv_chunk(x_in, 0, pw_sb, want_bf=False)
            n = layernorm_chunk(cv, 0, out_bf16=False)
            n_bf = small.tile([P, DT, CH], bf16, tag="small", name="n_bf")
            for d in range(DT):
                nc.scalar.activation(out=n_bf[:, d, :], in_=n[:, d, :], func=Act.Copy)
            return n, n_bf

        state = {"h": None}

        def l0_stageB(c, n, n_bf):
            state["h"] = gru_chunk(n_bf, n, fw_sb, state["h"])
            nc.sync.dma_start(out=dram3(xs[0], c, CH), in_=n)

        pend = []
        for c in range(NCH):
            pend.append((c, l0_stageA(c)))
            if len(pend) > 1:
                c0, art = pend.pop(0)
                l0_stageB(c0, *art)
        for c0, art in pend:
            l0_stageB(c0, *art)

        # ---------- mid iterations i=0..2: ln2_i, mlp_i, conv_{i+1}, gru_{i+1} ----------
        for i in range(L - 1):
            src, dst = xs[i % 2], xs[(i + 1) % 2]
            w1_sb = load_w("w1", w1T, i, [P, DT, H])
            w2_sb = load_w("w2", w2T, i, [P, HT, D])
            fw_sb = load_w("fw", fwT, i + 1, [P, DT, E2])
            pw_sb = load_w("pw", pwT, i + 1, [P, DT, D])
            state["h"] = None
            m_prev = None

            def mid_stageA(c, m_prev):
                x_in = big.tile([P, DT, CH], f32, tag="big", name="x_in")
                nc.sync.dma_start(out=x_in, in_=dram3(src, c, CH))
                a = layernorm_chunk(x_in, 1 + i, out_bf16=True)
                m = big.tile([P, DT, CH + 3], f32, tag="big", name="m")
                mlp_chunk(a, i, w1_sb, w2_sb, m, 3)
                if c == 0:
                    nc.vector.memset(m[:, :, 0:3], 0.0)
                else:
                    nc.vector.tensor_copy(out=m[:, :, 0:3], in_=m_prev[:, :, CH: CH + 3])
                return m

            def mid_stageB(c, m):
                cv, cv_bf = conv_chunk(m, i + 1, pw_sb, want_bf=True)
                state["h"] = gru_chunk(cv_bf, cv, fw_sb, state["h"])
                nc.sync.dma_start(out=dram3(dst, c, CH), in_=cv)

            pend = []
            for c in range(NCH):
                m = mid_stageA(c, m_prev)
                m_prev = m
                pend.append((c, m))
                if len(pend) > 1:
                    c0, art = pend.pop(0)
                    mid_stageB(c0, art)
            for c0, art in pend:
                mid_stageB(c0, art)

        # ---------- tail: ln2_3 + mlp_3 ----------
        src = xs[(L - 1) % 2]
        w1_sb = load_w("w1", w1T, L - 1, [P, DT, H])
        w2_sb = load_w("w2", w2T, L - 1, [P, HT, D])
        for c in range(NCH):
            x_in = big.tile([P, DT, CH], f32, tag="big", name="x_in")
            nc.sync.dma_start(out=x_in, in_=dram3(src, c, CH))
            a = layernorm_chunk(x_in, L, out_bf16=True)
            o = big.tile([P, DT, CH], f32, tag="big", name="o")
            mlp_chunk(a, L - 1, w1_sb, w2_sb, o, 0)
            nc.sync.dma_start(out=dram3(out_t, c, CH), in_=o)

    return nc


_CACHE = {}


def get_compiled_nc(T=4096, CH=512, has_lnb=False, **kw):
    key = (T, CH, has_lnb, tuple(sorted(kw.items())))
    if key not in _CACHE:
        nc = build_nc(T, CH, has_lnb, **kw)
        nc.compile()
        _CACHE[key] = nc
    return _CACHE[key]


def make_host_inputs(inputs, T=4096):
    f = np.float32
    w = {
        "fwT": np.ascontiguousarray(np.transpose(np.asarray(inputs["f_w"], f), (0, 2, 1))).astype(BF),
        "pwT": np.ascontiguousarray(np.transpose(np.asarray(inputs["conv_pw_w"], f), (0, 2, 1))).astype(BF),
        "w1T": np.ascontiguousarray(np.transpose(np.asarray(inputs["mlp_w1"], f), (0, 2, 1))).astype(BF),
        "w2T": np.ascontiguousarray(np.transpose(np.asarray(inputs["mlp_w2"], f), (0, 2, 1))).astype(BF),
        "dwK": np.ascontiguousarray(np.transpose(np.asarray(inputs["conv_dw_w"], f), (0, 2, 1))).astype(f),
        "dwb": np.asarray(inputs["conv_dw_b"], f),
        "pwb": np.asarray(inputs["conv_pw_b"], f),
        "b1v": np.asarray(inputs["mlp_b1"], f),
        "b2v": np.asarray(inputs["mlp_b2"], f),
        "lng": np.concatenate([np.asarray(inputs["ln1_g"], f)[None], np.asarray(inputs["ln2_g"], f)], 0),
        "lnb": np.concatenate([np.asarray(inputs["ln1_b"], f)[None], np.asarray(inputs["ln2_b"], f)], 0),
    }
    x = np.asarray(inputs["x"], f)
    nb = x.shape[0]
    in_maps = []
    for b in range(nb):
        xTp = np.zeros((D, T + 3), f)
        xTp[:, 3:] = x[b, :T].T
        in_maps.append({"xT": xTp, **w})
    has_lnb = bool(np.any(w["lnb"] != 0.0))
    return in_maps, has_lnb


def kernel(**inputs):
    from concourse.bass_utils import run_bass_kernel_spmd

    T = int(np.asarray(inputs["x"]).shape[1])
    in_maps, has_lnb = make_host_inputs(inputs, T)
    nc = get_compiled_nc(T=T, has_lnb=has_lnb)
    res = run_bass_kernel_spmd(nc, in_maps, core_ids=list(range(len(in_maps))))
    out = np.stack([r["out"].T for r in res.results])
    return np.ascontiguousarray(out.astype(np.float32))


# revision 14
# speedup vs baseline: 1.0680x; 1.0016x over previous
"""Trainium2 Bass kernel for nn_BlockV2 (conv -> LN -> minGRU -> MLP x4).

Strategy: data-parallel over batch (B=8 -> 8 cores). Per core, activations
are kept in [D_partitions, T_free] layout and streamed through each layer in
chunks of 512 tokens; inter-layer activations ping-pong through DRAM.
The minGRU recurrence h_t = c_t*h_{t-1} + v_t runs on the VectorE
tensor_tensor_scan instruction (fp32 state), chained across chunks.
Matmul inputs are bf16 (fp32 PSUM accumulate); everything on the
LN/scan/residual path stays fp32 (the late-layer signal is a ~5e-3
variation on an O(1) baseline, which bf16 storage would destroy).
LayerNorm is two-pass (center, then variance of centered values) to avoid
E[x^2]-mu^2 cancellation. Emission is software-pipelined: chunk c+1's
LN/MLP matmuls interleave with chunk c's conv/GRU tail so TensorE never
idles long enough to re-throttle (HAM).
"""
import sys

sys.path.insert(0, "/opt/trn_rl_repo")

from contextlib import ExitStack

import numpy as np
import ml_dtypes

import concourse.bass as bass
import concourse.tile as tile
from concourse import bacc, mybir

f32 = mybir.dt.float32
bf16 = mybir.dt.bfloat16
Alu = mybir.AluOpType
Act = mybir.ActivationFunctionType
BF = ml_dtypes.bfloat16

B, D, L, K, H = 8, 512, 4, 4, 2048
N_CORES = 8
LN_EPS = 1e-5
P = 128


def build_nc(T=4096, CH=512, has_lnb=False, stats_fp32=True, q_fp32=False,
             rstd_recip=False):
    NCH = T // CH
    DT = D // P      # 4 d-tiles
    HT = H // P      # 16 h-tiles
    E2 = 2 * D
    MT2 = E2 // P    # 8 m-tiles of the kh matmul

    nc = bacc.Bacc("TRN2", target_bir_lowering=False, debug=False)

    xT = nc.dram_tensor("xT", [D, T + 3], f32, kind="ExternalInput")
    fwT = nc.dram_tensor("fwT", [L, D, E2], bf16, kind="ExternalInput")
    pwT = nc.dram_tensor("pwT", [L, D, D], bf16, kind="ExternalInput")
    w1T = nc.dram_tensor("w1T", [L, D, H], bf16, kind="ExternalInput")
    w2T = nc.dram_tensor("w2T", [L, H, D], bf16, kind="ExternalInput")
    dwK = nc.dram_tensor("dwK", [L, D, K], f32, kind="ExternalInput")
    dwb = nc.dram_tensor("dwb", [L, D], f32, kind="ExternalInput")
    pwb = nc.dram_tensor("pwb", [L, D], f32, kind="ExternalInput")
    b1v = nc.dram_tensor("b1v", [L, H], f32, kind="ExternalInput")
    b2v = nc.dram_tensor("b2v", [L, D], f32, kind="ExternalInput")
    lng = nc.dram_tensor("lng", [L + 1, D], f32, kind="ExternalInput")
    lnb = nc.dram_tensor("lnb", [L + 1, D], f32, kind="ExternalInput")
    out_t = nc.dram_tensor("out", [D, T], f32, kind="ExternalOutput")
    xs = [nc.dram_tensor(f"xs{i}", [D, T], f32) for i in range(2)]

    def dram3(tensor, c, width):
        return tensor.ap().rearrange("(dt p) t -> p dt t", p=P)[:, :, c * CH: c * CH + width]

    with tile.TileContext(nc) as tc, ExitStack() as ctx:
        sing = ctx.enter_context(tc.tile_pool(name="sing", bufs=1))
        wpool = ctx.enter_context(tc.tile_pool(name="w", bufs=1))
        big = ctx.enter_context(tc.tile_pool(name="big", bufs=11))
        small = ctx.enter_context(tc.tile_pool(name="small", bufs=7))
        hidp = ctx.enter_context(tc.tile_pool(name="hid", bufs=2))
        statp = ctx.enter_context(tc.tile_pool(name="stat", bufs=4))
        psmm = ctx.enter_context(tc.tile_pool(name="psmm", bufs=4, space="PSUM"))
        psst = ctx.enter_context(tc.tile_pool(name="psst", bufs=2, space="PSUM"))
        psbc = ctx.enter_context(tc.tile_pool(name="psbc", bufs=2, space="PSUM"))

        ones_col = sing.tile([P, 1], bf16)
        nc.vector.memset(ones_col, 1.0)
        ones_colf = sing.tile([P, 1], f32)
        nc.vector.memset(ones_colf, 1.0)
        ones_row = sing.tile([1, P], f32)
        nc.vector.memset(ones_row, 1.0)
        eps1 = sing.tile([1, 1], f32)
        nc.vector.memset(eps1, LN_EPS)
        dw_sb = sing.tile([P, L * DT, K], f32)
        nc.sync.dma_start(out=dw_sb, in_=dwK.ap().rearrange("l (dt p) k -> p (l dt) k", p=P))
        dwb_sb = sing.tile([P, L * DT], f32)
        nc.sync.dma_start(out=dwb_sb, in_=dwb.ap().rearrange("l (dt p) -> p (l dt)", p=P))
        pwb_sb = sing.tile([P, L * DT], f32)
        nc.sync.dma_start(out=pwb_sb, in_=pwb.ap().rearrange("l (dt p) -> p (l dt)", p=P))
        b1_sb = sing.tile([P, L * HT], f32)
        nc.sync.dma_start(out=b1_sb, in_=b1v.ap().rearrange("l (ht p) -> p (l ht)", p=P))
        b2_sb = sing.tile([P, L * DT], f32)
        nc.sync.dma_start(out=b2_sb, in_=b2v.ap().rearrange("l (dt p) -> p (l dt)", p=P))
        lng_sb = sing.tile([P, (L + 1) * DT], f32)
        nc.sync.dma_start(out=lng_sb, in_=lng.ap().rearrange("l (dt p) -> p (l dt)", p=P))
        lnb_sb = sing.tile([P, (L + 1) * DT], f32)
        nc.sync.dma_start(out=lnb_sb, in_=lnb.ap().rearrange("l (dt p) -> p (l dt)", p=P))

        def load_w(kind, dram, l, shape):
            t = wpool.tile(shape, bf16, tag=kind, name=f"{kind}{l}")
            nc.sync.dma_start(out=t, in_=dram.ap()[l].rearrange("(kt p) e -> p kt e", p=P))
            return t

        def layernorm_chunk(x_tile, slot, out_bf16):
            """x_tile: [P, DT, CH] f32; centers x_tile IN PLACE; returns LN out."""
            S_ps = psst.tile([1, CH], f32, tag="ps_stat", name="S_ps")
            if stats_fp32:
                for kt in range(DT):
                    nc.tensor.matmul(S_ps[:, :], ones_colf[:, :], x_tile[:, kt, :],
                                     start=(kt == 0), stop=(kt == DT - 1))
            else:
                x_bf = small.tile([P, DT, CH], bf16, tag="small", name="x_bf")
                for d in range(DT):
                    nc.scalar.activation(out=x_bf[:, d, :], in_=x_tile[:, d, :], func=Act.Copy)
                for kt in range(DT):
                    nc.tensor.matmul(S_ps[:, :], ones_col[:, :], x_bf[:, kt, :],
                                     start=(kt == 0), stop=(kt == DT - 1))
            S_sb = statp.tile([1, CH], f32, tag="stat", name="S_sb")
            nc.vector.tensor_copy(out=S_sb[:, :], in_=S_ps[:, :])
            muB = psbc.tile([P, CH], f32, tag="ps_bc", name="muB")
            nc.tensor.matmul(muB[:, :], ones_row[:, :], S_sb[:, :], start=True, stop=True)
            for d in range(DT):
                nc.vector.scalar_tensor_tensor(
                    x_tile[:, d, :], muB[:, :], -1.0 / D, x_tile[:, d, :], Alu.mult, Alu.add)
            Q_ps = psst.tile([1, CH], f32, tag="ps_stat", name="Q_ps")
            if q_fp32:
                xsq = big.tile([P, DT, CH], f32, tag="big", name="xsq")
                for d in range(DT):
                    nc.vector.tensor_mul(xsq[:, d, :], x_tile[:, d, :], x_tile[:, d, :])
                for kt in range(DT):
                    nc.tensor.matmul(Q_ps[:, :], ones_colf[:, :], xsq[:, kt, :],
                                     start=(kt == 0), stop=(kt == DT - 1))
            else:
                xsq = small.tile([P, DT, CH], bf16, tag="small", name="xsq")
                for d in range(DT):
                    nc.vector.tensor_mul(xsq[:, d, :], x_tile[:, d, :], x_tile[:, d, :])
                for kt in range(DT):
                    nc.tensor.matmul(Q_ps[:, :], ones_col[:, :], xsq[:, kt, :],
                                     start=(kt == 0), stop=(kt == DT - 1))
            rstd = statp.tile([1, CH], f32, tag="stat", name="rstd")
            if rstd_recip:
                sd = statp.tile([1, CH], f32, tag="stat", name="sd")
                nc.scalar.activation(out=sd[:, :], in_=Q_ps[:, :], func=Act.Sqrt,
                                     bias=eps1[:, :], scale=1.0 / D)
                nc.vector.reciprocal(out=rstd[:, :], in_=sd[:, :])
            else:
                lnv = statp.tile([1, CH], f32, tag="stat", name="lnv")
                nc.scalar.activation(out=lnv[:, :], in_=Q_ps[:, :], func=Act.Ln,
                                     bias=eps1[:, :], scale=1.0 / D)
                nc.scalar.activation(out=rstd[:, :], in_=lnv[:, :], func=Act.Exp, scale=-0.5)
            rstdB = psbc.tile([P, CH], f32, tag="ps_bc", name="rstdB")
            nc.tensor.matmul(rstdB[:, :], ones_row[:, :], rstd[:, :], start=True, stop=True)
            if out_bf16:
                a_t = small.tile([P, DT, CH], bf16, tag="small", name="a_t")
            else:
                a_t = big.tile([P, DT, CH], f32, tag="big", name="a_t")
            for d in range(DT):
                nc.vector.scalar_tensor_tensor(
                    a_t[:, d, :], x_tile[:, d, :], lng_sb[:, slot * DT + d: slot * DT + d + 1],
                    rstdB[:, :], Alu.mult, Alu.mult)
            if has_lnb:
                for d in range(DT):
                    nc.vector.tensor_scalar(
                        out=a_t[:, d, :], in0=a_t[:, d, :],
                        scalar1=lnb_sb[:, slot * DT + d: slot * DT + d + 1], scalar2=None,
                        op0=Alu.add)
            return a_t

        def mlp_chunk(a_t, l, w1_sb, w2_sb, out_tile, out_off):
            hid = hidp.tile([P, HT, CH], bf16, tag="hid", name="hid")
            for mt in range(HT):
                ps = psmm.tile([P, CH], f32, tag="mm", name="ps1")
                for kt in range(DT):
                    nc.tensor.matmul(ps[:, :], w1_sb[:, kt, bass.ts(mt, P)], a_t[:, kt, :],
                                     start=(kt == 0), stop=(kt == DT - 1))
                nc.scalar.activation(out=hid[:, mt, :], in_=ps[:, :], func=Act.Relu,
                                     bias=b1_sb[:, l * HT + mt: l * HT + mt + 1], scale=1.0)
            for mt in range(DT):
                ps = psmm.tile([P, CH], f32, tag="mm", name="ps2")
                for kt in range(HT):
                    nc.tensor.matmul(ps[:, :], w2_sb[:, kt, bass.ts(mt, P)], hid[:, kt, :],
                                     start=(kt == 0), stop=(kt == HT - 1))
                nc.scalar.activation(out=out_tile[:, mt, out_off: out_off + CH], in_=ps[:, :],
                                     func=Act.Identity,
                                     bias=b2_sb[:, l * DT + mt: l * DT + mt + 1], scale=1.0)

        def conv_chunk(m_t, l, pw_sb, want_bf):
            """m_t: [P, DT, CH+3] f32 with data at cols 3..; returns (cv f32, cv_bf)."""
            acc = big.tile([P, DT, CH], f32, tag="big", name="acc")
            y = small.tile([P, DT, CH], bf16, tag="small", name="y")
            for d in range(DT):
                nc.vector.tensor_scalar(
                    out=acc[:, d, :], in0=m_t[:, d, 0: CH],
                    scalar1=dw_sb[:, l * DT + d, 0:1], scalar2=dwb_sb[:, l * DT + d: l * DT + d + 1],
                    op0=Alu.mult, op1=Alu.add)
                for j in range(1, K - 1):
                    nc.vector.scalar_tensor_tensor(
                        acc[:, d, :], m_t[:, d, j: j + CH], dw_sb[:, l * DT + d, j: j + 1],
                        acc[:, d, :], Alu.mult, Alu.add)
                nc.vector.scalar_tensor_tensor(
                    y[:, d, :], m_t[:, d, K - 1: K - 1 + CH], dw_sb[:, l * DT + d, K - 1: K],
                    acc[:, d, :], Alu.mult, Alu.add)
            cv = big.tile([P, DT, CH], f32, tag="big", name="cv")
            cv_bf = small.tile([P, DT, CH], bf16, tag="small", name="cv_bf") if want_bf else None
            for mt in range(DT):
                ps = psmm.tile([P, CH], f32, tag="mm", name="ps3")
                for kt in range(DT):
                    nc.tensor.matmul(ps[:, :], pw_sb[:, kt, bass.ts(mt, P)], y[:, kt, :],
                                     start=(kt == 0), stop=(kt == DT - 1))
                nc.scalar.activation(out=cv[:, mt, :], in_=ps[:, :], func=Act.Identity,
                                     bias=pwb_sb[:, l * DT + mt: l * DT + mt + 1], scale=1.0)
                if want_bf:
                    nc.scalar.activation(out=cv_bf[:, mt, :], in_=ps[:, :], func=Act.Identity,
                                         bias=pwb_sb[:, l * DT + mt: l * DT + mt + 1], scale=1.0)
            return cv, cv_bf

        def gru_chunk(rhs_bf, res_t, fw_sb, h_prev):
            """kh matmul + gates + scan + residual (in place into res_t). Returns h tile."""
            z = big.tile([P, DT, CH], f32, tag="big", name="z")
            cf = big.tile([P, DT, CH], f32, tag="big", name="cf")
            s = big.tile([P, DT, CH], f32, tag="big", name="s")
            v = big.tile([P, DT, CH], f32, tag="big", name="v")
            h = big.tile([P, DT, CH], f32, tag="big", name="h")
            for mt in range(MT2):
                ps = psmm.tile([P, CH], f32, tag="mm", name="ps4")
                for kt in range(DT):
                    nc.tensor.matmul(ps[:, :], fw_sb[:, kt, bass.ts(mt, P)], rhs_bf[:, kt, :],
                                     start=(kt == 0), stop=(kt == DT - 1))
                if mt < DT:
                    nc.scalar.activation(out=z[:, mt, :], in_=ps[:, :], func=Act.Sigmoid)
                    nc.scalar.activation(out=cf[:, mt, :], in_=ps[:, :], func=Act.Sigmoid,
                                         scale=-1.0)
                else:
                    d = mt - DT
                    nc.scalar.activation(out=s[:, d, :], in_=ps[:, :], func=Act.Sigmoid)
                    nc.vector.scalar_tensor_tensor(
                        s[:, d, :], ps[:, :], 0.5, s[:, d, :], Alu.add, Alu.max)
            for d in range(DT):
                nc.vector.tensor_mul(v[:, d, :], z[:, d, :], s[:, d, :])
            for d in range(DT):
                init = 0.5 if h_prev is None else h_prev[:, d, CH - 1: CH]
                nc.vector.tensor_tensor_scan(h[:, d, :], cf[:, d, :], v[:, d, :], init,
                                             Alu.mult, Alu.add)
            for d in range(DT):
                nc.vector.tensor_add(res_t[:, d, :], h[:, d, :], res_t[:, d, :])
            return h

        # ---------- layer 0: conv0 -> ln1 -> gru0 (+ residual on ln1 out) ----------
        # software-pipelined: stage A(c) = conv+LN (PE-heavy), stage B(c) = GRU tail
        fw_sb = load_w("fw", fwT, 0, [P, DT, E2])
        pw_sb = load_w("pw", pwT, 0, [P, DT, D])

        def l0_stageA(c):
            x_in = big.tile([P, DT, CH + 3], f32, tag="big", name="x_in")
            nc.sync.dma_start(out=x_in, in_=xT.ap().rearrange("(dt p) t -> p dt t", p=P)[:, :, c * CH: c * CH + CH + 3])
            cv, _ = conv_chunk(x_in, 0, pw_sb, want_bf=False)
            n = layernorm_chunk(cv, 0, out_bf16=False)
            n_bf = small.tile([P, DT, CH], bf16, tag="small", name="n_bf")
            for d in range(DT):
                nc.scalar.activation(out=n_bf[:, d, :], in_=n[:, d, :], func=Act.Copy)
            return n, n_bf

        state = {"h": None}

        def l0_stageB(c, n, n_bf):
            state["h"] = gru_chunk(n_bf, n, fw_sb, state["h"])
            nc.sync.dma_start(out=dram3(xs[0], c, CH), in_=n)

        pend = []
        for c in range(NCH):
            pend.append((c, l0_stageA(c)))
            if len(pend) > 1:
                c0, art = pend.pop(0)
                l0_stageB(c0, *art)
        for c0, art in pend:
            l0_stageB(c0, *art)

        # ---------- mid iterations i=0..2: ln2_i, mlp_i, conv_{i+1}, gru_{i+1} ----------
        for i in range(L - 1):
            src, dst = xs[i % 2], xs[(i + 1) % 2]
            w1_sb = load_w("w1", w1T, i, [P, DT, H])
            w2_sb = load_w("w2", w2T, i, [P, HT, D])
            fw_sb = load_w("fw", fwT, i + 1, [P, DT, E2])
            pw_sb = load_w("pw", pwT, i + 1, [P, DT, D])
            state["h"] = None
            m_prev = None

            def mid_stageA(c, m_prev):
                x_in = big.tile([P, DT, CH], f32, tag="big", name="x_in")
                nc.sync.dma_start(out=x_in, in_=dram3(src, c, CH))
                a = layernorm_chunk(x_in, 1 + i, out_bf16=True)
                m = big.tile([P, DT, CH + 3], f32, tag="big", name="m")
                mlp_chunk(a, i, w1_sb, w2_sb, m, 3)
                if c == 0:
                    nc.vector.memset(m[:, :, 0:3], 0.0)
                else:
                    nc.vector.tensor_copy(out=m[:, :, 0:3], in_=m_prev[:, :, CH: CH + 3])
                return m

            def mid_stageB(c, m):
                cv, cv_bf = conv_chunk(m, i + 1, pw_sb, want_bf=True)
                state["h"] = gru_chunk(cv_bf, cv, fw_sb, state["h"])
                nc.sync.dma_start(out=dram3(dst, c, CH), in_=cv)

            pend = []
            for c in range(NCH):
                m = mid_stageA(c, m_prev)
                m_prev = m
                pend.append((c, m))
                if len(pend) > 1:
                    c0, art = pend.pop(0)
                    mid_stageB(c0, art)
            for c0, art in pend:
                mid_stageB(c0, art)

        # ---------- tail: ln2_3 + mlp_3 ----------
        src = xs[(L - 1) % 2]
        w1_sb = load_w("w1", w1T, L - 1, [P, DT, H])
        w2_sb = load_w("w2", w2T, L - 1, [P, HT, D])
        for c in range(NCH):
            x_in = big.tile([P, DT, CH], f32, tag="big", name="x_in")
            nc.sync.dma_start(out=x_in, in_=dram3(src, c, CH))
            a = layernorm_chunk(x_in, L, out_bf16=True)
            o = big.tile([P, DT, CH], f32, tag="big", name="o")
            mlp_chunk(a, L - 1, w1_sb, w2_sb, o, 0)
            nc.sync.dma_start(out=dram3(out_t, c, CH), in_=o)

    return nc


_CACHE = {}


def get_compiled_nc(T=4096, CH=512, has_lnb=False, **kw):
    key = (T, CH, has_lnb, tuple(sorted(kw.items())))
    if key not in _CACHE:
        nc = build_nc(T, CH, has_lnb, **kw)
        nc.compile()
        _CACHE[key] = nc
    return _CACHE[key]


def make_host_inputs(inputs, T=4096):
    f = np.float32
    w = {
        "fwT": np.ascontiguousarray(np.transpose(np.asarray(inputs["f_w"], f), (0, 2, 1))).astype(BF),
        "pwT": np.ascontiguousarray(np.transpose(np.asarray(inputs["conv_pw_w"], f), (0, 2, 1))).astype(BF),
        "w1T": np.ascontiguousarray(np.transpose(np.asarray(inputs["mlp_w1"], f), (0, 2, 1))).astype(BF),
        "w2T": np.ascontiguousarray(np.transpose(np.asarray(inputs["mlp_w2"], f), (0, 2, 1))).astype(BF),
        "dwK": np.ascontiguousarray(np.transpose(np.asarray(inputs["conv_dw_w"], f), (0, 2, 1))).astype(f),
        "dwb": np.asarray(inputs["conv_dw_b"], f),
        "pwb": np.asarray(inputs["conv_pw_b"], f),
        "b1v": np.asarray(inputs["mlp_b1"], f),
        "b2v": np.asarray(inputs["mlp_b2"], f),
        "lng": np.concatenate([np.asarray(inputs["ln1_g"], f)[None], np.asarray(inputs["ln2_g"], f)], 0),
        "lnb": np.concatenate([np.asarray(inputs["ln1_b"], f)[None], np.asarray(inputs["ln2_b"], f)], 0),
    }
    x = np.asarray(inputs["x"], f)
    nb = x.shape[0]
    in_maps = []
    for b in range(nb):
        xTp = np.zeros((D, T + 3), f)
        xTp[:, 3:] = x[b, :T].T
        in_maps.append({"xT": xTp, **w})
    has_lnb = bool(np.any(w["lnb"] != 0.0))
    return in_maps, has_lnb


def kernel(**inputs):
    from concourse.bass_utils import run_bass_kernel_spmd

    T = int(np.asarray(inputs["x"]).shape[1])
    in_maps, has_lnb = make_host_inputs(inputs, T)
    nc = get_compiled_nc(T=T, has_lnb=has_lnb)
    res = run_bass_kernel_spmd(nc, in_maps, core_ids=list(range(len(in_maps))))
    out = np.stack([r["out"].T for r in res.results])
    return np.ascontiguousarray(out.astype(np.float32))


# revision 16
# speedup vs baseline: 1.2952x; 1.2126x over previous
"""Trainium2 Bass kernel for nn_BlockV2 (conv -> LN -> minGRU -> MLP x4).

Strategy: data-parallel over batch (B=8 -> 8 cores). Per core, activations
are kept in [D_partitions, T_free] layout and streamed through each layer in
chunks of 512 tokens; inter-layer activations ping-pong through DRAM.
The minGRU recurrence h_t = c_t*h_{t-1} + v_t runs on the VectorE
tensor_tensor_scan instruction (fp32 state), chained across chunks.
Matmul inputs are bf16 (fp32 PSUM accumulate); everything on the
LN/scan/residual path stays fp32 (the late-layer signal is a ~5e-3
variation on an O(1) baseline, which bf16 storage would destroy).
LayerNorm is two-pass (center, then variance of centered values) to avoid
E[x^2]-mu^2 cancellation. Emission is software-pipelined: chunk c+1's
LN/MLP matmuls interleave with chunk c's conv/GRU tail so TensorE never
idles long enough to re-throttle (HAM).
"""
import sys

sys.path.insert(0, "/opt/trn_rl_repo")

from contextlib import ExitStack

import numpy as np
import ml_dtypes

import concourse.bass as bass
import concourse.tile as tile
from concourse import bacc, mybir

f32 = mybir.dt.float32
bf16 = mybir.dt.bfloat16
Alu = mybir.AluOpType
Act = mybir.ActivationFunctionType
BF = ml_dtypes.bfloat16

B, D, L, K, H = 8, 512, 4, 4, 2048
N_CORES = 8
LN_EPS = 1e-5
P = 128


def build_nc(T=4096, CH=512, has_lnb=False, stats_fp32=True, q_fp32=False,
             rstd_recip=False):
    NCH = T // CH
    DT = D // P      # 4 d-tiles
    HT = H // P      # 16 h-tiles
    E2 = 2 * D
    MT2 = E2 // P    # 8 m-tiles of the kh matmul

    nc = bacc.Bacc("TRN2", target_bir_lowering=False, debug=False)

    xT = nc.dram_tensor("xT", [D, T + 3], f32, kind="ExternalInput")
    fwT = nc.dram_tensor("fwT", [L, D, E2], bf16, kind="ExternalInput")
    pwT = nc.dram_tensor("pwT", [L, D, D], bf16, kind="ExternalInput")
    w1T = nc.dram_tensor("w1T", [L, D, H], bf16, kind="ExternalInput")
    w2T = nc.dram_tensor("w2T", [L, H, D], bf16, kind="ExternalInput")
    dwK = nc.dram_tensor("dwK", [L, D, K], f32, kind="ExternalInput")
    dwb = nc.dram_tensor("dwb", [L, D], f32, kind="ExternalInput")
    pwb = nc.dram_tensor("pwb", [L, D], f32, kind="ExternalInput")
    b1v = nc.dram_tensor("b1v", [L, H], f32, kind="ExternalInput")
    b2v = nc.dram_tensor("b2v", [L, D], f32, kind="ExternalInput")
    lng = nc.dram_tensor("lng", [L + 1, D], f32, kind="ExternalInput")
    lnb = nc.dram_tensor("lnb", [L + 1, D], f32, kind="ExternalInput")
    out_t = nc.dram_tensor("out", [D, T], f32, kind="ExternalOutput")
    xs = [nc.dram_tensor(f"xs{i}", [D, T], f32) for i in range(2)]

    def dram3(tensor, c, width):
        return tensor.ap().rearrange("(dt p) t -> p dt t", p=P)[:, :, c * CH: c * CH + width]

    with tile.TileContext(nc) as tc, ExitStack() as ctx:
        sing = ctx.enter_context(tc.tile_pool(name="sing", bufs=1))
        wpool = ctx.enter_context(tc.tile_pool(name="w", bufs=1))
        big = ctx.enter_context(tc.tile_pool(name="big", bufs=11))
        small = ctx.enter_context(tc.tile_pool(name="small", bufs=7))
        hidp = ctx.enter_context(tc.tile_pool(name="hid", bufs=2))
        statp = ctx.enter_context(tc.tile_pool(name="stat", bufs=4))
        psmm = ctx.enter_context(tc.tile_pool(name="psmm", bufs=4, space="PSUM"))
        psst = ctx.enter_context(tc.tile_pool(name="psst", bufs=2, space="PSUM"))
        psbc = ctx.enter_context(tc.tile_pool(name="psbc", bufs=2, space="PSUM"))

        ones_col = sing.tile([P, 1], bf16)
        nc.vector.memset(ones_col, 1.0)
        ones_colf = sing.tile([P, 1], f32)
        nc.vector.memset(ones_colf, 1.0)
        ones_row = sing.tile([1, P], f32)
        nc.vector.memset(ones_row, 1.0)
        eps1 = sing.tile([1, 1], f32)
        nc.vector.memset(eps1, LN_EPS)
        dw_sb = sing.tile([P, L * DT, K], f32)
        nc.sync.dma_start(out=dw_sb, in_=dwK.ap().rearrange("l (dt p) k -> p (l dt) k", p=P))
        dwb_sb = sing.tile([P, L * DT], f32)
        nc.sync.dma_start(out=dwb_sb, in_=dwb.ap().rearrange("l (dt p) -> p (l dt)", p=P))
        pwb_sb = sing.tile([P, L * DT], f32)
        nc.sync.dma_start(out=pwb_sb, in_=pwb.ap().rearrange("l (dt p) -> p (l dt)", p=P))
        b1_sb = sing.tile([P, L * HT], f32)
        nc.sync.dma_start(out=b1_sb, in_=b1v.ap().rearrange("l (ht p) -> p (l ht)", p=P))
        b2_sb = sing.tile([P, L * DT], f32)
        nc.sync.dma_start(out=b2_sb, in_=b2v.ap().rearrange("l (dt p) -> p (l dt)", p=P))
        lng_sb = sing.tile([P, (L + 1) * DT], f32)
        nc.sync.dma_start(out=lng_sb, in_=lng.ap().rearrange("l (dt p) -> p (l dt)", p=P))
        lnb_sb = sing.tile([P, (L + 1) * DT], f32)
        nc.sync.dma_start(out=lnb_sb, in_=lnb.ap().rearrange("l (dt p) -> p (l dt)", p=P))

        def load_w(kind, dram, l, shape):
            t = wpool.tile(shape, bf16, tag=kind, name=f"{kind}{l}")
            nc.sync.dma_start(out=t, in_=dram.ap()[l].rearrange("(kt p) e -> p kt e", p=P))
            return t

        def ln_st1(x_tile):
            """S-MMs + evac to SBUF."""
            S_ps = psst.tile([1, CH], f32, tag="ps_stat", name="S_ps")
            for kt in range(DT):
                nc.tensor.matmul(S_ps[:, :], ones_colf[:, :], x_tile[:, kt, :],
                                 start=(kt == 0), stop=(kt == DT - 1))
            S_sb = statp.tile([1, CH], f32, tag="stat", name="S_sb")
            nc.vector.tensor_copy(out=S_sb[:, :], in_=S_ps[:, :])
            return S_ps, S_sb

        def ln_st2(x_tile, S_ps, S_sb, slot, out_bf16):
            """broadcast mu, center in place, variance (Q at partition 32 of the
            same stat bank), rstd, broadcast (same bc bank as mu), apply."""
            bc = psbc.tile([P, CH], f32, tag="ps_bc", name="bc")
            nc.tensor.matmul(bc[:, :], ones_row[:, :], S_sb[:, :], start=True, stop=True)
            for d in range(DT):
                nc.vector.scalar_tensor_tensor(
                    x_tile[:, d, :], bc[:, :], -1.0 / D, x_tile[:, d, :], Alu.mult, Alu.add)
            xsq = small.tile([P, DT, CH], bf16, tag="small", name="xsq")
            for d in range(DT):
                nc.vector.tensor_mul(xsq[:, d, :], x_tile[:, d, :], x_tile[:, d, :])
            Q_ps = psst.tile([1, CH], f32, tag="ps_stat", name="Q_ps")
            for kt in range(DT):
                nc.tensor.matmul(Q_ps[:, :], ones_col[:, :], xsq[:, kt, :],
                                 start=(kt == 0), stop=(kt == DT - 1))
            lnv = statp.tile([1, CH], f32, tag="stat", name="lnv")
            nc.scalar.activation(out=lnv[:, :], in_=Q_ps[:, :], func=Act.Ln,
                                 bias=eps1[:, :], scale=1.0 / D)
            rstd = statp.tile([1, CH], f32, tag="stat", name="rstd")
            nc.scalar.activation(out=rstd[:, :], in_=lnv[:, :], func=Act.Exp, scale=-0.5)
            nc.tensor.matmul(bc[:, :], ones_row[:, :], rstd[:, :], start=True, stop=True)
            if out_bf16:
                a_t = small.tile([P, DT, CH], bf16, tag="small", name="a_t")
            else:
                a_t = big.tile([P, DT, CH], f32, tag="big", name="a_t")
            for d in range(DT):
                nc.vector.scalar_tensor_tensor(
                    a_t[:, d, :], x_tile[:, d, :], lng_sb[:, slot * DT + d: slot * DT + d + 1],
                    bc[:, :], Alu.mult, Alu.mult)
            if has_lnb:
                for d in range(DT):
                    nc.vector.tensor_scalar(
                        out=a_t[:, d, :], in0=a_t[:, d, :],
                        scalar1=lnb_sb[:, slot * DT + d: slot * DT + d + 1], scalar2=None,
                        op0=Alu.add)
            return a_t

        def mlp_chunk(a_t, l, w1_sb, w2_sb, out_tile, out_off):
            hid = hidp.tile([P, HT, CH], bf16, tag="hid", name="hid")
            for mt in range(HT):
                ps = psmm.tile([P, CH], f32, tag="mm", name="ps1")
                for kt in range(DT):
                    nc.tensor.matmul(ps[:, :], w1_sb[:, kt, bass.ts(mt, P)], a_t[:, kt, :],
                                     start=(kt == 0), stop=(kt == DT - 1))
                nc.scalar.activation(out=hid[:, mt, :], in_=ps[:, :], func=Act.Relu,
                                     bias=b1_sb[:, l * HT + mt: l * HT + mt + 1], scale=1.0)
            for mt in range(DT):
                ps = psmm.tile([P, CH], f32, tag="mm", name="ps2")
                for kt in range(HT):
                    nc.tensor.matmul(ps[:, :], w2_sb[:, kt, bass.ts(mt, P)], hid[:, kt, :],
                                     start=(kt == 0), stop=(kt == HT - 1))
                nc.scalar.activation(out=out_tile[:, mt, out_off: out_off + CH], in_=ps[:, :],
                                     func=Act.Identity,
                                     bias=b2_sb[:, l * DT + mt: l * DT + mt + 1], scale=1.0)

        def conv_chunk(m_t, l, pw_sb, want_bf):
            """m_t: [P, DT, CH+3] f32 with data at cols 3..; returns (cv f32, cv_bf)."""
            acc = big.tile([P, DT, CH], f32, tag="big", name="acc")
            y = small.tile([P, DT, CH], bf16, tag="small", name="y")
            for d in range(DT):
                nc.vector.tensor_scalar(
                    out=acc[:, d, :], in0=m_t[:, d, 0: CH],
                    scalar1=dw_sb[:, l * DT + d, 0:1], scalar2=dwb_sb[:, l * DT + d: l * DT + d + 1],
                    op0=Alu.mult, op1=Alu.add)
                for j in range(1, K - 1):
                    nc.vector.scalar_tensor_tensor(
                        acc[:, d, :], m_t[:, d, j: j + CH], dw_sb[:, l * DT + d, j: j + 1],
                        acc[:, d, :], Alu.mult, Alu.add)
                nc.vector.scalar_tensor_tensor(
                    y[:, d, :], m_t[:, d, K - 1: K - 1 + CH], dw_sb[:, l * DT + d, K - 1: K],
                    acc[:, d, :], Alu.mult, Alu.add)
            cv = big.tile([P, DT, CH], f32, tag="big", name="cv")
            cv_bf = small.tile([P, DT, CH], bf16, tag="small", name="cv_bf") if want_bf else None
            for mt in range(DT):
                ps = psmm.tile([P, CH], f32, tag="mm", name="ps3")
                for kt in range(DT):
                    nc.tensor.matmul(ps[:, :], pw_sb[:, kt, bass.ts(mt, P)], y[:, kt, :],
                                     start=(kt == 0), stop=(kt == DT - 1))
                nc.scalar.activation(out=cv[:, mt, :], in_=ps[:, :], func=Act.Identity,
                                     bias=pwb_sb[:, l * DT + mt: l * DT + mt + 1], scale=1.0)
                if want_bf:
                    nc.scalar.activation(out=cv_bf[:, mt, :], in_=ps[:, :], func=Act.Identity,
                                         bias=pwb_sb[:, l * DT + mt: l * DT + mt + 1], scale=1.0)
            return cv, cv_bf

        def gru_chunk(rhs_bf, res_t, fw_sb, h_prev):
            """kh matmul + gates + scan + residual (in place into res_t). Returns h tile."""
            z = big.tile([P, DT, CH], f32, tag="big", name="z")
            cf = big.tile([P, DT, CH], f32, tag="big", name="cf")
            s = big.tile([P, DT, CH], f32, tag="big", name="s")
            v = big.tile([P, DT, CH], f32, tag="big", name="v")
            h = big.tile([P, DT, CH], f32, tag="big", name="h")
            for mt in range(MT2):
                ps = psmm.tile([P, CH], f32, tag="mm", name="ps4")
                for kt in range(DT):
                    nc.tensor.matmul(ps[:, :], fw_sb[:, kt, bass.ts(mt, P)], rhs_bf[:, kt, :],
                                     start=(kt == 0), stop=(kt == DT - 1))
                if mt < DT:
                    nc.scalar.activation(out=z[:, mt, :], in_=ps[:, :], func=Act.Sigmoid)
                    nc.scalar.activation(out=cf[:, mt, :], in_=ps[:, :], func=Act.Sigmoid,
                                         scale=-1.0)
                else:
                    d = mt - DT
                    nc.scalar.activation(out=s[:, d, :], in_=ps[:, :], func=Act.Sigmoid)
                    nc.vector.scalar_tensor_tensor(
                        s[:, d, :], ps[:, :], 0.5, s[:, d, :], Alu.add, Alu.max)
            for d in range(DT):
                nc.vector.tensor_mul(v[:, d, :], z[:, d, :], s[:, d, :])
            for d in range(DT):
                init = 0.5 if h_prev is None else h_prev[:, d, CH - 1: CH]
                nc.vector.tensor_tensor_scan(h[:, d, :], cf[:, d, :], v[:, d, :], init,
                                             Alu.mult, Alu.add)
            for d in range(DT):
                nc.vector.tensor_add(res_t[:, d, :], h[:, d, :], res_t[:, d, :])
            return h

        # ---------- layer 0: conv0 -> ln1 -> gru0 (+ residual on ln1 out) ----------
        # software-pipelined: stage A(c) = conv+LN (PE-heavy), stage B(c) = GRU tail
        fw_sb = load_w("fw", fwT, 0, [P, DT, E2])
        pw_sb = load_w("pw", pwT, 0, [P, DT, D])

        def l0_st1(c):
            x_in = big.tile([P, DT, CH + 3], f32, tag="big", name="x_in")
            nc.sync.dma_start(out=x_in, in_=xT.ap().rearrange("(dt p) t -> p dt t", p=P)[:, :, c * CH: c * CH + CH + 3])
            cv, _ = conv_chunk(x_in, 0, pw_sb, want_bf=False)
            return (cv,) + ln_st1(cv)

        def l0_st2(c, cv, stat_ps, S_sb):
            n = ln_st2(cv, stat_ps, S_sb, 0, out_bf16=False)
            n_bf = small.tile([P, DT, CH], bf16, tag="small", name="n_bf")
            for d in range(DT):
                nc.scalar.activation(out=n_bf[:, d, :], in_=n[:, d, :], func=Act.Copy)
            return n, n_bf

        state = {"h": None}

        def l0_stageB(c, n, n_bf):
            state["h"] = gru_chunk(n_bf, n, fw_sb, state["h"])
            nc.sync.dma_start(out=dram3(xs[0], c, CH), in_=n)

        q1, q2 = [], []
        for c in range(NCH):
            q1.append((c, l0_st1(c)))
            if len(q1) > 1:
                c1, art = q1.pop(0)
                q2.append((c1, l0_st2(c1, *art)))
            if len(q2) > 1:
                c2, art = q2.pop(0)
                l0_stageB(c2, *art)
        for c1, art in q1:
            q2.append((c1, l0_st2(c1, *art)))
        for c2, art in q2:
            l0_stageB(c2, *art)

        # ---------- mid iterations i=0..2: ln2_i, mlp_i, conv_{i+1}, gru_{i+1} ----------
        for i in range(L - 1):
            src, dst = xs[i % 2], xs[(i + 1) % 2]
            w1_sb = load_w("w1", w1T, i, [P, DT, H])
            w2_sb = load_w("w2", w2T, i, [P, HT, D])
            fw_sb = load_w("fw", fwT, i + 1, [P, DT, E2])
            pw_sb = load_w("pw", pwT, i + 1, [P, DT, D])
            state["h"] = None
            m_prev = None

            def mid_st1(c):
                x_in = big.tile([P, DT, CH], f32, tag="big", name="x_in")
                nc.sync.dma_start(out=x_in, in_=dram3(src, c, CH))
                return (x_in,) + ln_st1(x_in)

            def mid_st2(c, x_in, stat_ps, S_sb):
                return ln_st2(x_in, stat_ps, S_sb, 1 + i, out_bf16=True)

            def mid_stageA(c, a, m_prev):
                m = big.tile([P, DT, CH + 3], f32, tag="big", name="m")
                mlp_chunk(a, i, w1_sb, w2_sb, m, 3)
                if c == 0:
                    nc.vector.memset(m[:, :, 0:3], 0.0)
                else:
                    nc.vector.tensor_copy(out=m[:, :, 0:3], in_=m_prev[:, :, CH: CH + 3])
                return m

            def mid_stageB(c, m):
                cv, cv_bf = conv_chunk(m, i + 1, pw_sb, want_bf=True)
                state["h"] = gru_chunk(cv_bf, cv, fw_sb, state["h"])
                nc.sync.dma_start(out=dram3(dst, c, CH), in_=cv)

            q1, q2, qa = [], [], []
            for c in range(NCH):
                q1.append((c, mid_st1(c)))
                if len(q1) > 1:
                    c1, art = q1.pop(0)
                    q2.append((c1, mid_st2(c1, *art)))
                if len(q2) > 1:
                    c2, a = q2.pop(0)
                    m = mid_stageA(c2, a, m_prev)
                    m_prev = m
                    qa.append((c2, m))
                if len(qa) > 1:
                    c3, m3 = qa.pop(0)
                    mid_stageB(c3, m3)
            for c1, art in q1:
                q2.append((c1, mid_st2(c1, *art)))
            for c2, a in q2:
                m = mid_stageA(c2, a, m_prev)
                m_prev = m
                qa.append((c2, m))
            for c3, m3 in qa:
                mid_stageB(c3, m3)

        # ---------- tail: ln2_3 + mlp_3 ----------
        src = xs[(L - 1) % 2]
        w1_sb = load_w("w1", w1T, L - 1, [P, DT, H])
        w2_sb = load_w("w2", w2T, L - 1, [P, HT, D])
        def tail_st1(c):
            x_in = big.tile([P, DT, CH], f32, tag="big", name="x_in")
            nc.sync.dma_start(out=x_in, in_=dram3(src, c, CH))
            return (x_in,) + ln_st1(x_in)

        def tail_rest(c, x_in, stat_ps, S_sb):
            a = ln_st2(x_in, stat_ps, S_sb, L, out_bf16=True)
            o = big.tile([P, DT, CH], f32, tag="big", name="o")
            mlp_chunk(a, L - 1, w1_sb, w2_sb, o, 0)
            nc.sync.dma_start(out=dram3(out_t, c, CH), in_=o)

        q1 = []
        for c in range(NCH):
            q1.append((c, tail_st1(c)))
            if len(q1) > 1:
                c1, art = q1.pop(0)
                tail_rest(c1, *art)
        for c1, art in q1:
            tail_rest(c1, *art)

    return nc


_CACHE = {}


def get_compiled_nc(T=4096, CH=512, has_lnb=False, **kw):
    key = (T, CH, has_lnb, tuple(sorted(kw.items())))
    if key not in _CACHE:
        nc = build_nc(T, CH, has_lnb, **kw)
        nc.compile()
        _CACHE[key] = nc
    return _CACHE[key]


def make_host_inputs(inputs, T=4096):
    f = np.float32
    w = {
        "fwT": np.ascontiguousarray(np.transpose(np.asarray(inputs["f_w"], f), (0, 2, 1))).astype(BF),
        "pwT": np.ascontiguousarray(np.transpose(np.asarray(inputs["conv_pw_w"], f), (0, 2, 1))).astype(BF),
        "w1T": np.ascontiguousarray(np.transpose(np.asarray(inputs["mlp_w1"], f), (0, 2, 1))).astype(BF),
        "w2T": np.ascontiguousarray(np.transpose(np.asarray(inputs["mlp_w2"], f), (0, 2, 1))).astype(BF),
        "dwK": np.ascontiguousarray(np.transpose(np.asarray(inputs["conv_dw_w"], f), (0, 2, 1))).astype(f),
        "dwb": np.asarray(inputs["conv_dw_b"], f),
        "pwb": np.asarray(inputs["conv_pw_b"], f),
        "b1v": np.asarray(inputs["mlp_b1"], f),
        "b2v": np.asarray(inputs["mlp_b2"], f),
        "lng": np.concatenate([np.asarray(inputs["ln1_g"], f)[None], np.asarray(inputs["ln2_g"], f)], 0),
        "lnb": np.concatenate([np.asarray(inputs["ln1_b"], f)[None], np.asarray(inputs["ln2_b"], f)], 0),
    }
    x = np.asarray(inputs["x"], f)
    nb = x.shape[0]
    in_maps = []
    for b in range(nb):
        xTp = np.zeros((D, T + 3), f)
        xTp[:, 3:] = x[b, :T].T
        in_maps.append({"xT": xTp, **w})
    has_lnb = bool(np.any(w["lnb"] != 0.0))
    return in_maps, has_lnb


def kernel(**inputs):
    from concourse.bass_utils import run_bass_kernel_spmd

    T = int(np.asarray(inputs["x"]).shape[1])
    in_maps, has_lnb = make_host_inputs(inputs, T)
    nc = get_compiled_nc(T=T, has_lnb=has_lnb)
    res = run_bass_kernel_spmd(nc, in_maps, core_ids=list(range(len(in_maps))))
    out = np.stack([r["out"].T for r in res.results])
    return np.ascontiguousarray(out.astype(np.float32))


# revision 17
# speedup vs baseline: 1.3101x; 1.0115x over previous
"""Trainium2 Bass kernel for nn_BlockV2 (conv -> LN -> minGRU -> MLP x4).

Strategy: data-parallel over batch (B=8 -> 8 cores). Per core, activations
are kept in [D_partitions, T_free] layout and streamed through each layer in
chunks of 512 tokens; inter-layer activations ping-pong through DRAM.
The minGRU recurrence h_t = c_t*h_{t-1} + v_t runs on the VectorE
tensor_tensor_scan instruction (fp32 state), chained across chunks.
Matmul inputs are bf16 (fp32 PSUM accumulate); everything on the
LN/scan/residual path stays fp32 (the late-layer signal is a ~5e-3
variation on an O(1) baseline, which bf16 storage would destroy).
LayerNorm is two-pass (center, then variance of centered values) to avoid
E[x^2]-mu^2 cancellation. Emission is software-pipelined: chunk c+1's
LN/MLP matmuls interleave with chunk c's conv/GRU tail so TensorE never
idles long enough to re-throttle (HAM).
"""
import sys

sys.path.insert(0, "/opt/trn_rl_repo")

from contextlib import ExitStack

import numpy as np
import ml_dtypes

import concourse.bass as bass
import concourse.tile as tile
from concourse import bacc, mybir

f32 = mybir.dt.float32
bf16 = mybir.dt.bfloat16
Alu = mybir.AluOpType
Act = mybir.ActivationFunctionType
BF = ml_dtypes.bfloat16

B, D, L, K, H = 8, 512, 4, 4, 2048
N_CORES = 8
LN_EPS = 1e-5
P = 128


def build_nc(T=4096, CH=512, has_lnb=False, stats_fp32=True, q_fp32=False,
             rstd_recip=False):
    NCH = T // CH
    DT = D // P      # 4 d-tiles
    HT = H // P      # 16 h-tiles
    E2 = 2 * D
    MT2 = E2 // P    # 8 m-tiles of the kh matmul

    nc = bacc.Bacc("TRN2", target_bir_lowering=False, debug=False)

    xT = nc.dram_tensor("xT", [D, T + 3], f32, kind="ExternalInput")
    fwT = nc.dram_tensor("fwT", [L, D, E2], bf16, kind="ExternalInput")
    pwT = nc.dram_tensor("pwT", [L, D, D], bf16, kind="ExternalInput")
    w1T = nc.dram_tensor("w1T", [L, D, H], bf16, kind="ExternalInput")
    w2T = nc.dram_tensor("w2T", [L, H, D], bf16, kind="ExternalInput")
    dwK = nc.dram_tensor("dwK", [L, D, K], f32, kind="ExternalInput")
    dwb = nc.dram_tensor("dwb", [L, D], f32, kind="ExternalInput")
    pwb = nc.dram_tensor("pwb", [L, D], f32, kind="ExternalInput")
    b1v = nc.dram_tensor("b1v", [L, H], f32, kind="ExternalInput")
    b2v = nc.dram_tensor("b2v", [L, D], f32, kind="ExternalInput")
    lng = nc.dram_tensor("lng", [L + 1, D], f32, kind="ExternalInput")
    lnb = nc.dram_tensor("lnb", [L + 1, D], f32, kind="ExternalInput")
    out_t = nc.dram_tensor("out", [D, T], f32, kind="ExternalOutput")
    xs = [nc.dram_tensor(f"xs{i}", [D, T], f32) for i in range(2)]

    def dram3(tensor, c, width):
        return tensor.ap().rearrange("(dt p) t -> p dt t", p=P)[:, :, c * CH: c * CH + width]

    with tile.TileContext(nc) as tc, ExitStack() as ctx:
        sing = ctx.enter_context(tc.tile_pool(name="sing", bufs=1))
        wpool = ctx.enter_context(tc.tile_pool(name="w", bufs=1))
        big = ctx.enter_context(tc.tile_pool(name="big", bufs=11))
        small = ctx.enter_context(tc.tile_pool(name="small", bufs=7))
        hidp = ctx.enter_context(tc.tile_pool(name="hid", bufs=2))
        statp = ctx.enter_context(tc.tile_pool(name="stat", bufs=4))
        psmm = ctx.enter_context(tc.tile_pool(name="psmm", bufs=4, space="PSUM"))
        psst = ctx.enter_context(tc.tile_pool(name="psst", bufs=2, space="PSUM"))
        psbc = ctx.enter_context(tc.tile_pool(name="psbc", bufs=2, space="PSUM"))

        ones_col = sing.tile([P, 1], bf16)
        nc.vector.memset(ones_col, 1.0)
        ones_colf = sing.tile([P, 1], f32)
        nc.vector.memset(ones_colf, 1.0)
        ones_row = sing.tile([1, P], f32)
        nc.vector.memset(ones_row, 1.0)
        eps1 = sing.tile([1, 1], f32)
        nc.vector.memset(eps1, LN_EPS)
        dw_sb = sing.tile([P, L * DT, K], f32)
        nc.sync.dma_start(out=dw_sb, in_=dwK.ap().rearrange("l (dt p) k -> p (l dt) k", p=P))
        dwb_sb = sing.tile([P, L * DT], f32)
        nc.sync.dma_start(out=dwb_sb, in_=dwb.ap().rearrange("l (dt p) -> p (l dt)", p=P))
        pwb_sb = sing.tile([P, L * DT], f32)
        nc.sync.dma_start(out=pwb_sb, in_=pwb.ap().rearrange("l (dt p) -> p (l dt)", p=P))
        b1_sb = sing.tile([P, L * HT], f32)
        nc.sync.dma_start(out=b1_sb, in_=b1v.ap().rearrange("l (ht p) -> p (l ht)", p=P))
        b2_sb = sing.tile([P, L * DT], f32)
        nc.sync.dma_start(out=b2_sb, in_=b2v.ap().rearrange("l (dt p) -> p (l dt)", p=P))
        lng_sb = sing.tile([P, (L + 1) * DT], f32)
        nc.sync.dma_start(out=lng_sb, in_=lng.ap().rearrange("l (dt p) -> p (l dt)", p=P))
        lnb_sb = sing.tile([P, (L + 1) * DT], f32)
        nc.sync.dma_start(out=lnb_sb, in_=lnb.ap().rearrange("l (dt p) -> p (l dt)", p=P))

        def load_w(kind, dram, l, shape):
            t = wpool.tile(shape, bf16, tag=kind, name=f"{kind}{l}")
            nc.sync.dma_start(out=t, in_=dram.ap()[l].rearrange("(kt p) e -> p kt e", p=P))
            return t

        def ln_st1(x_tile):
            """S-MMs + evac to SBUF."""
            S_ps = psst.tile([1, CH], f32, tag="ps_stat", name="S_ps")
            for kt in range(DT):
                nc.tensor.matmul(S_ps[:, :], ones_colf[:, :], x_tile[:, kt, :],
                                 start=(kt == 0), stop=(kt == DT - 1))
            S_sb = statp.tile([1, CH], f32, tag="stat", name="S_sb")
            nc.vector.tensor_copy(out=S_sb[:, :], in_=S_ps[:, :])
            return S_ps, S_sb

        def ln_st2(x_tile, S_ps, S_sb, slot, out_bf16):
            """broadcast mu, center in place, variance (Q at partition 32 of the
            same stat bank), rstd, broadcast (same bc bank as mu), apply."""
            bc = psbc.tile([P, CH], f32, tag="ps_bc", name="bc")
            nc.tensor.matmul(bc[:, :], ones_row[:, :], S_sb[:, :], start=True, stop=True)
            for d in range(DT):
                nc.vector.scalar_tensor_tensor(
                    x_tile[:, d, :], bc[:, :], -1.0 / D, x_tile[:, d, :], Alu.mult, Alu.add)
            xsq = small.tile([P, DT, CH], bf16, tag="small", name="xsq")
            for d in range(DT):
                nc.vector.tensor_mul(xsq[:, d, :], x_tile[:, d, :], x_tile[:, d, :])
            Q_ps = psst.tile([1, CH], f32, tag="ps_stat", name="Q_ps")
            for kt in range(DT):
                nc.tensor.matmul(Q_ps[:, :], ones_col[:, :], xsq[:, kt, :],
                                 start=(kt == 0), stop=(kt == DT - 1))
            lnv = statp.tile([1, CH], f32, tag="stat", name="lnv")
            nc.scalar.activation(out=lnv[:, :], in_=Q_ps[:, :], func=Act.Ln,
                                 bias=eps1[:, :], scale=1.0 / D)
            rstd = statp.tile([1, CH], f32, tag="stat", name="rstd")
            nc.scalar.activation(out=rstd[:, :], in_=lnv[:, :], func=Act.Exp, scale=-0.5)
            nc.tensor.matmul(bc[:, :], ones_row[:, :], rstd[:, :], start=True, stop=True)
            if out_bf16:
                a_t = small.tile([P, DT, CH], bf16, tag="small", name="a_t")
            else:
                a_t = big.tile([P, DT, CH], f32, tag="big", name="a_t")
            for d in range(DT):
                nc.vector.scalar_tensor_tensor(
                    a_t[:, d, :], x_tile[:, d, :], lng_sb[:, slot * DT + d: slot * DT + d + 1],
                    bc[:, :], Alu.mult, Alu.mult)
            if has_lnb:
                for d in range(DT):
                    nc.vector.tensor_scalar(
                        out=a_t[:, d, :], in0=a_t[:, d, :],
                        scalar1=lnb_sb[:, slot * DT + d: slot * DT + d + 1], scalar2=None,
                        op0=Alu.add)
            return a_t

        def mlp_chunk(a_t, l, w1_sb, w2_sb, out_tile, out_off):
            hid = hidp.tile([P, HT, CH], bf16, tag="hid", name="hid")
            for mt in range(HT):
                ps = psmm.tile([P, CH], f32, tag="mm", name="ps1")
                for kt in range(DT):
                    nc.tensor.matmul(ps[:, :], w1_sb[:, kt, bass.ts(mt, P)], a_t[:, kt, :],
                                     start=(kt == 0), stop=(kt == DT - 1))
                nc.scalar.activation(out=hid[:, mt, :], in_=ps[:, :], func=Act.Relu,
                                     bias=b1_sb[:, l * HT + mt: l * HT + mt + 1], scale=1.0)
            for mt in range(DT):
                ps = psmm.tile([P, CH], f32, tag="mm", name="ps2")
                for kt in range(HT):
                    nc.tensor.matmul(ps[:, :], w2_sb[:, kt, bass.ts(mt, P)], hid[:, kt, :],
                                     start=(kt == 0), stop=(kt == HT - 1))
                nc.scalar.activation(out=out_tile[:, mt, out_off: out_off + CH], in_=ps[:, :],
                                     func=Act.Identity,
                                     bias=b2_sb[:, l * DT + mt: l * DT + mt + 1], scale=1.0)

        def conv_dw(m_t, l):
            acc = big.tile([P, DT, CH], f32, tag="big", name="acc")
            y = small.tile([P, DT, CH], bf16, tag="small", name="y")
            for d in range(DT):
                nc.vector.tensor_scalar(
                    out=acc[:, d, :], in0=m_t[:, d, 0: CH],
                    scalar1=dw_sb[:, l * DT + d, 0:1], scalar2=dwb_sb[:, l * DT + d: l * DT + d + 1],
                    op0=Alu.mult, op1=Alu.add)
                for j in range(1, K - 1):
                    nc.vector.scalar_tensor_tensor(
                        acc[:, d, :], m_t[:, d, j: j + CH], dw_sb[:, l * DT + d, j: j + 1],
                        acc[:, d, :], Alu.mult, Alu.add)
                nc.vector.scalar_tensor_tensor(
                    y[:, d, :], m_t[:, d, K - 1: K - 1 + CH], dw_sb[:, l * DT + d, K - 1: K],
                    acc[:, d, :], Alu.mult, Alu.add)
            return y

        def conv_pw(y, l, pw_sb, want_bf):
            cv = big.tile([P, DT, CH], f32, tag="big", name="cv")
            cv_bf = small.tile([P, DT, CH], bf16, tag="small", name="cv_bf") if want_bf else None
            for mt in range(DT):
                ps = psmm.tile([P, CH], f32, tag="mm", name="ps3")
                for kt in range(DT):
                    nc.tensor.matmul(ps[:, :], pw_sb[:, kt, bass.ts(mt, P)], y[:, kt, :],
                                     start=(kt == 0), stop=(kt == DT - 1))
                nc.scalar.activation(out=cv[:, mt, :], in_=ps[:, :], func=Act.Identity,
                                     bias=pwb_sb[:, l * DT + mt: l * DT + mt + 1], scale=1.0)
                if want_bf:
                    nc.scalar.activation(out=cv_bf[:, mt, :], in_=ps[:, :], func=Act.Identity,
                                         bias=pwb_sb[:, l * DT + mt: l * DT + mt + 1], scale=1.0)
            return cv, cv_bf

        def conv_chunk(m_t, l, pw_sb, want_bf):
            return conv_pw(conv_dw(m_t, l), l, pw_sb, want_bf)

        def gru_chunk(rhs_bf, res_t, fw_sb, h_prev):
            """kh matmul + gates + scan + residual (in place into res_t). Returns h tile."""
            z = big.tile([P, DT, CH], f32, tag="big", name="z")
            cf = big.tile([P, DT, CH], f32, tag="big", name="cf")
            s = big.tile([P, DT, CH], f32, tag="big", name="s")
            v = big.tile([P, DT, CH], f32, tag="big", name="v")
            h = big.tile([P, DT, CH], f32, tag="big", name="h")
            for mt in range(MT2):
                ps = psmm.tile([P, CH], f32, tag="mm", name="ps4")
                for kt in range(DT):
                    nc.tensor.matmul(ps[:, :], fw_sb[:, kt, bass.ts(mt, P)], rhs_bf[:, kt, :],
                                     start=(kt == 0), stop=(kt == DT - 1))
                if mt < DT:
                    nc.scalar.activation(out=z[:, mt, :], in_=ps[:, :], func=Act.Sigmoid)
                    nc.scalar.activation(out=cf[:, mt, :], in_=ps[:, :], func=Act.Sigmoid,
                                         scale=-1.0)
                else:
                    d = mt - DT
                    nc.scalar.activation(out=s[:, d, :], in_=ps[:, :], func=Act.Sigmoid)
                    nc.vector.scalar_tensor_tensor(
                        s[:, d, :], ps[:, :], 0.5, s[:, d, :], Alu.add, Alu.max)
            for d in range(DT):
                nc.vector.tensor_mul(v[:, d, :], z[:, d, :], s[:, d, :])
            for d in range(DT):
                init = 0.5 if h_prev is None else h_prev[:, d, CH - 1: CH]
                nc.vector.tensor_tensor_scan(h[:, d, :], cf[:, d, :], v[:, d, :], init,
                                             Alu.mult, Alu.add)
            for d in range(DT):
                nc.vector.tensor_add(res_t[:, d, :], h[:, d, :], res_t[:, d, :])
            return h

        # ---------- layer 0: conv0 -> ln1 -> gru0 (+ residual on ln1 out) ----------
        # software-pipelined: stage A(c) = conv+LN (PE-heavy), stage B(c) = GRU tail
        fw_sb = load_w("fw", fwT, 0, [P, DT, E2])
        pw_sb = load_w("pw", pwT, 0, [P, DT, D])

        def l0_s0(c):
            x_in = big.tile([P, DT, CH + 3], f32, tag="big", name="x_in")
            nc.sync.dma_start(out=x_in, in_=xT.ap().rearrange("(dt p) t -> p dt t", p=P)[:, :, c * CH: c * CH + CH + 3])
            return conv_dw(x_in, 0)

        def l0_st1(c, y):
            cv, _ = conv_pw(y, 0, pw_sb, want_bf=False)
            return (cv,) + ln_st1(cv)

        def l0_st2(c, cv, stat_ps, S_sb):
            n = ln_st2(cv, stat_ps, S_sb, 0, out_bf16=False)
            n_bf = small.tile([P, DT, CH], bf16, tag="small", name="n_bf")
            for d in range(DT):
                nc.scalar.activation(out=n_bf[:, d, :], in_=n[:, d, :], func=Act.Copy)
            return n, n_bf

        state = {"h": None}

        def l0_stageB(c, n, n_bf):
            state["h"] = gru_chunk(n_bf, n, fw_sb, state["h"])
            nc.sync.dma_start(out=dram3(xs[0], c, CH), in_=n)

        q0, q1, q2 = [], [], []
        for c in range(NCH):
            q0.append((c, l0_s0(c)))
            if len(q0) > 1:
                c0, y0 = q0.pop(0)
                q1.append((c0, l0_st1(c0, y0)))
            if len(q1) > 1:
                c1, art = q1.pop(0)
                q2.append((c1, l0_st2(c1, *art)))
            if len(q2) > 1:
                c2, art = q2.pop(0)
                l0_stageB(c2, *art)
        for c0, y0 in q0:
            q1.append((c0, l0_st1(c0, y0)))
        for c1, art in q1:
            q2.append((c1, l0_st2(c1, *art)))
        for c2, art in q2:
            l0_stageB(c2, *art)

        # ---------- mid iterations i=0..2: ln2_i, mlp_i, conv_{i+1}, gru_{i+1} ----------
        for i in range(L - 1):
            src, dst = xs[i % 2], xs[(i + 1) % 2]
            w1_sb = load_w("w1", w1T, i, [P, DT, H])
            w2_sb = load_w("w2", w2T, i, [P, HT, D])
            fw_sb = load_w("fw", fwT, i + 1, [P, DT, E2])
            pw_sb = load_w("pw", pwT, i + 1, [P, DT, D])
            state["h"] = None
            m_prev = None

            def mid_st1(c):
                x_in = big.tile([P, DT, CH], f32, tag="big", name="x_in")
                nc.sync.dma_start(out=x_in, in_=dram3(src, c, CH))
                return (x_in,) + ln_st1(x_in)

            def mid_st2(c, x_in, stat_ps, S_sb):
                return ln_st2(x_in, stat_ps, S_sb, 1 + i, out_bf16=True)

            def mid_stageA(c, a, m_prev):
                m = big.tile([P, DT, CH + 3], f32, tag="big", name="m")
                mlp_chunk(a, i, w1_sb, w2_sb, m, 3)
                if c == 0:
                    nc.vector.memset(m[:, :, 0:3], 0.0)
                else:
                    nc.vector.tensor_copy(out=m[:, :, 0:3], in_=m_prev[:, :, CH: CH + 3])
                return m

            def mid_stageB(c, m):
                cv, cv_bf = conv_chunk(m, i + 1, pw_sb, want_bf=True)
                state["h"] = gru_chunk(cv_bf, cv, fw_sb, state["h"])
                nc.sync.dma_start(out=dram3(dst, c, CH), in_=cv)

            q1, q2, qa = [], [], []
            for c in range(NCH):
                q1.append((c, mid_st1(c)))
                if len(q1) > 1:
                    c1, art = q1.pop(0)
                    q2.append((c1, mid_st2(c1, *art)))
                if len(q2) > 1:
                    c2, a = q2.pop(0)
                    m = mid_stageA(c2, a, m_prev)
                    m_prev = m
                    qa.append((c2, m))
                if len(qa) > 1:
                    c3, m3 = qa.pop(0)
                    mid_stageB(c3, m3)
            for c1, art in q1:
                q2.append((c1, mid_st2(c1, *art)))
            for c2, a in q2:
                m = mid_stageA(c2, a, m_prev)
                m_prev = m
                qa.append((c2, m))
            for c3, m3 in qa:
                mid_stageB(c3, m3)

        # ---------- tail: ln2_3 + mlp_3 ----------
        src = xs[(L - 1) % 2]
        w1_sb = load_w("w1", w1T, L - 1, [P, DT, H])
        w2_sb = load_w("w2", w2T, L - 1, [P, HT, D])
        def tail_st1(c):
            x_in = big.tile([P, DT, CH], f32, tag="big", name="x_in")
            nc.sync.dma_start(out=x_in, in_=dram3(src, c, CH))
            return (x_in,) + ln_st1(x_in)

        def tail_rest(c, x_in, stat_ps, S_sb):
            a = ln_st2(x_in, stat_ps, S_sb, L, out_bf16=True)
            o = big.tile([P, DT, CH], f32, tag="big", name="o")
            mlp_chunk(a, L - 1, w1_sb, w2_sb, o, 0)
            nc.sync.dma_start(out=dram3(out_t, c, CH), in_=o)

        q1 = []
        for c in range(NCH):
            q1.append((c, tail_st1(c)))
            if len(q1) > 1:
                c1, art = q1.pop(0)
                tail_rest(c1, *art)
        for c1, art in q1:
            tail_rest(c1, *art)

    return nc


_CACHE = {}


def get_compiled_nc(T=4096, CH=512, has_lnb=False, **kw):
    key = (T, CH, has_lnb, tuple(sorted(kw.items())))
    if key not in _CACHE:
        nc = build_nc(T, CH, has_lnb, **kw)
        nc.compile()
        _CACHE[key] = nc
    return _CACHE[key]


def make_host_inputs(inputs, T=4096):
    f = np.float32
    w = {
        "fwT": np.ascontiguousarray(np.transpose(np.asarray(inputs["f_w"], f), (0, 2, 1))).astype(BF),
        "pwT": np.ascontiguousarray(np.transpose(np.asarray(inputs["conv_pw_w"], f), (0, 2, 1))).astype(BF),
        "w1T": np.ascontiguousarray(np.transpose(np.asarray(inputs["mlp_w1"], f), (0, 2, 1))).astype(BF),
        "w2T": np.ascontiguousarray(np.transpose(np.asarray(inputs["mlp_w2"], f), (0, 2, 1))).astype(BF),
        "dwK": np.ascontiguousarray(np.transpose(np.asarray(inputs["conv_dw_w"], f), (0, 2, 1))).astype(f),
        "dwb": np.asarray(inputs["conv_dw_b"], f),
        "pwb": np.asarray(inputs["conv_pw_b"], f),
        "b1v": np.asarray(inputs["mlp_b1"], f),
        "b2v": np.asarray(inputs["mlp_b2"], f),
        "lng": np.concatenate([np.asarray(inputs["ln1_g"], f)[None], np.asarray(inputs["ln2_g"], f)], 0),
        "lnb": np.concatenate([np.asarray(inputs["ln1_b"], f)[None], np.asarray(inputs["ln2_b"], f)], 0),
    }
    x = np.asarray(inputs["x"], f)
    nb = x.shape[0]
    in_maps = []
    for b in range(nb):
        xTp = np.zeros((D, T + 3), f)
        xTp[:, 3:] = x[b, :T].T
        in_maps.append({"xT": xTp, **w})
    has_lnb = bool(np.any(w["lnb"] != 0.0))
    return in_maps, has_lnb


def kernel(**inputs):
    from concourse.bass_utils import run_bass_kernel_spmd

    T = int(np.asarray(inputs["x"]).shape[1])
    in_maps, has_lnb = make_host_inputs(inputs, T)
    nc = get_compiled_nc(T=T, has_lnb=has_lnb)
    res = run_bass_kernel_spmd(nc, in_maps, core_ids=list(range(len(in_maps))))
    out = np.stack([r["out"].T for r in res.results])
    return np.ascontiguousarray(out.astype(np.float32))


# revision 18
# speedup vs baseline: 1.3837x; 1.0562x over previous
"""Trainium2 Bass kernel for nn_BlockV2 (conv -> LN -> minGRU -> MLP x4).

Strategy: data-parallel over batch (B=8 -> 8 cores). Per core, activations
are kept in [D_partitions, T_free] layout and streamed through each layer in
chunks of 512 tokens; inter-layer activations ping-pong through DRAM.
The minGRU recurrence h_t = c_t*h_{t-1} + v_t runs on the VectorE
tensor_tensor_scan instruction (fp32 state), chained across chunks.
Matmul inputs are bf16 (fp32 PSUM accumulate); everything on the
LN/scan/residual path stays fp32 (the late-layer signal is a ~5e-3
variation on an O(1) baseline, which bf16 storage would destroy).
LayerNorm is two-pass (center, then variance of centered values) to avoid
E[x^2]-mu^2 cancellation. Emission is software-pipelined: chunk c+1's
LN/MLP matmuls interleave with chunk c's conv/GRU tail so TensorE never
idles long enough to re-throttle (HAM).
"""
import sys

sys.path.insert(0, "/opt/trn_rl_repo")

from contextlib import ExitStack

import numpy as np
import ml_dtypes

import concourse.bass as bass
import concourse.tile as tile
from concourse import bacc, mybir

f32 = mybir.dt.float32
bf16 = mybir.dt.bfloat16
Alu = mybir.AluOpType
Act = mybir.ActivationFunctionType
BF = ml_dtypes.bfloat16

B, D, L, K, H = 8, 512, 4, 4, 2048
N_CORES = 8
LN_EPS = 1e-5
P = 128


def build_nc(T=4096, CH=512, has_lnb=False, stats_fp32=True, q_fp32=False,
             rstd_recip=False):
    NCH = T // CH
    DT = D // P      # 4 d-tiles
    HT = H // P      # 16 h-tiles
    E2 = 2 * D
    MT2 = E2 // P    # 8 m-tiles of the kh matmul

    nc = bacc.Bacc("TRN2", target_bir_lowering=False, debug=False)

    xT = nc.dram_tensor("xT", [D, T + 3], f32, kind="ExternalInput")
    fwT = nc.dram_tensor("fwT", [L, D, E2], bf16, kind="ExternalInput")
    pwT = nc.dram_tensor("pwT", [L, D, D], bf16, kind="ExternalInput")
    w1T = nc.dram_tensor("w1T", [L, D, H], bf16, kind="ExternalInput")
    w2T = nc.dram_tensor("w2T", [L, H, D], bf16, kind="ExternalInput")
    dwK = nc.dram_tensor("dwK", [L, D, K], f32, kind="ExternalInput")
    dwb = nc.dram_tensor("dwb", [L, D], f32, kind="ExternalInput")
    pwb = nc.dram_tensor("pwb", [L, D], f32, kind="ExternalInput")
    b1v = nc.dram_tensor("b1v", [L, H], f32, kind="ExternalInput")
    b2v = nc.dram_tensor("b2v", [L, D], f32, kind="ExternalInput")
    lng = nc.dram_tensor("lng", [L + 1, D], f32, kind="ExternalInput")
    lnb = nc.dram_tensor("lnb", [L + 1, D], f32, kind="ExternalInput")
    out_t = nc.dram_tensor("out", [D, T], f32, kind="ExternalOutput")
    xs = [nc.dram_tensor(f"xs{i}", [D, T], f32) for i in range(2)]

    def dram3(tensor, c, width):
        return tensor.ap().rearrange("(dt p) t -> p dt t", p=P)[:, :, c * CH: c * CH + width]

    with tile.TileContext(nc) as tc, ExitStack() as ctx:
        sing = ctx.enter_context(tc.tile_pool(name="sing", bufs=1))
        wpool = ctx.enter_context(tc.tile_pool(name="w", bufs=1))
        big = ctx.enter_context(tc.tile_pool(name="big", bufs=11))
        small = ctx.enter_context(tc.tile_pool(name="small", bufs=7))
        hidp = ctx.enter_context(tc.tile_pool(name="hid", bufs=2))
        statp = ctx.enter_context(tc.tile_pool(name="stat", bufs=4))
        psmm = ctx.enter_context(tc.tile_pool(name="psmm", bufs=4, space="PSUM"))
        psst = ctx.enter_context(tc.tile_pool(name="psst", bufs=2, space="PSUM"))
        psbc = ctx.enter_context(tc.tile_pool(name="psbc", bufs=2, space="PSUM"))

        ones_col = sing.tile([P, 1], bf16)
        nc.vector.memset(ones_col, 1.0)
        ones_colf = sing.tile([P, 1], f32)
        nc.vector.memset(ones_colf, 1.0)
        ones_row = sing.tile([1, P], f32)
        nc.vector.memset(ones_row, 1.0)
        eps1 = sing.tile([1, 1], f32)
        nc.vector.memset(eps1, LN_EPS)
        dw_sb = sing.tile([P, L * DT, K], f32)
        nc.sync.dma_start(out=dw_sb, in_=dwK.ap().rearrange("l (dt p) k -> p (l dt) k", p=P))
        dwb_sb = sing.tile([P, L * DT], f32)
        nc.sync.dma_start(out=dwb_sb, in_=dwb.ap().rearrange("l (dt p) -> p (l dt)", p=P))
        pwb_sb = sing.tile([P, L * DT], f32)
        nc.sync.dma_start(out=pwb_sb, in_=pwb.ap().rearrange("l (dt p) -> p (l dt)", p=P))
        b1_sb = sing.tile([P, L * HT], f32)
        nc.sync.dma_start(out=b1_sb, in_=b1v.ap().rearrange("l (ht p) -> p (l ht)", p=P))
        b2_sb = sing.tile([P, L * DT], f32)
        nc.sync.dma_start(out=b2_sb, in_=b2v.ap().rearrange("l (dt p) -> p (l dt)", p=P))
        lng_sb = sing.tile([P, (L + 1) * DT], f32)
        nc.sync.dma_start(out=lng_sb, in_=lng.ap().rearrange("l (dt p) -> p (l dt)", p=P))
        lnb_sb = sing.tile([P, (L + 1) * DT], f32)
        nc.sync.dma_start(out=lnb_sb, in_=lnb.ap().rearrange("l (dt p) -> p (l dt)", p=P))

        def load_w(kind, dram, l, shape):
            t = wpool.tile(shape, bf16, tag=kind, name=f"{kind}{l}")
            nc.sync.dma_start(out=t, in_=dram.ap()[l].rearrange("(kt p) e -> p kt e", p=P))
            return t

        def ln_st1(x_tile):
            """S-MMs + evac to SBUF."""
            S_ps = psst.tile([1, CH], f32, tag="ps_stat", name="S_ps")
            for kt in range(DT):
                nc.tensor.matmul(S_ps[:, :], ones_colf[:, :], x_tile[:, kt, :],
                                 start=(kt == 0), stop=(kt == DT - 1))
            S_sb = statp.tile([1, CH], f32, tag="stat", name="S_sb")
            nc.vector.tensor_copy(out=S_sb[:, :], in_=S_ps[:, :])
            return S_ps, S_sb

        def ln_st2(x_tile, S_ps, S_sb, slot, out_bf16):
            """broadcast mu, center in place, variance (Q at partition 32 of the
            same stat bank), rstd, broadcast (same bc bank as mu), apply."""
            bc = psbc.tile([P, CH], f32, tag="ps_bc", name="bc")
            nc.tensor.matmul(bc[:, :], ones_row[:, :], S_sb[:, :], start=True, stop=True)
            for d in range(DT):
                nc.vector.scalar_tensor_tensor(
                    x_tile[:, d, :], bc[:, :], -1.0 / D, x_tile[:, d, :], Alu.mult, Alu.add)
            xsq = small.tile([P, DT, CH], bf16, tag="small", name="xsq")
            for d in range(DT):
                nc.vector.tensor_mul(xsq[:, d, :], x_tile[:, d, :], x_tile[:, d, :])
            Q_ps = psst.tile([1, CH], f32, tag="ps_stat", name="Q_ps")
            for kt in range(DT):
                nc.tensor.matmul(Q_ps[:, :], ones_col[:, :], xsq[:, kt, :],
                                 start=(kt == 0), stop=(kt == DT - 1))
            lnv = statp.tile([1, CH], f32, tag="stat", name="lnv")
            nc.scalar.activation(out=lnv[:, :], in_=Q_ps[:, :], func=Act.Ln,
                                 bias=eps1[:, :], scale=1.0 / D)
            rstd = statp.tile([1, CH], f32, tag="stat", name="rstd")
            nc.scalar.activation(out=rstd[:, :], in_=lnv[:, :], func=Act.Exp, scale=-0.5)
            nc.tensor.matmul(bc[:, :], ones_row[:, :], rstd[:, :], start=True, stop=True)
            if out_bf16:
                a_t = small.tile([P, DT, CH], bf16, tag="small", name="a_t")
            else:
                a_t = big.tile([P, DT, CH], f32, tag="big", name="a_t")
            for d in range(DT):
                nc.vector.scalar_tensor_tensor(
                    a_t[:, d, :], x_tile[:, d, :], lng_sb[:, slot * DT + d: slot * DT + d + 1],
                    bc[:, :], Alu.mult, Alu.mult)
            if has_lnb:
                for d in range(DT):
                    nc.vector.tensor_scalar(
                        out=a_t[:, d, :], in0=a_t[:, d, :],
                        scalar1=lnb_sb[:, slot * DT + d: slot * DT + d + 1], scalar2=None,
                        op0=Alu.add)
            return a_t

        def mlp_chunk(a_t, l, w1_sb, w2_sb, out_tile, out_off):
            hid = hidp.tile([P, HT, CH], bf16, tag="hid", name="hid")
            for mt in range(HT):
                ps = psmm.tile([P, CH], f32, tag="mm", name="ps1")
                for kt in range(DT):
                    nc.tensor.matmul(ps[:, :], w1_sb[:, kt, bass.ts(mt, P)], a_t[:, kt, :],
                                     start=(kt == 0), stop=(kt == DT - 1))
                nc.scalar.activation(out=hid[:, mt, :], in_=ps[:, :], func=Act.Relu,
                                     bias=b1_sb[:, l * HT + mt: l * HT + mt + 1], scale=1.0)
            for mt in range(DT):
                ps = psmm.tile([P, CH], f32, tag="mm", name="ps2")
                for kt in range(HT):
                    nc.tensor.matmul(ps[:, :], w2_sb[:, kt, bass.ts(mt, P)], hid[:, kt, :],
                                     start=(kt == 0), stop=(kt == HT - 1))
                nc.scalar.activation(out=out_tile[:, mt, out_off: out_off + CH], in_=ps[:, :],
                                     func=Act.Identity,
                                     bias=b2_sb[:, l * DT + mt: l * DT + mt + 1], scale=1.0)

        def conv_dw(m_t, l):
            acc = big.tile([P, DT, CH], f32, tag="big", name="acc")
            y = small.tile([P, DT, CH], bf16, tag="small", name="y")
            for d in range(DT):
                nc.vector.tensor_scalar(
                    out=acc[:, d, :], in0=m_t[:, d, 0: CH],
                    scalar1=dw_sb[:, l * DT + d, 0:1], scalar2=dwb_sb[:, l * DT + d: l * DT + d + 1],
                    op0=Alu.mult, op1=Alu.add)
                for j in range(1, K - 1):
                    nc.vector.scalar_tensor_tensor(
                        acc[:, d, :], m_t[:, d, j: j + CH], dw_sb[:, l * DT + d, j: j + 1],
                        acc[:, d, :], Alu.mult, Alu.add)
                nc.vector.scalar_tensor_tensor(
                    y[:, d, :], m_t[:, d, K - 1: K - 1 + CH], dw_sb[:, l * DT + d, K - 1: K],
                    acc[:, d, :], Alu.mult, Alu.add)
            return y

        def conv_pw(y, l, pw_sb, want_bf):
            cv = big.tile([P, DT, CH], f32, tag="big", name="cv")
            cv_bf = small.tile([P, DT, CH], bf16, tag="small", name="cv_bf") if want_bf else None
            for mt in range(DT):
                ps = psmm.tile([P, CH], f32, tag="mm", name="ps3")
                for kt in range(DT):
                    nc.tensor.matmul(ps[:, :], pw_sb[:, kt, bass.ts(mt, P)], y[:, kt, :],
                                     start=(kt == 0), stop=(kt == DT - 1))
                nc.scalar.activation(out=cv[:, mt, :], in_=ps[:, :], func=Act.Identity,
                                     bias=pwb_sb[:, l * DT + mt: l * DT + mt + 1], scale=1.0)
                if want_bf:
                    nc.scalar.activation(out=cv_bf[:, mt, :], in_=ps[:, :], func=Act.Identity,
                                         bias=pwb_sb[:, l * DT + mt: l * DT + mt + 1], scale=1.0)
            return cv, cv_bf

        def conv_chunk(m_t, l, pw_sb, want_bf):
            return conv_pw(conv_dw(m_t, l), l, pw_sb, want_bf)

        def gru_chunk(rhs_bf, res_t, fw_sb, h_prev):
            """kh matmul + gates + scan + residual (in place into res_t). Returns h tile."""
            z = big.tile([P, DT, CH], f32, tag="big", name="z")
            cf = big.tile([P, DT, CH], f32, tag="big", name="cf")
            s = big.tile([P, DT, CH], f32, tag="big", name="s")
            v = big.tile([P, DT, CH], f32, tag="big", name="v")
            h = big.tile([P, DT, CH], f32, tag="big", name="h")
            for mt in range(MT2):
                ps = psmm.tile([P, CH], f32, tag="mm", name="ps4")
                for kt in range(DT):
                    nc.tensor.matmul(ps[:, :], fw_sb[:, kt, bass.ts(mt, P)], rhs_bf[:, kt, :],
                                     start=(kt == 0), stop=(kt == DT - 1))
                if mt < DT:
                    nc.scalar.activation(out=z[:, mt, :], in_=ps[:, :], func=Act.Sigmoid)
                    nc.scalar.activation(out=cf[:, mt, :], in_=ps[:, :], func=Act.Sigmoid,
                                         scale=-1.0)
                else:
                    d = mt - DT
                    nc.scalar.activation(out=s[:, d, :], in_=ps[:, :], func=Act.Sigmoid)
                    nc.vector.scalar_tensor_tensor(
                        s[:, d, :], ps[:, :], 0.5, s[:, d, :], Alu.add, Alu.max)
            for d in range(DT):
                nc.vector.tensor_mul(v[:, d, :], z[:, d, :], s[:, d, :])
            for d in range(DT):
                init = 0.5 if h_prev is None else h_prev[:, d, CH - 1: CH]
                nc.vector.tensor_tensor_scan(h[:, d, :], cf[:, d, :], v[:, d, :], init,
                                             Alu.mult, Alu.add)
            for d in range(DT):
                nc.vector.tensor_add(res_t[:, d, :], h[:, d, :], res_t[:, d, :])
            return h

        # ---------- layer 0: conv0 -> ln1 -> gru0 (+ residual on ln1 out) ----------
        # software-pipelined: stage A(c) = conv+LN (PE-heavy), stage B(c) = GRU tail
        fw_sb = load_w("fw", fwT, 0, [P, DT, E2])
        pw_sb = load_w("pw", pwT, 0, [P, DT, D])

        def l0_s0(c):
            x_in = big.tile([P, DT, CH + 3], f32, tag="big", name="x_in")
            nc.sync.dma_start(out=x_in, in_=xT.ap().rearrange("(dt p) t -> p dt t", p=P)[:, :, c * CH: c * CH + CH + 3])
            return conv_dw(x_in, 0)

        def l0_st1(c, y):
            cv, _ = conv_pw(y, 0, pw_sb, want_bf=False)
            return (cv,) + ln_st1(cv)

        def l0_st2(c, cv, stat_ps, S_sb):
            n = ln_st2(cv, stat_ps, S_sb, 0, out_bf16=False)
            n_bf = small.tile([P, DT, CH], bf16, tag="small", name="n_bf")
            for d in range(DT):
                nc.scalar.activation(out=n_bf[:, d, :], in_=n[:, d, :], func=Act.Copy)
            return n, n_bf

        state = {"h": None}

        def l0_stageB(c, n, n_bf):
            state["h"] = gru_chunk(n_bf, n, fw_sb, state["h"])
            nc.sync.dma_start(out=dram3(xs[0], c, CH), in_=n)

        q0, q1, q2 = [], [], []
        for c in range(NCH):
            q0.append((c, l0_s0(c)))
            if len(q0) > 1:
                c0, y0 = q0.pop(0)
                q1.append((c0, l0_st1(c0, y0)))
            if len(q1) > 1:
                c1, art = q1.pop(0)
                q2.append((c1, l0_st2(c1, *art)))
            if len(q2) > 1:
                c2, art = q2.pop(0)
                l0_stageB(c2, *art)
        for c0, y0 in q0:
            q1.append((c0, l0_st1(c0, y0)))
        for c1, art in q1:
            q2.append((c1, l0_st2(c1, *art)))
        for c2, art in q2:
            l0_stageB(c2, *art)

        # ---------- mid iterations i=0..2: ln2_i, mlp_i, conv_{i+1}, gru_{i+1} ----------
        for i in range(L - 1):
            src, dst = xs[i % 2], xs[(i + 1) % 2]
            w1_sb = load_w("w1", w1T, i, [P, DT, H])
            w2_sb = load_w("w2", w2T, i, [P, HT, D])
            fw_sb = load_w("fw", fwT, i + 1, [P, DT, E2])
            pw_sb = load_w("pw", pwT, i + 1, [P, DT, D])
            state["h"] = None
            m_prev = None

            def mid_st1(c):
                x_in = big.tile([P, DT, CH], f32, tag="big", name="x_in")
                nc.sync.dma_start(out=x_in, in_=dram3(src, c, CH))
                return (x_in,) + ln_st1(x_in)

            def mid_st2(c, x_in, stat_ps, S_sb):
                return ln_st2(x_in, stat_ps, S_sb, 1 + i, out_bf16=True)

            def mid_stageA(c, a, m_prev):
                m = big.tile([P, DT, CH + 3], f32, tag="big", name="m")
                mlp_chunk(a, i, w1_sb, w2_sb, m, 3)
                if c == 0:
                    nc.vector.memset(m[:, :, 0:3], 0.0)
                else:
                    nc.vector.tensor_copy(out=m[:, :, 0:3], in_=m_prev[:, :, CH: CH + 3])
                return m

            def mid_stageB1(c, m):
                return conv_dw(m, i + 1)

            def mid_stageB(c, y):
                cv, cv_bf = conv_pw(y, i + 1, pw_sb, want_bf=True)
                state["h"] = gru_chunk(cv_bf, cv, fw_sb, state["h"])
                nc.sync.dma_start(out=dram3(dst, c, CH), in_=cv)

            q1, q2, qa, qb = [], [], [], []
            for c in range(NCH):
                q1.append((c, mid_st1(c)))
                if len(q1) > 1:
                    c1, art = q1.pop(0)
                    q2.append((c1, mid_st2(c1, *art)))
                if len(q2) > 1:
                    c2, a = q2.pop(0)
                    m = mid_stageA(c2, a, m_prev)
                    m_prev = m
                    qa.append((c2, m))
                if len(qa) > 1:
                    c3, m3 = qa.pop(0)
                    qb.append((c3, mid_stageB1(c3, m3)))
                if len(qb) > 1:
                    c4, y4 = qb.pop(0)
                    mid_stageB(c4, y4)
            for c1, art in q1:
                q2.append((c1, mid_st2(c1, *art)))
            for c2, a in q2:
                m = mid_stageA(c2, a, m_prev)
                m_prev = m
                qa.append((c2, m))
            for c3, m3 in qa:
                qb.append((c3, mid_stageB1(c3, m3)))
            for c4, y4 in qb:
                mid_stageB(c4, y4)

        # ---------- tail: ln2_3 + mlp_3 ----------
        src = xs[(L - 1) % 2]
        w1_sb = load_w("w1", w1T, L - 1, [P, DT, H])
        w2_sb = load_w("w2", w2T, L - 1, [P, HT, D])
        def tail_st1(c):
            x_in = big.tile([P, DT, CH], f32, tag="big", name="x_in")
            nc.sync.dma_start(out=x_in, in_=dram3(src, c, CH))
            return (x_in,) + ln_st1(x_in)

        def tail_rest(c, x_in, stat_ps, S_sb):
            a = ln_st2(x_in, stat_ps, S_sb, L, out_bf16=True)
            o = big.tile([P, DT, CH], f32, tag="big", name="o")
            mlp_chunk(a, L - 1, w1_sb, w2_sb, o, 0)
            nc.sync.dma_start(out=dram3(out_t, c, CH), in_=o)

        q1 = []
        for c in range(NCH):
            q1.append((c, tail_st1(c)))
            if len(q1) > 1:
                c1, art = q1.pop(0)
                tail_rest(c1, *art)
        for c1, art in q1:
            tail_rest(c1, *art)

    return nc


_CACHE = {}


def get_compiled_nc(T=4096, CH=512, has_lnb=False, **kw):
    key = (T, CH, has_lnb, tuple(sorted(kw.items())))
    if key not in _CACHE:
        nc = build_nc(T, CH, has_lnb, **kw)
        nc.compile()
        _CACHE[key] = nc
    return _CACHE[key]


def make_host_inputs(inputs, T=4096):
    f = np.float32
    w = {
        "fwT": np.ascontiguousarray(np.transpose(np.asarray(inputs["f_w"], f), (0, 2, 1))).astype(BF),
        "pwT": np.ascontiguousarray(np.transpose(np.asarray(inputs["conv_pw_w"], f), (0, 2, 1))).astype(BF),
        "w1T": np.ascontiguousarray(np.transpose(np.asarray(inputs["mlp_w1"], f), (0, 2, 1))).astype(BF),
        "w2T": np.ascontiguousarray(np.transpose(np.asarray(inputs["mlp_w2"], f), (0, 2, 1))).astype(BF),
        "dwK": np.ascontiguousarray(np.transpose(np.asarray(inputs["conv_dw_w"], f), (0, 2, 1))).astype(f),
        "dwb": np.asarray(inputs["conv_dw_b"], f),
        "pwb": np.asarray(inputs["conv_pw_b"], f),
        "b1v": np.asarray(inputs["mlp_b1"], f),
        "b2v": np.asarray(inputs["mlp_b2"], f),
        "lng": np.concatenate([np.asarray(inputs["ln1_g"], f)[None], np.asarray(inputs["ln2_g"], f)], 0),
        "lnb": np.concatenate([np.asarray(inputs["ln1_b"], f)[None], np.asarray(inputs["ln2_b"], f)], 0),
    }
    x = np.asarray(inputs["x"], f)
    nb = x.shape[0]
    in_maps = []
    for b in range(nb):
        xTp = np.zeros((D, T + 3), f)
        xTp[:, 3:] = x[b, :T].T
        in_maps.append({"xT": xTp, **w})
    has_lnb = bool(np.any(w["lnb"] != 0.0))
    return in_maps, has_lnb


def kernel(**inputs):
    from concourse.bass_utils import run_bass_kernel_spmd

    T = int(np.asarray(inputs["x"]).shape[1])
    in_maps, has_lnb = make_host_inputs(inputs, T)
    nc = get_compiled_nc(T=T, has_lnb=has_lnb)
    res = run_bass_kernel_spmd(nc, in_maps, core_ids=list(range(len(in_maps))))
    out = np.stack([r["out"].T for r in res.results])
    return np.ascontiguousarray(out.astype(np.float32))


# revision 19
# speedup vs baseline: 1.4186x; 1.0252x over previous
"""Trainium2 Bass kernel for nn_BlockV2 (conv -> LN -> minGRU -> MLP x4).

Strategy: data-parallel over batch (B=8 -> 8 cores). Per core, activations
are kept in [D_partitions, T_free] layout and streamed through each layer in
chunks of 512 tokens; inter-layer activations ping-pong through DRAM.
The minGRU recurrence h_t = c_t*h_{t-1} + v_t runs on the VectorE
tensor_tensor_scan instruction (fp32 state), chained across chunks.
Matmul inputs are bf16 (fp32 PSUM accumulate); everything on the
LN/scan/residual path stays fp32 (the late-layer signal is a ~5e-3
variation on an O(1) baseline, which bf16 storage would destroy).
LayerNorm is two-pass (center, then variance of centered values) to avoid
E[x^2]-mu^2 cancellation. Emission is software-pipelined: chunk c+1's
LN/MLP matmuls interleave with chunk c's conv/GRU tail so TensorE never
idles long enough to re-throttle (HAM).
"""
import sys

sys.path.insert(0, "/opt/trn_rl_repo")

from contextlib import ExitStack

import numpy as np
import ml_dtypes

import concourse.bass as bass
import concourse.tile as tile
from concourse import bacc, mybir

f32 = mybir.dt.float32
bf16 = mybir.dt.bfloat16
Alu = mybir.AluOpType
Act = mybir.ActivationFunctionType
BF = ml_dtypes.bfloat16

B, D, L, K, H = 8, 512, 4, 4, 2048
N_CORES = 8
LN_EPS = 1e-5
P = 128


def build_nc(T=4096, CH=512, has_lnb=False, stats_fp32=True, q_fp32=False,
             rstd_recip=False):
    NCH = T // CH
    DT = D // P      # 4 d-tiles
    HT = H // P      # 16 h-tiles
    E2 = 2 * D
    MT2 = E2 // P    # 8 m-tiles of the kh matmul

    nc = bacc.Bacc("TRN2", target_bir_lowering=False, debug=False)

    xT = nc.dram_tensor("xT", [D, T + 3], f32, kind="ExternalInput")
    fwT = nc.dram_tensor("fwT", [L, D, E2], bf16, kind="ExternalInput")
    pwT = nc.dram_tensor("pwT", [L, D, D], bf16, kind="ExternalInput")
    w1T = nc.dram_tensor("w1T", [L, D, H], bf16, kind="ExternalInput")
    w2T = nc.dram_tensor("w2T", [L, H, D], bf16, kind="ExternalInput")
    dwK = nc.dram_tensor("dwK", [L, D, K], f32, kind="ExternalInput")
    dwb = nc.dram_tensor("dwb", [L, D], f32, kind="ExternalInput")
    pwb = nc.dram_tensor("pwb", [L, D], f32, kind="ExternalInput")
    b1v = nc.dram_tensor("b1v", [L, H], f32, kind="ExternalInput")
    b2v = nc.dram_tensor("b2v", [L, D], f32, kind="ExternalInput")
    lng = nc.dram_tensor("lng", [L + 1, D], f32, kind="ExternalInput")
    lnb = nc.dram_tensor("lnb", [L + 1, D], f32, kind="ExternalInput")
    out_t = nc.dram_tensor("out", [D, T], f32, kind="ExternalOutput")
    xs = [nc.dram_tensor(f"xs{i}", [D, T], f32) for i in range(2)]

    def dram3(tensor, c, width):
        return tensor.ap().rearrange("(dt p) t -> p dt t", p=P)[:, :, c * CH: c * CH + width]

    with tile.TileContext(nc) as tc, ExitStack() as ctx:
        sing = ctx.enter_context(tc.tile_pool(name="sing", bufs=1))
        wpool = ctx.enter_context(tc.tile_pool(name="w", bufs=1))
        big = ctx.enter_context(tc.tile_pool(name="big", bufs=11))
        small = ctx.enter_context(tc.tile_pool(name="small", bufs=7))
        hidp = ctx.enter_context(tc.tile_pool(name="hid", bufs=2))
        statp = ctx.enter_context(tc.tile_pool(name="stat", bufs=4))
        psmm = ctx.enter_context(tc.tile_pool(name="psmm", bufs=4, space="PSUM"))
        psst = ctx.enter_context(tc.tile_pool(name="psst", bufs=2, space="PSUM"))
        psbc = ctx.enter_context(tc.tile_pool(name="psbc", bufs=2, space="PSUM"))

        ones_col = sing.tile([P, 1], bf16)
        nc.vector.memset(ones_col, 1.0)
        ones_colf = sing.tile([P, 1], f32)
        nc.vector.memset(ones_colf, 1.0)
        ones_row = sing.tile([1, P], f32)
        nc.vector.memset(ones_row, 1.0)
        ones_row_bf = sing.tile([1, P], bf16)
        nc.vector.memset(ones_row_bf, 1.0)
        eps1 = sing.tile([1, 1], f32)
        nc.vector.memset(eps1, LN_EPS)
        dw_sb = sing.tile([P, L * DT, K], f32)
        nc.sync.dma_start(out=dw_sb, in_=dwK.ap().rearrange("l (dt p) k -> p (l dt) k", p=P))
        dwb_sb = sing.tile([P, L * DT], f32)
        nc.sync.dma_start(out=dwb_sb, in_=dwb.ap().rearrange("l (dt p) -> p (l dt)", p=P))
        pwb_sb = sing.tile([P, L * DT], f32)
        nc.sync.dma_start(out=pwb_sb, in_=pwb.ap().rearrange("l (dt p) -> p (l dt)", p=P))
        b1_sb = sing.tile([P, L * HT], f32)
        nc.sync.dma_start(out=b1_sb, in_=b1v.ap().rearrange("l (ht p) -> p (l ht)", p=P))
        b2_sb = sing.tile([P, L * DT], f32)
        nc.sync.dma_start(out=b2_sb, in_=b2v.ap().rearrange("l (dt p) -> p (l dt)", p=P))
        lng_sb = sing.tile([P, (L + 1) * DT], f32)
        nc.sync.dma_start(out=lng_sb, in_=lng.ap().rearrange("l (dt p) -> p (l dt)", p=P))
        lnb_sb = sing.tile([P, (L + 1) * DT], f32)
        nc.sync.dma_start(out=lnb_sb, in_=lnb.ap().rearrange("l (dt p) -> p (l dt)", p=P))

        def load_w(kind, dram, l, shape):
            t = wpool.tile(shape, bf16, tag=kind, name=f"{kind}{l}")
            nc.sync.dma_start(out=t, in_=dram.ap()[l].rearrange("(kt p) e -> p kt e", p=P))
            return t

        def ln_st1(x_tile):
            """S-MMs + evac to SBUF."""
            S_ps = psst.tile([1, CH], f32, tag="ps_stat", name="S_ps")
            for kt in range(DT):
                nc.tensor.matmul(S_ps[:, :], ones_colf[:, :], x_tile[:, kt, :],
                                 start=(kt == 0), stop=(kt == DT - 1))
            S_sb = statp.tile([1, CH], f32, tag="stat", name="S_sb")
            nc.vector.tensor_copy(out=S_sb[:, :], in_=S_ps[:, :])
            return S_ps, S_sb

        def ln_st2(x_tile, S_ps, S_sb, slot, out_bf16):
            """broadcast mu, center in place, variance (Q at partition 32 of the
            same stat bank), rstd, broadcast (same bc bank as mu), apply."""
            bc = psbc.tile([P, CH], f32, tag="ps_bc", name="bc")
            nc.tensor.matmul(bc[:, :], ones_row[:, :], S_sb[:, :], start=True, stop=True)
            for d in range(DT):
                nc.vector.scalar_tensor_tensor(
                    x_tile[:, d, :], bc[:, :], -1.0 / D, x_tile[:, d, :], Alu.mult, Alu.add)
            xsq = small.tile([P, DT, CH], bf16, tag="small", name="xsq")
            for d in range(DT):
                nc.vector.tensor_mul(xsq[:, d, :], x_tile[:, d, :], x_tile[:, d, :])
            Q_ps = psst.tile([1, CH], f32, tag="ps_stat", name="Q_ps")
            for kt in range(DT):
                nc.tensor.matmul(Q_ps[:, :], ones_col[:, :], xsq[:, kt, :],
                                 start=(kt == 0), stop=(kt == DT - 1))
            lnv = statp.tile([1, CH], f32, tag="stat", name="lnv")
            nc.scalar.activation(out=lnv[:, :], in_=Q_ps[:, :], func=Act.Ln,
                                 bias=eps1[:, :], scale=1.0 / D)
            rstd = statp.tile([1, CH], bf16, tag="stat", name="rstd")
            nc.scalar.activation(out=rstd[:, :], in_=lnv[:, :], func=Act.Exp, scale=-0.5)
            nc.tensor.matmul(bc[:, :], ones_row_bf[:, :], rstd[:, :], start=True, stop=True)
            if out_bf16:
                a_t = small.tile([P, DT, CH], bf16, tag="small", name="a_t")
            else:
                a_t = big.tile([P, DT, CH], f32, tag="big", name="a_t")
            for d in range(DT):
                nc.vector.scalar_tensor_tensor(
                    a_t[:, d, :], x_tile[:, d, :], lng_sb[:, slot * DT + d: slot * DT + d + 1],
                    bc[:, :], Alu.mult, Alu.mult)
            if has_lnb:
                for d in range(DT):
                    nc.vector.tensor_scalar(
                        out=a_t[:, d, :], in0=a_t[:, d, :],
                        scalar1=lnb_sb[:, slot * DT + d: slot * DT + d + 1], scalar2=None,
                        op0=Alu.add)
            return a_t

        def mlp_chunk(a_t, l, w1_sb, w2_sb, out_tile, out_off):
            hid = hidp.tile([P, HT, CH], bf16, tag="hid", name="hid")
            for mt in range(HT):
                ps = psmm.tile([P, CH], f32, tag="mm", name="ps1")
                for kt in range(DT):
                    nc.tensor.matmul(ps[:, :], w1_sb[:, kt, bass.ts(mt, P)], a_t[:, kt, :],
                                     start=(kt == 0), stop=(kt == DT - 1))
                nc.scalar.activation(out=hid[:, mt, :], in_=ps[:, :], func=Act.Relu,
                                     bias=b1_sb[:, l * HT + mt: l * HT + mt + 1], scale=1.0)
            for mt in range(DT):
                ps = psmm.tile([P, CH], f32, tag="mm", name="ps2")
                for kt in range(HT):
                    nc.tensor.matmul(ps[:, :], w2_sb[:, kt, bass.ts(mt, P)], hid[:, kt, :],
                                     start=(kt == 0), stop=(kt == HT - 1))
                nc.scalar.activation(out=out_tile[:, mt, out_off: out_off + CH], in_=ps[:, :],
                                     func=Act.Identity,
                                     bias=b2_sb[:, l * DT + mt: l * DT + mt + 1], scale=1.0)

        def conv_dw(m_t, l):
            acc = big.tile([P, DT, CH], f32, tag="big", name="acc")
            y = small.tile([P, DT, CH], bf16, tag="small", name="y")
            for d in range(DT):
                nc.vector.tensor_scalar(
                    out=acc[:, d, :], in0=m_t[:, d, 0: CH],
                    scalar1=dw_sb[:, l * DT + d, 0:1], scalar2=dwb_sb[:, l * DT + d: l * DT + d + 1],
                    op0=Alu.mult, op1=Alu.add)
                for j in range(1, K - 1):
                    nc.vector.scalar_tensor_tensor(
                        acc[:, d, :], m_t[:, d, j: j + CH], dw_sb[:, l * DT + d, j: j + 1],
                        acc[:, d, :], Alu.mult, Alu.add)
                nc.vector.scalar_tensor_tensor(
                    y[:, d, :], m_t[:, d, K - 1: K - 1 + CH], dw_sb[:, l * DT + d, K - 1: K],
                    acc[:, d, :], Alu.mult, Alu.add)
            return y

        def conv_pw(y, l, pw_sb, want_bf):
            cv = big.tile([P, DT, CH], f32, tag="big", name="cv")
            cv_bf = small.tile([P, DT, CH], bf16, tag="small", name="cv_bf") if want_bf else None
            for mt in range(DT):
                ps = psmm.tile([P, CH], f32, tag="mm", name="ps3")
                for kt in range(DT):
                    nc.tensor.matmul(ps[:, :], pw_sb[:, kt, bass.ts(mt, P)], y[:, kt, :],
                                     start=(kt == 0), stop=(kt == DT - 1))
                nc.scalar.activation(out=cv[:, mt, :], in_=ps[:, :], func=Act.Identity,
                                     bias=pwb_sb[:, l * DT + mt: l * DT + mt + 1], scale=1.0)
                if want_bf:
                    nc.scalar.activation(out=cv_bf[:, mt, :], in_=ps[:, :], func=Act.Identity,
                                         bias=pwb_sb[:, l * DT + mt: l * DT + mt + 1], scale=1.0)
            return cv, cv_bf

        def conv_chunk(m_t, l, pw_sb, want_bf):
            return conv_pw(conv_dw(m_t, l), l, pw_sb, want_bf)

        def gru_chunk(rhs_bf, res_t, fw_sb, h_prev):
            """kh matmul + gates + scan + residual (in place into res_t). Returns h tile."""
            z = big.tile([P, DT, CH], f32, tag="big", name="z")
            cf = big.tile([P, DT, CH], f32, tag="big", name="cf")
            s = big.tile([P, DT, CH], f32, tag="big", name="s")
            v = big.tile([P, DT, CH], f32, tag="big", name="v")
            h = big.tile([P, DT, CH], f32, tag="big", name="h")
            for mt in range(MT2):
                ps = psmm.tile([P, CH], f32, tag="mm", name="ps4")
                for kt in range(DT):
                    nc.tensor.matmul(ps[:, :], fw_sb[:, kt, bass.ts(mt, P)], rhs_bf[:, kt, :],
                                     start=(kt == 0), stop=(kt == DT - 1))
                if mt < DT:
                    nc.scalar.activation(out=z[:, mt, :], in_=ps[:, :], func=Act.Sigmoid)
                    nc.scalar.activation(out=cf[:, mt, :], in_=ps[:, :], func=Act.Sigmoid,
                                         scale=-1.0)
                else:
                    d = mt - DT
                    nc.scalar.activation(out=s[:, d, :], in_=ps[:, :], func=Act.Sigmoid)
                    nc.vector.scalar_tensor_tensor(
                        s[:, d, :], ps[:, :], 0.5, s[:, d, :], Alu.add, Alu.max)
            for d in range(DT):
                nc.vector.tensor_mul(v[:, d, :], z[:, d, :], s[:, d, :])
            for d in range(DT):
                init = 0.5 if h_prev is None else h_prev[:, d, CH - 1: CH]
                nc.vector.tensor_tensor_scan(h[:, d, :], cf[:, d, :], v[:, d, :], init,
                                             Alu.mult, Alu.add)
            for d in range(DT):
                nc.vector.tensor_add(res_t[:, d, :], h[:, d, :], res_t[:, d, :])
            return h

        # ---------- layer 0: conv0 -> ln1 -> gru0 (+ residual on ln1 out) ----------
        # software-pipelined: stage A(c) = conv+LN (PE-heavy), stage B(c) = GRU tail
        fw_sb = load_w("fw", fwT, 0, [P, DT, E2])
        pw_sb = load_w("pw", pwT, 0, [P, DT, D])

        def l0_s0(c):
            x_in = big.tile([P, DT, CH + 3], f32, tag="big", name="x_in")
            nc.sync.dma_start(out=x_in, in_=xT.ap().rearrange("(dt p) t -> p dt t", p=P)[:, :, c * CH: c * CH + CH + 3])
            return conv_dw(x_in, 0)

        def l0_st1(c, y):
            cv, _ = conv_pw(y, 0, pw_sb, want_bf=False)
            return (cv,) + ln_st1(cv)

        def l0_st2(c, cv, stat_ps, S_sb):
            n = ln_st2(cv, stat_ps, S_sb, 0, out_bf16=False)
            n_bf = small.tile([P, DT, CH], bf16, tag="small", name="n_bf")
            for d in range(DT):
                nc.scalar.activation(out=n_bf[:, d, :], in_=n[:, d, :], func=Act.Copy)
            return n, n_bf

        state = {"h": None}

        def l0_stageB(c, n, n_bf):
            state["h"] = gru_chunk(n_bf, n, fw_sb, state["h"])
            nc.sync.dma_start(out=dram3(xs[0], c, CH), in_=n)

        q0, q1, q2 = [], [], []
        for c in range(NCH):
            q0.append((c, l0_s0(c)))
            if len(q0) > 2:
                c0, y0 = q0.pop(0)
                q1.append((c0, l0_st1(c0, y0)))
            if len(q1) > 1:
                c1, art = q1.pop(0)
                q2.append((c1, l0_st2(c1, *art)))
            if len(q2) > 1:
                c2, art = q2.pop(0)
                l0_stageB(c2, *art)
        for c0, y0 in q0:
            q1.append((c0, l0_st1(c0, y0)))
        for c1, art in q1:
            q2.append((c1, l0_st2(c1, *art)))
        for c2, art in q2:
            l0_stageB(c2, *art)

        # ---------- mid iterations i=0..2: ln2_i, mlp_i, conv_{i+1}, gru_{i+1} ----------
        for i in range(L - 1):
            src, dst = xs[i % 2], xs[(i + 1) % 2]
            w1_sb = load_w("w1", w1T, i, [P, DT, H])
            w2_sb = load_w("w2", w2T, i, [P, HT, D])
            fw_sb = load_w("fw", fwT, i + 1, [P, DT, E2])
            pw_sb = load_w("pw", pwT, i + 1, [P, DT, D])
            state["h"] = None
            m_prev = None

            def mid_st1(c):
                x_in = big.tile([P, DT, CH], f32, tag="big", name="x_in")
                nc.sync.dma_start(out=x_in, in_=dram3(src, c, CH))
                return (x_in,) + ln_st1(x_in)

            def mid_st2(c, x_in, stat_ps, S_sb):
                return ln_st2(x_in, stat_ps, S_sb, 1 + i, out_bf16=True)

            def mid_stageA(c, a, m_prev):
                m = big.tile([P, DT, CH + 3], f32, tag="big", name="m")
                mlp_chunk(a, i, w1_sb, w2_sb, m, 3)
                if c == 0:
                    nc.vector.memset(m[:, :, 0:3], 0.0)
                else:
                    nc.vector.tensor_copy(out=m[:, :, 0:3], in_=m_prev[:, :, CH: CH + 3])
                return m

            def mid_stageB1(c, m):
                return conv_dw(m, i + 1)

            def mid_stageB(c, y):
                cv, cv_bf = conv_pw(y, i + 1, pw_sb, want_bf=True)
                state["h"] = gru_chunk(cv_bf, cv, fw_sb, state["h"])
                nc.sync.dma_start(out=dram3(dst, c, CH), in_=cv)

            q1, q2, qa, qb = [], [], [], []
            for c in range(NCH):
                q1.append((c, mid_st1(c)))
                if len(q1) > 1:
                    c1, art = q1.pop(0)
                    q2.append((c1, mid_st2(c1, *art)))
                if len(q2) > 1:
                    c2, a = q2.pop(0)
                    m = mid_stageA(c2, a, m_prev)
                    m_prev = m
                    qa.append((c2, m))
                if len(qa) > 1:
                    c3, m3 = qa.pop(0)
                    qb.append((c3, mid_stageB1(c3, m3)))
                if len(qb) > 1:
                    c4, y4 = qb.pop(0)
                    mid_stageB(c4, y4)
            for c1, art in q1:
                q2.append((c1, mid_st2(c1, *art)))
            for c2, a in q2:
                m = mid_stageA(c2, a, m_prev)
                m_prev = m
                qa.append((c2, m))
            for c3, m3 in qa:
                qb.append((c3, mid_stageB1(c3, m3)))
            for c4, y4 in qb:
                mid_stageB(c4, y4)

        # ---------- tail: ln2_3 + mlp_3 ----------
        src = xs[(L - 1) % 2]
        w1_sb = load_w("w1", w1T, L - 1, [P, DT, H])
        w2_sb = load_w("w2", w2T, L - 1, [P, HT, D])
        def tail_st1(c):
            x_in = big.tile([P, DT, CH], f32, tag="big", name="x_in")
            nc.sync.dma_start(out=x_in, in_=dram3(src, c, CH))
            return (x_in,) + ln_st1(x_in)

        def tail_rest(c, x_in, stat_ps, S_sb):
            a = ln_st2(x_in, stat_ps, S_sb, L, out_bf16=True)
            o = big.tile([P, DT, CH], f32, tag="big", name="o")
            mlp_chunk(a, L - 1, w1_sb, w2_sb, o, 0)
            nc.sync.dma_start(out=dram3(out_t, c, CH), in_=o)

        q1 = []
        for c in range(NCH):
            q1.append((c, tail_st1(c)))
            if len(q1) > 1:
                c1, art = q1.pop(0)
                tail_rest(c1, *art)
        for c1, art in q1:
            tail_rest(c1, *art)

    return nc


_CACHE = {}


def get_compiled_nc(T=4096, CH=512, has_lnb=False, **kw):
    key = (T, CH, has_lnb, tuple(sorted(kw.items())))
    if key not in _CACHE:
        nc = build_nc(T, CH, has_lnb, **kw)
        nc.compile()
        _CACHE[key] = nc
    return _CACHE[key]


def make_host_inputs(inputs, T=4096):
    f = np.float32
    w = {
        "fwT": np.ascontiguousarray(np.transpose(np.asarray(inputs["f_w"], f), (0, 2, 1))).astype(BF),
        "pwT": np.ascontiguousarray(np.transpose(np.asarray(inputs["conv_pw_w"], f), (0, 2, 1))).astype(BF),
        "w1T": np.ascontiguousarray(np.transpose(np.asarray(inputs["mlp_w1"], f), (0, 2, 1))).astype(BF),
        "w2T": np.ascontiguousarray(np.transpose(np.asarray(inputs["mlp_w2"], f), (0, 2, 1))).astype(BF),
        "dwK": np.ascontiguousarray(np.transpose(np.asarray(inputs["conv_dw_w"], f), (0, 2, 1))).astype(f),
        "dwb": np.asarray(inputs["conv_dw_b"], f),
        "pwb": np.asarray(inputs["conv_pw_b"], f),
        "b1v": np.asarray(inputs["mlp_b1"], f),
        "b2v": np.asarray(inputs["mlp_b2"], f),
        "lng": np.concatenate([np.asarray(inputs["ln1_g"], f)[None], np.asarray(inputs["ln2_g"], f)], 0),
        "lnb": np.concatenate([np.asarray(inputs["ln1_b"], f)[None], np.asarray(inputs["ln2_b"], f)], 0),
    }
    x = np.asarray(inputs["x"], f)
    nb = x.shape[0]
    in_maps = []
    for b in range(nb):
        xTp = np.zeros((D, T + 3), f)
        xTp[:, 3:] = x[b, :T].T
        in_maps.append({"xT": xTp, **w})
    has_lnb = bool(np.any(w["lnb"] != 0.0))
    return in_maps, has_lnb


def kernel(**inputs):
    from concourse.bass_utils import run_bass_kernel_spmd

    T = int(np.asarray(inputs["x"]).shape[1])
    in_maps, has_lnb = make_host_inputs(inputs, T)
    nc = get_compiled_nc(T=T, has_lnb=has_lnb)
    res = run_bass_kernel_spmd(nc, in_maps, core_ids=list(range(len(in_maps))))
    out = np.stack([r["out"].T for r in res.results])
    return np.ascontiguousarray(out.astype(np.float32))
